# revision 22
# baseline (speedup 1.0000x reference)
"""DeepseekV2 decoder layer (MLA attention + SwiGLU MLP) on 8 TRN2 NeuronCores.

Sharding: core c -> batch b = c//4, query tiles {j, j+4, j+8, j+12} (j = c%4,
128 rows each, causally interleaved). The interleave makes the causal
structure identical on every core: key tile kt is only needed by query slots
s >= kt//4, so the SPMD program skips 37.5% of score/AV work uniformly.

fp8 (e4m3, DoubleRow 2x matmul) is used for the hidden-state input, the
q_a/kv_a projections, the packed q/k score tiles, v, and the attention
probabilities; q_b/kv_b, o_proj and the FFN stay bf16 (fp8 there blows the
2e-2 error budget). fp8 weights are scaled x16 on the host to escape
denormals; the 1/16 compensation is folded into the rsqrt stat scales
(ws64 variants), so no extra ops are spent.

Per-head packed layout for scores: k/q tiles [128, 2, n] where pair slot 0 =
nope (128 feats), slot 1 = rope (64 feats) + 64 zero rows, so one DoubleRow
matmul computes a whole 192-dim score tile.
"""

import json

import numpy as np
import ml_dtypes

B, S, H = 2, 2048, 2048
NH = 16
Q_LORA = 1536
KV_LORA = 512
NOPE = 128
ROPE = 64
QHD = NOPE + ROPE  # 192
VHD = 128
FF = 8192
EPS = 1e-6
P = 128
QR = 512  # query rows per core
TK = S // P  # 16 key tiles
KI_H = H // P  # 16
KI_QL = Q_LORA // P  # 12
KI_KVL = KV_LORA // P  # 4
NF_FF = FF // P  # 64
ATTN_SCALE = QHD ** -0.5
WS = 16.0  # fp8 weight scale (16x: escapes denormals; keeps 16x-scaled k_pe/v tails far below fp8's 240 max)

BF16 = ml_dtypes.bfloat16
F8 = ml_dtypes.float8_e4m3

_COMPILED = {}


# ---------------------------------------------------------------------------
# compiler workaround: this container's walrus rejects >1 sem wait per
# instruction; split extra waits onto single-wait NoOps.
# ---------------------------------------------------------------------------
def _install_multiwait_fix(bass):
    if getattr(bass.Bass, "_multiwait_fix_installed", False):
        return
    orig = bass.Bass.to_json_bytes

    def _split(m):
        for f in m.get("functions", []):
            for b in f.get("blocks", []):
                out = []
                for inst in b.get("instructions", []):
                    si = inst.get("sync_info") or {}
                    waits = si.get("on_wait") or []
                    if len(waits) > 1:
                        for k, w in enumerate(waits[:-1]):
                            out.append(
                                {
                                    "debug": inst.get("debug", 0),
                                    "engine": inst["engine"],
                                    "ins": [],
                                    "name": f"{inst['name']}_w{k}",
                                    "opcode": "NoOp",
                                    "outs": [],
                                    "sync_info": {"on_update": [], "on_wait": [w]},
                                }
                            )
                        si["on_wait"] = [waits[-1]]
                    out.append(inst)
                b["instructions"] = out
        return m

    def patched(self):
        raw = orig(self)
        try:
            return json.dumps(_split(json.loads(raw))).encode()
        except Exception:
            return raw

    bass.Bass.to_json_bytes = patched
    bass.Bass._multiwait_fix_installed = True


def _install_drain_fix(tile, ScopedClock, VectorClock):
    if getattr(tile.TileContext, "_drain_fix_installed", False):
        return

    def _drain_and_barrier(self, tick_clock, wait_clock):
        gc = tick_clock.global_clock
        n = len(gc)
        for p in range(n):
            t = gc[p]
            if t > 0:
                vc = VectorClock([0] * n)
                vc.require_at_least(p, t)
                d = self.nc.sync.drain()
                wait_clock.add_sem_waits(d.ins, ScopedClock({None: vc}))
        self.nc.all_engine_barrier()
        popped = self.nc._tile_sem_poison_stack.pop()
        assert popped is self._sem_poison
        self.nc.clear_and_free_semaphores(list(self.sems.allocated().values()))
        self.nc.all_engine_barrier()

    tile.TileContext._drain_and_barrier = _drain_and_barrier
    tile.TileContext._drain_fix_installed = True


# ---------------------------------------------------------------------------
# device program
# ---------------------------------------------------------------------------
def _build_nc():
    import concourse.bass as bass
    import concourse.mybir as mybir
    import concourse.tile as tile
    from concourse.vector_clock import ScopedClock, VectorClock

    _install_multiwait_fix(bass)
    _install_drain_fix(tile, ScopedClock, VectorClock)

    dt = mybir.dt
    AF = mybir.ActivationFunctionType
    MUL = mybir.AluOpType.mult
    ADD = mybir.AluOpType.add
    SUB = mybir.AluOpType.subtract
    DRM = mybir.MatmulPerfMode.DoubleRow

    nc = bass.Bass()

    # register EPS so float bias=EPS works on the scalar engine
    _eps_t = nc.alloc_sbuf_tensor(f"const-float32-{EPS}", [128, 1], dt.float32)
    nc.gpsimd.memset(_eps_t.ap(), EPS)
    nc.const_aps.aps[(dt.float32, EPS)] = _eps_t.ap()
    EPS64 = EPS * WS * WS
    _eps64_t = nc.alloc_sbuf_tensor(f"const-float32-{EPS64}", [128, 1], dt.float32)
    nc.gpsimd.memset(_eps64_t.ap(), EPS64)
    nc.const_aps.aps[(dt.float32, EPS64)] = _eps64_t.ap()
    import math
    NLNWS = -math.log(WS)
    _nlnws_t = nc.alloc_sbuf_tensor(f"const-float32-{NLNWS}", [128, 1], dt.float32)
    nc.gpsimd.memset(_nlnws_t.ap(), NLNWS)
    nc.const_aps.aps[(dt.float32, NLNWS)] = _nlnws_t.ap()
    nc.all_engine_barrier()

    # ---- inputs ----
    hTb = nc.dram_tensor("hTb", [H, S], dt.float8e4, kind="ExternalInput")
    hTqb = nc.dram_tensor("hTqb", [H, QR], dt.float8e4, kind="ExternalInput")
    hTq = nc.dram_tensor("hTq", [H, QR], dt.float32, kind="ExternalInput")
    cosT = nc.dram_tensor("cosT", [32, S], dt.float32, kind="ExternalInput")
    sinT = nc.dram_tensor("sinT", [32, S], dt.float32, kind="ExternalInput")
    cosTq = nc.dram_tensor("cosTq", [32, QR], dt.float32, kind="ExternalInput")
    sinTq = nc.dram_tensor("sinTq", [32, QR], dt.float32, kind="ExternalInput")
    masks = nc.dram_tensor("masks", [P, TK, P], dt.float8e4, kind="ExternalInput")
    w_qa = nc.dram_tensor("w_qa", [KI_QL, P, KI_H, P], dt.float8e4, kind="ExternalInput")
    w_qb = nc.dram_tensor("w_qb", [NH, P, KI_QL, QHD], dt.bfloat16, kind="ExternalInput")
    w_kva = nc.dram_tensor("w_kva", [P, KI_H, KV_LORA + ROPE], dt.float8e4, kind="ExternalInput")
    w_kv_k = nc.dram_tensor("w_kv_k", [NH // 4, P, KI_KVL, 512], dt.bfloat16, kind="ExternalInput")
    w_kv_v = nc.dram_tensor("w_kv_v", [NH // 4, P, KI_KVL, 512], dt.bfloat16, kind="ExternalInput")
    w_o = nc.dram_tensor("w_o", [KI_H, P, NH, VHD], dt.bfloat16, kind="ExternalInput")
    w_g = nc.dram_tensor("w_g", [NF_FF, P, KI_H, P], dt.bfloat16, kind="ExternalInput")
    w_u = nc.dram_tensor("w_u", [NF_FF, P, KI_H, P], dt.bfloat16, kind="ExternalInput")
    w_d = nc.dram_tensor("w_d", [KI_H, P, NF_FF, P], dt.bfloat16, kind="ExternalInput")
    out = nc.dram_tensor("out", [H, QR], dt.float32, kind="ExternalOutput")

    import contextlib

    with tile.TileContext(nc) as tc, contextlib.ExitStack() as top:
        tp = lambda **kw: top.enter_context(tc.tile_pool(**kw))
        ones = tp(name="ones", bufs=1)
        tmp = tp(name="tmp", bufs=3)
        ld = tp(name="ld", bufs=3)
        ps = tp(name="ps", bufs=3, space="PSUM")
        attn_pool = tp(name="attn_pool", bufs=1)
        attn = attn_pool.tile([P, NH, QR], dt.bfloat16)
        maskp = tp(name="maskp", bufs=1)
        maskt = maskp.tile([P, TK, P], dt.float8e4)
        wo_pool = tp(name="wo_pool", bufs=2)

        ones_bfP = ones.tile([P, P], dt.bfloat16)
        nc.vector.memset(ones_bfP[:], 1.0)
        ones8 = ones.tile([P, 2, P], dt.float8e4)
        nc.vector.memset(ones8[:], 1.0)

        def dr(o, l, r, start, stop):
            nc.tensor.matmul(o, l, r, start=start, stop=stop, perf_mode=DRM)

        def rsqrt_into(acc, denom, dst, ws64=False):
            # dst = 1/sqrt(mean + eps) (optionally a further 1/WS) computed as
            # exp(-0.5*ln(.)) -- two scalar-engine table ops, avoiding the
            # 3.3us DVE reciprocal that would serialize the vector queue.
            lnt = tmp.tile([P, acc.shape[-1]], dt.float32, tag="h1t", bufs=2)
            nc.scalar.activation(
                out=lnt[:], in_=acc[:], func=AF.Ln, bias=EPS, scale=1.0 / denom
            )
            if ws64:
                nc.scalar.activation(out=dst[:], in_=lnt[:], func=AF.Exp, scale=-0.5, bias=NLNWS)
            else:
                nc.scalar.activation(out=dst[:], in_=lnt[:], func=AF.Exp, scale=-0.5)

        def rsqrt_tmp(acc, denom, tag, ws64=False):
            d = tmp.tile([P, acc.shape[-1]], dt.float32, tag=tag, bufs=2)
            rsqrt_into(acc, denom, d, ws64)
            return d

        with contextlib.ExitStack() as mid:
            lat = mid.enter_context(tc.tile_pool(name="lat", bufs=1))
            ckv = lat.tile([P, KI_KVL, S], dt.bfloat16)  # normalized kv latents (1x, bf16)
            kpe = lat.tile([ROPE, S], dt.float8e4)  # roped shared key-pe (1x)
            pA = mid.enter_context(tc.tile_pool(name="pA", bufs=1))
            xqbf = pA.tile([P, KI_H, QR], dt.float8e4)
            s1qrep64 = pA.tile([P, QR], dt.float32)
            qpk_pool = mid.enter_context(tc.tile_pool(name="qpk", bufs=1))
            qpk = qpk_pool.tile([P, NH, 2, QR], dt.float8e4)
            wq_pool = mid.enter_context(tc.tile_pool(name="wq", bufs=1))

            wq_cache = {}

            def wqa_tile(nf):
                if nf not in wq_cache:
                    t = wq_pool.tile([P, KI_H, P], dt.float8e4, tag="wqa", bufs=2)
                    nc.sync.dma_start(t[:], w_qa[nf])
                    wq_cache[nf] = t
                return wq_cache.pop(nf)

            # ==== phase A: ln1 stats + kv latents + shared roped k_pe ====
            with tc.tile_pool(name="pB", bufs=1) as pB, tc.tile_pool(
                name="psA", bufs=2, space="PSUM"
            ) as psA:
                nc.sync.dma_start(maskt[:], masks[:])
                for ki in range(KI_H):
                    nc.sync.dma_start(xqbf[:, ki, :], hTqb[ki * P : (ki + 1) * P, :])
                wkva = pB.tile([P, KI_H, KV_LORA + ROPE], dt.float8e4)
                nc.sync.dma_start(wkva[:], w_kva[:])
                cosb = pB.tile([32, S], dt.float32)
                sinb = pB.tile([32, S], dt.float32)
                nc.sync.dma_start(cosb[:], cosT[:])
                nc.sync.dma_start(sinb[:], sinT[:])
                # prefetch first q_a weights for phase B
                wqa_pre = wq_pool.tile([P, KI_H, P], dt.float8e4, tag="wqa", bufs=2)
                nc.sync.dma_start(wqa_pre[:], w_qa[0])
                wq_cache[0] = wqa_pre

                # chunk pipeline: squares for chunk t+1 run on the SCALAR
                # engine while the PE does chunk t's kv matmuls and the DVE
                # does chunk t's PSUM drains; acc DRs for t+1 are emitted after
                # kv work so the PE never waits on the square/stat chain.
                xcs = {}
                s1s = {}
                sq_tiles = {}

                def xc_dma(t):
                    tsl = slice(t * 512, (t + 1) * 512)
                    xc = pB.tile([P, KI_H, 512], dt.float8e4, tag="xc", bufs=3, name="xc")
                    for ki in range(KI_H):
                        nc.sync.dma_start(xc[:, ki, :], hTb[ki * P : (ki + 1) * P, tsl])
                    xcs[t] = xc

                def sq_emit(t):
                    xc = xcs[t]
                    tiles = []
                    for kp_ in range(KI_H // 2):
                        sqp = tmp.tile([P, 2, 512], dt.float8e4, tag="sqp", bufs=8, name="sqp")
                        for i in (0, 1):
                            ki = 2 * kp_ + i
                            if kp_ % 2 == 0:
                                nc.scalar.activation(
                                    out=sqp[:, i, :], in_=xc[:, ki, :], func=AF.Square
                                )
                            else:
                                nc.vector.tensor_tensor(
                                    sqp[:, i, :], xc[:, ki, :], xc[:, ki, :], MUL
                                )
                        tiles.append(sqp)
                    sq_tiles[t] = tiles

                def acc_emit(t):
                    tiles = sq_tiles.pop(t)
                    acc = psA.tile([P, 512], dt.float32, tag="acc", name="acc")
                    for kp_, sqp in enumerate(tiles):
                        dr(acc[:], ones8[:], sqp[:], kp_ == 0, kp_ == KI_H // 2 - 1)
                    s1s[t] = rsqrt_tmp(acc, H, "s1rb", ws64=True)

                xc_dma(0)
                xc_dma(1)
                sq_emit(0)

                # q-slice ln1 stats on the DVE, overlapping the scalar squares
                accq = psA.tile([P, QR], dt.float32, tag="acc")
                for kp_ in range(KI_H // 2):
                    sqq = tmp.tile([P, 2, QR], dt.float8e4, tag="sqq", bufs=2)
                    for i in (0, 1):
                        ki = 2 * kp_ + i
                        nc.vector.tensor_tensor(sqq[:, i, :], xqbf[:, ki, :], xqbf[:, ki, :], MUL)
                    dr(accq[:], ones8[:], sqq[:], kp_ == 0, kp_ == KI_H // 2 - 1)
                rsqrt_into(accq, H, s1qrep64, ws64=True)
                acc_emit(0)

                for t in range(S // 512):
                    tsl = slice(t * 512, (t + 1) * 512)
                    if t + 2 < S // 512:
                        xc_dma(t + 2)
                    if t + 1 < S // 512:
                        sq_emit(t + 1)
                    xc = xcs.pop(t)
                    s1rep64 = s1s.pop(t)
                    ckvt = pB.tile([P, KI_KVL, 512], dt.bfloat16, tag="ckvt", bufs=2)
                    kvacc = psA.tile([P, 512], dt.float32, tag="acc2")
                    sqp2s = []
                    sqp2 = None
                    for nf in range(KI_KVL):
                        pt = ps.tile([P, 512], dt.float32, tag="mm")
                        for kp_ in range(KI_H // 2):
                            dr(
                                pt[:],
                                wkva[:, 2 * kp_ : 2 * kp_ + 2, nf * P : (nf + 1) * P],
                                xc[:, 2 * kp_ : 2 * kp_ + 2, :],
                                kp_ == 0,
                                kp_ == KI_H // 2 - 1,
                            )
                        nc.vector.tensor_tensor(ckvt[:, nf, :], pt[:], s1rep64[:], MUL)
                        if nf % 2 == 0:
                            sqp2 = tmp.tile([P, 2, 512], dt.float8e4, tag="sqp2", bufs=2)
                            sqp2s.append(sqp2)
                        nc.vector.tensor_tensor(
                            sqp2[:, nf % 2, :], ckvt[:, nf, :], ckvt[:, nf, :], MUL
                        )
                    # k_pe: last 64 cols of w_kva (1x true scale via s1rep64)
                    pt = ps.tile([ROPE, 512], dt.float32, tag="mm")
                    for kp_ in range(KI_H // 2):
                        dr(
                            pt[:],
                            wkva[:, 2 * kp_ : 2 * kp_ + 2, KV_LORA : KV_LORA + ROPE],
                            xc[:, 2 * kp_ : 2 * kp_ + 2, :],
                            kp_ == 0,
                            kp_ == KI_H // 2 - 1,
                        )
                    for q2, sqp2 in enumerate(sqp2s):
                        dr(kvacc[:], ones8[:], sqp2[:], q2 == 0, q2 == KI_KVL // 2 - 1)
                    if t + 1 < S // 512:
                        acc_emit(t + 1)
                    pes = tmp.tile([ROPE, 512], dt.float32, tag="pes", bufs=2)
                    nc.vector.tensor_tensor(pes[:], pt[:], s1rep64[:ROPE, :], MUL)
                    x2h = tmp.tile([32, 512], dt.float32, tag="x2h", bufs=2)
                    nc.sync.dma_start(x2h[:], pes[32:, :])
                    t1 = tmp.tile([32, 512], dt.float32, tag="t1", bufs=2)
                    t2 = tmp.tile([32, 512], dt.float32, tag="t2", bufs=2)
                    o2 = tmp.tile([32, 512], dt.float8e4, tag="o2", bufs=2)
                    nc.vector.tensor_tensor(t1[:], pes[:32, :], cosb[:, tsl], MUL)
                    nc.vector.tensor_tensor(t2[:], x2h[:], sinb[:, tsl], MUL)
                    nc.vector.tensor_tensor(kpe[:32, tsl], t1[:], t2[:], SUB)
                    nc.vector.tensor_tensor(t1[:], x2h[:], cosb[:, tsl], MUL)
                    nc.vector.tensor_tensor(t2[:], pes[:32, :], sinb[:, tsl], MUL)
                    nc.vector.tensor_tensor(o2[:], t1[:], t2[:], ADD)
                    nc.sync.dma_start(kpe[32:, tsl], o2[:])
                    # kv_a rmsnorm rescale -> bf16 latents
                    rkv = rsqrt_tmp(kvacc, KV_LORA, "s1rc")
                    for nf in range(KI_KVL):
                        nc.vector.tensor_tensor(ckv[:, nf, tsl], ckvt[:, nf, :], rkv[:], MUL)

            # ==== attention k/v weight pool created early so hg=0 prefetches ====
            p3 = mid.enter_context(tc.tile_pool(name="p3", bufs=1))
            psC = mid.enter_context(tc.tile_pool(name="psC", bufs=2, space="PSUM"))
            ksb0 = p3.tile([P, 2, S], dt.float8e4, tag="ksb0", bufs=1)
            ksb1 = p3.tile([P, 2, S], dt.float8e4, tag="ksb1", bufs=1)
            ksbs = [ksb0, ksb1]
            for i in range(2):
                nc.vector.memset(ksbs[i][64:, 1, :], 0.0)
            kv_cache = {}

            def kvw_tile(tag, src, hg):
                key = (tag, hg)
                if key not in kv_cache:
                    tt = p3.tile([P, KI_KVL, 512], dt.bfloat16, tag=tag, bufs=2)
                    nc.sync.dma_start(tt[:], src[hg])
                    kv_cache[key] = tt
                return kv_cache.pop(key)

            kv_cache[("wkh", 0)] = p3.tile([P, KI_KVL, 512], dt.bfloat16, tag="wkh", bufs=2, name="wkh_p")
            nc.sync.dma_start(kv_cache[("wkh", 0)][:], w_kv_k[0])
            kv_cache[("wvh", 0)] = p3.tile([P, KI_KVL, 512], dt.bfloat16, tag="wvh", bufs=2, name="wvh_p")
            nc.sync.dma_start(kv_cache[("wvh", 0)][:], w_kv_v[0])

            # ==== phase B: q path ====
            with tc.tile_pool(name="p2", bufs=1) as p2:
                qlat = p2.tile([P, KI_QL, QR], dt.bfloat16)
                qacc = psC.tile([P, QR], dt.float32, tag="acc", bufs=1)
                sqp = None
                for nf in range(KI_QL):
                    wt = wqa_tile(nf)
                    if nf + 1 < KI_QL:
                        wqa_tile_pre = wq_pool.tile([P, KI_H, P], dt.float8e4, tag="wqa", bufs=2)
                        nc.sync.dma_start(wqa_tile_pre[:], w_qa[nf + 1])
                        wq_cache[nf + 1] = wqa_tile_pre
                    pt = ps.tile([P, QR], dt.float32, tag="mm")
                    for kp_ in range(KI_H // 2):
                        dr(
                            pt[:],
                            wt[:, 2 * kp_ : 2 * kp_ + 2, :],
                            xqbf[:, 2 * kp_ : 2 * kp_ + 2, :],
                            kp_ == 0,
                            kp_ == KI_H // 2 - 1,
                        )
                    nc.vector.tensor_tensor(qlat[:, nf, :], pt[:], s1qrep64[:], MUL)
                    if nf % 2 == 0:
                        sqp = tmp.tile([P, 2, QR], dt.float8e4, tag="sqb", bufs=2)
                    nc.vector.tensor_tensor(sqp[:, nf % 2, :], qlat[:, nf, :], qlat[:, nf, :], MUL)
                    if nf % 2 == 1:
                        dr(qacc[:], ones8[:], sqp[:], nf // 2 == 0, nf // 2 == KI_QL // 2 - 1)
                sqrep = p2.tile([P, QR], dt.float32)
                rsqrt_into(qacc, Q_LORA, sqrep)

                # rope tables for q with q_a_ln scale (and 1/64) folded in
                cosq = p2.tile([32, QR], dt.float32)
                sinq = p2.tile([32, QR], dt.float32)
                nc.sync.dma_start(cosq[:], cosTq[:])
                nc.sync.dma_start(sinq[:], sinTq[:])
                nc.vector.tensor_tensor(cosq[:], cosq[:], sqrep[:32, :], MUL)
                nc.vector.tensor_tensor(sinq[:], sinq[:], sqrep[:32, :], MUL)

                for h in range(NH):
                    nc.vector.memset(qpk[64:, h, 1, :], 0.0)
                    wt = p2.tile([P, KI_QL, QHD], dt.bfloat16, tag="wqb", bufs=2)
                    nc.sync.dma_start(wt[:], w_qb[h])
                    pt = ps.tile([P, QR], dt.float32, tag="mm")
                    for ki in range(KI_QL):
                        nc.tensor.matmul(
                            pt[:], wt[:, ki, :NOPE], qlat[:, ki, :],
                            start=(ki == 0), stop=(ki == KI_QL - 1),
                        )
                    nc.vector.tensor_tensor(qpk[:, h, 0, :], pt[:], sqrep[:], MUL)
                    pt2 = ps.tile([ROPE, QR], dt.float32, tag="mm")
                    for ki in range(KI_QL):
                        nc.tensor.matmul(
                            pt2[:], wt[:, ki, NOPE:QHD], qlat[:, ki, :],
                            start=(ki == 0), stop=(ki == KI_QL - 1),
                        )
                    pesq = tmp.tile([ROPE, QR], dt.float32, tag="pes", bufs=2)
                    nc.vector.tensor_copy(pesq[:], pt2[:])
                    x2q = tmp.tile([32, QR], dt.float32, tag="x2h", bufs=2)
                    nc.sync.dma_start(x2q[:], pesq[32:, :])
                    t1 = tmp.tile([32, QR], dt.float32, tag="t1", bufs=2)
                    t2 = tmp.tile([32, QR], dt.float32, tag="t2", bufs=2)
                    o2 = tmp.tile([32, QR], dt.float8e4, tag="o2", bufs=2)
                    nc.vector.tensor_tensor(t1[:], pesq[:32, :], cosq[:], MUL)
                    nc.vector.tensor_tensor(t2[:], x2q[:], sinq[:], MUL)
                    nc.vector.tensor_tensor(qpk[:32, h, 1, :], t1[:], t2[:], SUB)
                    nc.vector.tensor_tensor(t1[:], x2q[:], cosq[:], MUL)
                    nc.vector.tensor_tensor(t2[:], pesq[:32, :], sinq[:], MUL)
                    nc.vector.tensor_tensor(o2[:], t1[:], t2[:], ADD)
                    nc.sync.dma_start(qpk[32:64, h, 1, :], o2[:])

            # ==== phase C: attention ====
            wo_cache = {}

            def wo_tile(nf):
                if nf not in wo_cache:
                    tt = wo_pool.tile([P, NH, VHD], dt.bfloat16, tag="wo", bufs=2)
                    nc.sync.dma_start(tt[:], w_o[nf])
                    wo_cache[nf] = tt
                return wo_cache.pop(nf)

            def normalize(h, av_t, se_t):
                # 1/se as exp(-ln(se)) on the scalar engine; se is
                # pre-replicated [P, QR] by the all-ones se reduction.
                rc = tmp.tile([P, QR], dt.float32, tag="s1r", bufs=2)
                lnt = tmp.tile([P, QR], dt.float32, tag="h1t", bufs=2)
                nc.scalar.activation(out=lnt[:], in_=se_t[:], func=AF.Ln)
                nc.scalar.activation(out=rc[:], in_=lnt[:], func=AF.Exp, scale=-1.0)
                nc.vector.tensor_tensor(attn[:, h, :], av_t[:], rc[:], MUL)

            def k_expand(h, wkh_t):
                # expand k_nope for head h into its packed ksb tile, one head
                # ahead of the score loop so its DVE drains are off the
                # critical path.
                ksb_t = ksbs[h % 2]
                hh_ = h % 4
                for c in range(S // 512):
                    csl = slice(c * 512, (c + 1) * 512)
                    pt = ps.tile([P, 512], dt.float32, tag="mm", name="pt")
                    for l in range(KI_KVL):
                        nc.tensor.matmul(
                            pt[:],
                            wkh_t[:, l, hh_ * P : (hh_ + 1) * P],
                            ckv[:, l, csl],
                            start=(l == 0), stop=(l == KI_KVL - 1),
                        )
                    nc.vector.tensor_copy(ksb_t[:, 0, csl], pt[:])
                nc.vector.tensor_copy(ksb_t[:64, 1, :], kpe[:])

            prev = None
            wkh_cur = kvw_tile("wkh", w_kv_k, 0)
            wvh_cur = kvw_tile("wvh", w_kv_v, 0)
            wkh_nxt = wvh_nxt = None
            k_expand(0, wkh_cur)
            for hg in range(NH // 4):
                if hg + 1 < NH // 4:
                    wkh_nxt = kvw_tile("wkh", w_kv_k, hg + 1)
                    wvh_nxt = kvw_tile("wvh", w_kv_v, hg + 1)
                # v for 4 heads at once
                vsb = p3.tile([P, TK, 4 * VHD], dt.float8e4, tag="vsb", bufs=2)
                for kt in range(TK):
                    pt = ps.tile([P, 4 * VHD], dt.float32, tag="mm")
                    for l in range(KI_KVL):
                        nc.tensor.matmul(
                            pt[:],
                            ckv[:, l, kt * P : (kt + 1) * P],
                            wvh_cur[:, l, :],
                            start=(l == 0), stop=(l == KI_KVL - 1),
                        )
                    nc.vector.tensor_copy(vsb[:, kt, :], pt[:])
                for hh in range(4):
                    h = hg * 4 + hh
                    ksb = ksbs[h % 2]
                    if h + 1 < NH:
                        k_expand(h + 1, wkh_cur if hh < 3 else wkh_nxt)
                    if h == NH - 1:  # prefetch o_proj weights under last head
                        wo_cache[0] = wo_pool.tile([P, NH, VHD], dt.bfloat16, tag="wo", bufs=2, name="wo_p")
                        nc.sync.dma_start(wo_cache[0][:], w_o[0])
                        wo_cache[1] = wo_pool.tile([P, NH, VHD], dt.bfloat16, tag="wo", bufs=2, name="wo_p")
                        nc.sync.dma_start(wo_cache[1][:], w_o[1])
                    if prev is not None:
                        normalize(*prev)
                        prev = None
                    av = psC.tile([P, QR], dt.float32, tag="av")
                    se = psC.tile([P, QR], dt.float32, tag="se")

                    def _drain(p8_, prt, off):
                        dr(se[:, off:], ones8[:], prt[:, :, off:], p8_ == 0, p8_ == TK // 2 - 1)
                        dr(
                            av[:, off:],
                            vsb[:, 2 * p8_ : 2 * p8_ + 2, hh * VHD : (hh + 1) * VHD],
                            prt[:, :, off:],
                            p8_ == 0,
                            p8_ == TK // 2 - 1,
                        )

                    pend = []
                    for p8_ in range(TK // 2):
                        off = (p8_ // 2) * P
                        prt = tmp.tile([P, 2, QR], dt.float8e4, tag="pr", bufs=4)
                        for i in (0, 1):
                            kt = 2 * p8_ + i
                            sc = ps.tile([P, QR], dt.float32, tag="mm")
                            nc.tensor.matmul(
                                sc[:, off:],
                                ksb[:, :, kt * P : (kt + 1) * P],
                                qpk[:, h, :, off:],
                                start=True,
                                stop=True,
                                perf_mode=DRM,
                            )
                            nc.scalar.activation(
                                out=prt[:, i, off:], in_=sc[:, off:], func=AF.Exp,
                                scale=ATTN_SCALE,
                            )
                            nc.vector.tensor_tensor(
                                prt[:, i, off : off + P],
                                prt[:, i, off : off + P],
                                maskt[:, kt, :],
                                MUL,
                            )
                        pend.append((p8_, prt, off))
                        if len(pend) > 2:
                            _drain(*pend.pop(0))
                    while pend:
                        _drain(*pend.pop(0))
                    prev = (h, av, se)
                wkh_cur, wvh_cur = wkh_nxt, wvh_nxt
            normalize(*prev)

        # ==== phase D: o_proj + residual + ln2 (h1 resident in SBUF) ====
        with contextlib.ExitStack() as sc45:
            x2m = sc45.enter_context(tc.tile_pool(name="x2m", bufs=1))
            x2 = x2m.tile([P, KI_H, QR], dt.bfloat16)
            h1sb = x2m.tile([P, KI_H, QR], dt.bfloat16)
            msb = x2m.tile([P, NF_FF, QR], dt.bfloat16)
            p5 = sc45.enter_context(tc.tile_pool(name="p5", bufs=1))
            ffn_cache = {}

            def ffn_tile(tag, src, nf):
                key = (tag, nf)
                if key not in ffn_cache:
                    tt = p5.tile([P, KI_H, P], dt.bfloat16, tag=tag, bufs=2)
                    for hk in range(2):
                        nc.sync.dma_start(
                            tt[:, hk * 8 : (hk + 1) * 8, :], src[nf, :, hk * 8 : (hk + 1) * 8, :]
                        )
                    ffn_cache[key] = tt
                return ffn_cache.pop(key)

            ffn_cache[("wg", 0)] = p5.tile([P, KI_H, P], dt.bfloat16, tag="wg", bufs=2, name="wg_p")
            nc.sync.dma_start(ffn_cache[("wg", 0)][:], w_g[0])
            ffn_cache[("wu", 0)] = p5.tile([P, KI_H, P], dt.bfloat16, tag="wu", bufs=2, name="wu_p")
            nc.sync.dma_start(ffn_cache[("wu", 0)][:], w_u[0])
            wg_dma_split = True

            with tc.tile_pool(name="psD", bufs=1, space="PSUM") as psD:
                oacc = psD.tile([P, QR], dt.float32, tag="acc")
                for nf in range(KI_H):
                    wt = wo_tile(nf)
                    if nf + 1 < KI_H and nf + 1 not in wo_cache:
                        wo_cache[nf + 1] = wo_pool.tile([P, NH, VHD], dt.bfloat16, tag="wo", bufs=2, name="wo_p")
                        nc.sync.dma_start(wo_cache[nf + 1][:], w_o[nf + 1])
                    pt = ps.tile([P, QR], dt.float32, tag="mm")
                    for kh in range(NH):
                        nc.tensor.matmul(
                            pt[:],
                            wt[:, kh, :],
                            attn[:, kh, :],
                            start=(kh == 0),
                            stop=(kh == NH - 1),
                        )
                    ht = ld.tile([P, QR], dt.float32, tag="hload")
                    nc.sync.dma_start(ht[:], hTq[nf * P : (nf + 1) * P, :])
                    nc.vector.tensor_tensor(h1sb[:, nf, :], pt[:], ht[:], ADD)
                    sq = tmp.tile([P, QR], dt.bfloat16, tag="sq")
                    nc.vector.tensor_tensor(sq[:], h1sb[:, nf, :], h1sb[:, nf, :], MUL)
                    nc.tensor.matmul(
                        oacc[:], ones_bfP[:], sq[:], start=(nf == 0), stop=(nf == KI_H - 1)
                    )
                s2rep = tmp.tile([P, QR], dt.float32, tag="s1r", bufs=2)
                rsqrt_into(oacc, H, s2rep)
                for nf in range(KI_H):
                    nc.vector.tensor_tensor(x2[:, nf, :], h1sb[:, nf, :], s2rep[:], MUL)

            # ==== phase E: FFN (SwiGLU), bf16 ====
            for nf in range(NF_FF):
                wtg = ffn_tile("wg", w_g, nf)
                if nf + 1 < NF_FF:
                    wgp = p5.tile([P, KI_H, P], dt.bfloat16, tag="wg", bufs=2, name="wg_p")
                    for hk in range(2):
                        nc.sync.dma_start(
                            wgp[:, hk * 8 : (hk + 1) * 8, :], w_g[nf + 1, :, hk * 8 : (hk + 1) * 8, :]
                        )
                    ffn_cache[("wg", nf + 1)] = wgp
                pg = ps.tile([P, QR], dt.float32, tag="mm")
                for ki in range(KI_H):
                    nc.tensor.matmul(
                        pg[:], wtg[:, ki, :], x2[:, ki, :],
                        start=(ki == 0), stop=(ki == KI_H - 1),
                    )
                gs = tmp.tile([P, QR], dt.bfloat16, tag="sq")
                nc.scalar.activation(out=gs[:], in_=pg[:], func=AF.Silu)
                wtu = ffn_tile("wu", w_u, nf)
                if nf + 1 < NF_FF:
                    wup = p5.tile([P, KI_H, P], dt.bfloat16, tag="wu", bufs=2, name="wu_p")
                    for hk in range(2):
                        nc.sync.dma_start(
                            wup[:, hk * 8 : (hk + 1) * 8, :], w_u[nf + 1, :, hk * 8 : (hk + 1) * 8, :]
                        )
                    ffn_cache[("wu", nf + 1)] = wup
                pu = ps.tile([P, QR], dt.float32, tag="mm")
                for ki in range(KI_H):
                    nc.tensor.matmul(
                        pu[:], wtu[:, ki, :], x2[:, ki, :],
                        start=(ki == 0), stop=(ki == KI_H - 1),
                    )
                nc.vector.tensor_tensor(msb[:, nf, :], pu[:], gs[:], MUL)

            for nf in range(KI_H):
                pt = ps.tile([P, QR], dt.float32, tag="mm")
                for qtr in range(4):
                    wt = p5.tile([P, NF_FF // 4, P], dt.bfloat16, tag="wd", bufs=2)
                    for hk in range(4):
                        nc.sync.dma_start(
                            wt[:, hk * 4 : (hk + 1) * 4, :],
                            w_d[nf, :, qtr * 16 + hk * 4 : qtr * 16 + (hk + 1) * 4, :],
                        )
                    for ki in range(NF_FF // 4):
                        kk = qtr * 16 + ki
                        nc.tensor.matmul(
                            pt[:], wt[:, ki, :], msb[:, kk, :],
                            start=(kk == 0), stop=(kk == NF_FF - 1),
                        )
                ot = tmp.tile([P, QR], dt.float32, tag="h1t", bufs=2)
                nc.vector.tensor_tensor(ot[:], pt[:], h1sb[:, nf, :], ADD)
                nc.sync.dma_start(out[nf * P : (nf + 1) * P, :], ot[:])

    return nc


# ---------------------------------------------------------------------------
# host-side packing
# ---------------------------------------------------------------------------
def _deint_perm():
    # deinterleave: out[i] = in[2i] (i<32), in[2(i-32)+1] (i>=32)
    return np.concatenate([np.arange(0, ROPE, 2), np.arange(1, ROPE, 2)])


def _pack_lhst(w, nki, nnf, nfree=P, dtype=BF16, scale=1.0):
    # w [nki*P, nnf*nfree] -> [nnf, P, nki, nfree]
    return np.ascontiguousarray(
        (w.reshape(nki, P, nnf, nfree) * scale).transpose(2, 1, 0, 3).astype(dtype)
    )


def _prep_shared(inputs):
    perm = _deint_perm()
    ln1 = inputs["ln1_w"].astype(np.float32)
    qaln = inputs["q_a_ln_w"].astype(np.float32)
    kvln = inputs["kv_a_ln_w"].astype(np.float32)
    ln2 = inputs["ln2_w"].astype(np.float32)

    w_qa = inputs["q_a_kernel"].astype(np.float32) * ln1[:, None]
    w_kva = inputs["kv_a_kernel"].astype(np.float32) * ln1[:, None]
    w_kva = w_kva.copy()
    w_kva[:, KV_LORA:] = w_kva[:, KV_LORA:][:, perm]
    w_qb = inputs["q_b_kernel"].astype(np.float32) * qaln[:, None]
    w_qb = w_qb.copy()
    for h in range(NH):
        blk = slice(h * QHD + NOPE, (h + 1) * QHD)
        w_qb[:, blk] = w_qb[:, blk][:, perm]
    w_kvb = inputs["kv_b_kernel"].astype(np.float32) * kvln[:, None]
    w_o = inputs["o_kernel"].astype(np.float32)
    w_g = inputs["gate_kernel"].astype(np.float32) * ln2[:, None]
    w_u = inputs["up_kernel"].astype(np.float32) * ln2[:, None]
    w_d = inputs["down_kernel"].astype(np.float32)

    shared = {
        "w_qa": _pack_lhst(w_qa, KI_H, KI_QL, P, F8, WS),
        # w_qb: [NH, P, KI_QL, QHD]
        "w_qb": np.ascontiguousarray(
            w_qb.reshape(KI_QL, P, NH, QHD).transpose(2, 1, 0, 3).astype(BF16)
        ),
        # w_kva resident: [P, KI_H, 576]
        "w_kva": np.ascontiguousarray(
            (w_kva.reshape(KI_H, P, KV_LORA + ROPE) * WS).transpose(1, 0, 2).astype(F8)
        ),
        # w_kvb split into k/v halves, packed per head-group of 4:
        "w_kv_k": np.ascontiguousarray(
            w_kvb.reshape(KI_KVL, P, NH // 4, 4, 2, 128)[:, :, :, :, 0, :]
            .transpose(2, 1, 0, 3, 4)
            .reshape(NH // 4, P, KI_KVL, 512)
            .astype(BF16)
        ),
        "w_kv_v": np.ascontiguousarray(
            w_kvb.reshape(KI_KVL, P, NH // 4, 4, 2, 128)[:, :, :, :, 1, :]
            .transpose(2, 1, 0, 3, 4)
            .reshape(NH // 4, P, KI_KVL, 512)
            .astype(BF16)
        ),
        # w_o: [KI_H(nf), P, NH, VHD]
        "w_o": np.ascontiguousarray(
            w_o.reshape(NH, VHD, KI_H, P).transpose(2, 1, 0, 3).astype(BF16)
        ),
        "w_g": _pack_lhst(w_g, KI_H, NF_FF),
        "w_u": _pack_lhst(w_u, KI_H, NF_FF),
        "w_d": _pack_lhst(w_d, NF_FF, KI_H),
    }
    return shared


def _prep_batch(inputs, b):
    hid = np.asarray(inputs["hidden_states"][b], dtype=np.float32)  # [S, H]
    hT = np.ascontiguousarray(hid.T)  # [H, S] f32
    hT8 = hT.astype(F8)
    pos = np.asarray(inputs["position_ids"][b]).astype(np.int64)
    cos_g = np.asarray(inputs["cos"], dtype=np.float32)[pos][:, :32]  # [S, 32]
    sin_g = np.asarray(inputs["sin"], dtype=np.float32)[pos][:, :32]
    return hT, hT8, np.ascontiguousarray(cos_g.T), np.ascontiguousarray(sin_g.T)


def _core_rows(j):
    return np.concatenate(
        [np.arange((j + 4 * i) * P, (j + 4 * i + 1) * P) for i in range(4)]
    )


def _core_masks(j):
    # one 128-col mask block per key tile, applied at query slot kt//4
    kp = np.arange(P)[:, None]
    qf = np.arange(P)[None, :]
    m = np.zeros((P, TK, P), dtype=F8)
    for kt in range(TK):
        r = kt % 4
        if j > r:
            m[:, kt, :] = 1.0
        elif j == r:
            m[:, kt, :] = (kp <= qf).astype(F8)
        # j < r: stays zero
    return m


def kernel(**inputs) -> np.ndarray:
    import concourse.bass as bass  # noqa: F401  (env check)
    from concourse.bass_utils import run_bass_kernel_spmd

    if "nc" not in _COMPILED:
        _COMPILED["nc"] = _build_nc()
    nc = _COMPILED["nc"]

    shared = _prep_shared(inputs)
    in_maps = []
    per_batch = [_prep_batch(inputs, b) for b in range(B)]
    for c in range(8):
        b, j = c // 4, c % 4
        hT, hT8, cosTb, sinTb = per_batch[b]
        rows = _core_rows(j)
        in_map = dict(shared)
        in_map["hTb"] = hT8
        in_map["hTqb"] = np.ascontiguousarray(hT8[:, rows])
        in_map["hTq"] = np.ascontiguousarray(hT[:, rows])
        in_map["cosT"] = cosTb
        in_map["sinT"] = sinTb
        in_map["cosTq"] = np.ascontiguousarray(cosTb[:, rows])
        in_map["sinTq"] = np.ascontiguousarray(sinTb[:, rows])
        in_map["masks"] = _core_masks(j)
        in_maps.append(in_map)

    res = run_bass_kernel_spmd(nc, in_maps, core_ids=list(range(8)))
    globals()["LAST_RESULT"] = res

    out = np.empty((B, S, H), dtype=np.float32)
    for c in range(8):
        b, j = c // 4, c % 4
        out[b, _core_rows(j), :] = res.results[c]["out"].T
    return out


# revision 23
# speedup vs baseline: 1.0042x; 1.0042x over previous
"""DeepseekV2 decoder layer (MLA attention + SwiGLU MLP) on 8 TRN2 NeuronCores.

Sharding: core c -> batch b = c//4, query tiles {j, j+4, j+8, j+12} (j = c%4,
128 rows each, causally interleaved). The interleave makes the causal
structure identical on every core: key tile kt is only needed by query slots
s >= kt//4, so the SPMD program skips 37.5% of score/AV work uniformly.

fp8 (e4m3, DoubleRow 2x matmul) is used for the hidden-state input, the
q_a/kv_a projections, the packed q/k score tiles, v, and the attention
probabilities; q_b/kv_b, o_proj and the FFN stay bf16 (fp8 there blows the
2e-2 error budget). fp8 weights are scaled x16 on the host to escape
denormals; the 1/16 compensation is folded into the rsqrt stat scales
(ws64 variants), so no extra ops are spent.

Per-head packed layout for scores: k/q tiles [128, 2, n] where pair slot 0 =
nope (128 feats), slot 1 = rope (64 feats) + 64 zero rows, so one DoubleRow
matmul computes a whole 192-dim score tile.
"""

import json

import numpy as np
import ml_dtypes

B, S, H = 2, 2048, 2048
NH = 16
Q_LORA = 1536
KV_LORA = 512
NOPE = 128
ROPE = 64
QHD = NOPE + ROPE  # 192
VHD = 128
FF = 8192
EPS = 1e-6
P = 128
QR = 512  # query rows per core
TK = S // P  # 16 key tiles
KI_H = H // P  # 16
KI_QL = Q_LORA // P  # 12
KI_KVL = KV_LORA // P  # 4
NF_FF = FF // P  # 64
ATTN_SCALE = QHD ** -0.5
WS = 16.0  # fp8 weight scale (16x: escapes denormals; keeps 16x-scaled k_pe/v tails far below fp8's 240 max)

BF16 = ml_dtypes.bfloat16
F8 = ml_dtypes.float8_e4m3

_COMPILED = {}


# ---------------------------------------------------------------------------
# compiler workaround: this container's walrus rejects >1 sem wait per
# instruction; split extra waits onto single-wait NoOps.
# ---------------------------------------------------------------------------
def _install_multiwait_fix(bass):
    if getattr(bass.Bass, "_multiwait_fix_installed", False):
        return
    orig = bass.Bass.to_json_bytes

    def _split(m):
        for f in m.get("functions", []):
            for b in f.get("blocks", []):
                out = []
                for inst in b.get("instructions", []):
                    si = inst.get("sync_info") or {}
                    waits = si.get("on_wait") or []
                    if len(waits) > 1:
                        for k, w in enumerate(waits[:-1]):
                            out.append(
                                {
                                    "debug": inst.get("debug", 0),
                                    "engine": inst["engine"],
                                    "ins": [],
                                    "name": f"{inst['name']}_w{k}",
                                    "opcode": "NoOp",
                                    "outs": [],
                                    "sync_info": {"on_update": [], "on_wait": [w]},
                                }
                            )
                        si["on_wait"] = [waits[-1]]
                    out.append(inst)
                b["instructions"] = out
        return m

    def patched(self):
        raw = orig(self)
        try:
            return json.dumps(_split(json.loads(raw))).encode()
        except Exception:
            return raw

    bass.Bass.to_json_bytes = patched
    bass.Bass._multiwait_fix_installed = True


def _install_drain_fix(tile, ScopedClock, VectorClock):
    if getattr(tile.TileContext, "_drain_fix_installed", False):
        return

    def _drain_and_barrier(self, tick_clock, wait_clock):
        gc = tick_clock.global_clock
        n = len(gc)
        for p in range(n):
            t = gc[p]
            if t > 0:
                vc = VectorClock([0] * n)
                vc.require_at_least(p, t)
                d = self.nc.sync.drain()
                wait_clock.add_sem_waits(d.ins, ScopedClock({None: vc}))
        self.nc.all_engine_barrier()
        popped = self.nc._tile_sem_poison_stack.pop()
        assert popped is self._sem_poison
        self.nc.clear_and_free_semaphores(list(self.sems.allocated().values()))
        self.nc.all_engine_barrier()

    tile.TileContext._drain_and_barrier = _drain_and_barrier
    tile.TileContext._drain_fix_installed = True


# ---------------------------------------------------------------------------
# device program
# ---------------------------------------------------------------------------
def _build_nc():
    import concourse.bass as bass
    import concourse.mybir as mybir
    import concourse.tile as tile
    from concourse.vector_clock import ScopedClock, VectorClock

    _install_multiwait_fix(bass)
    _install_drain_fix(tile, ScopedClock, VectorClock)

    dt = mybir.dt
    AF = mybir.ActivationFunctionType
    MUL = mybir.AluOpType.mult
    ADD = mybir.AluOpType.add
    SUB = mybir.AluOpType.subtract
    DRM = mybir.MatmulPerfMode.DoubleRow

    nc = bass.Bass()

    # register EPS so float bias=EPS works on the scalar engine
    _eps_t = nc.alloc_sbuf_tensor(f"const-float32-{EPS}", [128, 1], dt.float32)
    nc.gpsimd.memset(_eps_t.ap(), EPS)
    nc.const_aps.aps[(dt.float32, EPS)] = _eps_t.ap()
    EPS64 = EPS * WS * WS
    _eps64_t = nc.alloc_sbuf_tensor(f"const-float32-{EPS64}", [128, 1], dt.float32)
    nc.gpsimd.memset(_eps64_t.ap(), EPS64)
    nc.const_aps.aps[(dt.float32, EPS64)] = _eps64_t.ap()
    import math
    NLNWS = -math.log(WS)
    _nlnws_t = nc.alloc_sbuf_tensor(f"const-float32-{NLNWS}", [128, 1], dt.float32)
    nc.gpsimd.memset(_nlnws_t.ap(), NLNWS)
    nc.const_aps.aps[(dt.float32, NLNWS)] = _nlnws_t.ap()
    nc.all_engine_barrier()

    # ---- inputs ----
    hTb = nc.dram_tensor("hTb", [H, S], dt.float8e4, kind="ExternalInput")
    hTqb = nc.dram_tensor("hTqb", [H, QR], dt.float8e4, kind="ExternalInput")
    hTq = nc.dram_tensor("hTq", [H, QR], dt.float32, kind="ExternalInput")
    cosT = nc.dram_tensor("cosT", [32, S], dt.float32, kind="ExternalInput")
    sinT = nc.dram_tensor("sinT", [32, S], dt.float32, kind="ExternalInput")
    cosTq = nc.dram_tensor("cosTq", [32, QR], dt.float32, kind="ExternalInput")
    sinTq = nc.dram_tensor("sinTq", [32, QR], dt.float32, kind="ExternalInput")
    masks = nc.dram_tensor("masks", [P, TK, P], dt.float8e4, kind="ExternalInput")
    w_qa = nc.dram_tensor("w_qa", [KI_QL, P, KI_H, P], dt.float8e4, kind="ExternalInput")
    w_qb = nc.dram_tensor("w_qb", [NH, P, KI_QL, QHD], dt.bfloat16, kind="ExternalInput")
    w_kva = nc.dram_tensor("w_kva", [P, KI_H, KV_LORA + ROPE], dt.float8e4, kind="ExternalInput")
    w_kv_k = nc.dram_tensor("w_kv_k", [NH // 4, P, KI_KVL, 512], dt.bfloat16, kind="ExternalInput")
    w_kv_v = nc.dram_tensor("w_kv_v", [NH // 4, P, KI_KVL, 512], dt.bfloat16, kind="ExternalInput")
    w_o = nc.dram_tensor("w_o", [KI_H, P, NH, VHD], dt.bfloat16, kind="ExternalInput")
    w_g = nc.dram_tensor("w_g", [NF_FF, P, KI_H, P], dt.bfloat16, kind="ExternalInput")
    w_u = nc.dram_tensor("w_u", [NF_FF, P, KI_H, P], dt.bfloat16, kind="ExternalInput")
    w_d = nc.dram_tensor("w_d", [KI_H, P, NF_FF, P], dt.bfloat16, kind="ExternalInput")
    out = nc.dram_tensor("out", [H, QR], dt.float32, kind="ExternalOutput")

    import contextlib

    with tile.TileContext(nc) as tc, contextlib.ExitStack() as top:
        tp = lambda **kw: top.enter_context(tc.tile_pool(**kw))
        ones = tp(name="ones", bufs=1)
        tmp = tp(name="tmp", bufs=3)
        ld = tp(name="ld", bufs=3)
        ps = tp(name="ps", bufs=3, space="PSUM")
        attn_pool = tp(name="attn_pool", bufs=1)
        attn = attn_pool.tile([P, NH, QR], dt.bfloat16)
        maskp = tp(name="maskp", bufs=1)
        maskt = maskp.tile([P, TK, P], dt.float8e4)
        wo_pool = tp(name="wo_pool", bufs=2)

        ones_bfP = ones.tile([P, P], dt.bfloat16)
        nc.vector.memset(ones_bfP[:], 1.0)
        ones8 = ones.tile([P, 2, P], dt.float8e4)
        nc.vector.memset(ones8[:], 1.0)

        def dr(o, l, r, start, stop):
            nc.tensor.matmul(o, l, r, start=start, stop=stop, perf_mode=DRM)

        def rsqrt_into(acc, denom, dst, ws64=False):
            # dst = 1/sqrt(mean + eps) (optionally a further 1/WS) computed as
            # exp(-0.5*ln(.)) -- two scalar-engine table ops, avoiding the
            # 3.3us DVE reciprocal that would serialize the vector queue.
            lnt = tmp.tile([P, acc.shape[-1]], dt.float32, tag="h1t", bufs=2)
            nc.scalar.activation(
                out=lnt[:], in_=acc[:], func=AF.Ln, bias=EPS, scale=1.0 / denom
            )
            if ws64:
                nc.scalar.activation(out=dst[:], in_=lnt[:], func=AF.Exp, scale=-0.5, bias=NLNWS)
            else:
                nc.scalar.activation(out=dst[:], in_=lnt[:], func=AF.Exp, scale=-0.5)

        def rsqrt_tmp(acc, denom, tag, ws64=False):
            d = tmp.tile([P, acc.shape[-1]], dt.float32, tag=tag, bufs=2)
            rsqrt_into(acc, denom, d, ws64)
            return d

        with contextlib.ExitStack() as mid:
            lat = mid.enter_context(tc.tile_pool(name="lat", bufs=1))
            ckv = lat.tile([P, KI_KVL, S], dt.bfloat16)  # normalized kv latents (1x, bf16)
            kpe = lat.tile([ROPE, S], dt.float8e4)  # roped shared key-pe (1x)
            pA = mid.enter_context(tc.tile_pool(name="pA", bufs=1))
            xqbf = pA.tile([P, KI_H, QR], dt.float8e4)
            s1qrep64 = pA.tile([P, QR], dt.float32)
            qpk_pool = mid.enter_context(tc.tile_pool(name="qpk", bufs=1))
            qpk = qpk_pool.tile([P, NH, 2, QR], dt.float8e4)
            wq_pool = mid.enter_context(tc.tile_pool(name="wq", bufs=1))

            wq_cache = {}

            def wqa_tile(nf):
                if nf not in wq_cache:
                    t = wq_pool.tile([P, KI_H, P], dt.float8e4, tag="wqa", bufs=2)
                    nc.sync.dma_start(t[:], w_qa[nf])
                    wq_cache[nf] = t
                return wq_cache.pop(nf)

            # ==== phase A: ln1 stats + kv latents + shared roped k_pe ====
            with tc.tile_pool(name="pB", bufs=1) as pB, tc.tile_pool(
                name="psA", bufs=2, space="PSUM"
            ) as psA:
                nc.sync.dma_start(maskt[:], masks[:])
                for ki in range(KI_H):
                    nc.sync.dma_start(xqbf[:, ki, :], hTqb[ki * P : (ki + 1) * P, :])
                wkva = pB.tile([P, KI_H, KV_LORA + ROPE], dt.float8e4)
                nc.sync.dma_start(wkva[:], w_kva[:])
                cosb = pB.tile([32, S], dt.float32)
                sinb = pB.tile([32, S], dt.float32)
                nc.sync.dma_start(cosb[:], cosT[:])
                nc.sync.dma_start(sinb[:], sinT[:])
                # prefetch first q_a weights for phase B
                wqa_pre = wq_pool.tile([P, KI_H, P], dt.float8e4, tag="wqa", bufs=2)
                nc.sync.dma_start(wqa_pre[:], w_qa[0])
                wq_cache[0] = wqa_pre

                # chunk pipeline: squares for chunk t+1 run on the SCALAR
                # engine while the PE does chunk t's kv matmuls and the DVE
                # does chunk t's PSUM drains; acc DRs for t+1 are emitted after
                # kv work so the PE never waits on the square/stat chain.
                xcs = {}
                s1s = {}
                sq_tiles = {}

                def xc_dma(t):
                    tsl = slice(t * 512, (t + 1) * 512)
                    xc = pB.tile([P, KI_H, 512], dt.float8e4, tag="xc", bufs=3, name="xc")
                    for ki in range(KI_H):
                        nc.sync.dma_start(xc[:, ki, :], hTb[ki * P : (ki + 1) * P, tsl])
                    xcs[t] = xc

                def sq_emit(t):
                    xc = xcs[t]
                    tiles = []
                    for kp_ in range(KI_H // 2):
                        sqp = tmp.tile([P, 2, 512], dt.float8e4, tag="sqp", bufs=8, name="sqp")
                        for i in (0, 1):
                            ki = 2 * kp_ + i
                            if kp_ % 2 == 0:
                                nc.scalar.activation(
                                    out=sqp[:, i, :], in_=xc[:, ki, :], func=AF.Square
                                )
                            else:
                                nc.vector.tensor_tensor(
                                    sqp[:, i, :], xc[:, ki, :], xc[:, ki, :], MUL
                                )
                        tiles.append(sqp)
                    sq_tiles[t] = tiles

                def acc_emit(t):
                    tiles = sq_tiles.pop(t)
                    acc = psA.tile([P, 512], dt.float32, tag="acc", name="acc")
                    for kp_, sqp in enumerate(tiles):
                        dr(acc[:], ones8[:], sqp[:], kp_ == 0, kp_ == KI_H // 2 - 1)
                    s1s[t] = rsqrt_tmp(acc, H, "s1rb", ws64=True)

                xc_dma(0)
                xc_dma(1)
                sq_emit(0)

                # q-slice ln1 stats on the DVE, overlapping the scalar squares
                accq = psA.tile([P, QR], dt.float32, tag="acc")
                for kp_ in range(KI_H // 2):
                    sqq = tmp.tile([P, 2, QR], dt.float8e4, tag="sqq", bufs=2)
                    for i in (0, 1):
                        ki = 2 * kp_ + i
                        nc.vector.tensor_tensor(sqq[:, i, :], xqbf[:, ki, :], xqbf[:, ki, :], MUL)
                    dr(accq[:], ones8[:], sqq[:], kp_ == 0, kp_ == KI_H // 2 - 1)
                rsqrt_into(accq, H, s1qrep64, ws64=True)
                acc_emit(0)

                for t in range(S // 512):
                    tsl = slice(t * 512, (t + 1) * 512)
                    if t + 2 < S // 512:
                        xc_dma(t + 2)
                    if t + 1 < S // 512:
                        sq_emit(t + 1)
                    xc = xcs.pop(t)
                    s1rep64 = s1s.pop(t)
                    ckvt = pB.tile([P, KI_KVL, 512], dt.bfloat16, tag="ckvt", bufs=2)
                    kvacc = psA.tile([P, 512], dt.float32, tag="acc2")
                    sqp2s = []
                    sqp2 = None
                    for nf in range(KI_KVL):
                        pt = ps.tile([P, 512], dt.float32, tag="mm")
                        for kp_ in range(KI_H // 2):
                            dr(
                                pt[:],
                                wkva[:, 2 * kp_ : 2 * kp_ + 2, nf * P : (nf + 1) * P],
                                xc[:, 2 * kp_ : 2 * kp_ + 2, :],
                                kp_ == 0,
                                kp_ == KI_H // 2 - 1,
                            )
                        nc.vector.tensor_tensor(ckvt[:, nf, :], pt[:], s1rep64[:], MUL)
                        if nf % 2 == 0:
                            sqp2 = tmp.tile([P, 2, 512], dt.float8e4, tag="sqp2", bufs=2)
                            sqp2s.append(sqp2)
                        nc.vector.tensor_tensor(
                            sqp2[:, nf % 2, :], ckvt[:, nf, :], ckvt[:, nf, :], MUL
                        )
                    # k_pe: last 64 cols of w_kva (1x true scale via s1rep64)
                    pt = ps.tile([ROPE, 512], dt.float32, tag="mm")
                    for kp_ in range(KI_H // 2):
                        dr(
                            pt[:],
                            wkva[:, 2 * kp_ : 2 * kp_ + 2, KV_LORA : KV_LORA + ROPE],
                            xc[:, 2 * kp_ : 2 * kp_ + 2, :],
                            kp_ == 0,
                            kp_ == KI_H // 2 - 1,
                        )
                    for q2, sqp2 in enumerate(sqp2s):
                        dr(kvacc[:], ones8[:], sqp2[:], q2 == 0, q2 == KI_KVL // 2 - 1)
                    if t + 1 < S // 512:
                        acc_emit(t + 1)
                    pes = tmp.tile([ROPE, 512], dt.float32, tag="pes", bufs=2)
                    nc.vector.tensor_tensor(pes[:], pt[:], s1rep64[:ROPE, :], MUL)
                    x2h = tmp.tile([32, 512], dt.float32, tag="x2h", bufs=2)
                    nc.sync.dma_start(x2h[:], pes[32:, :])
                    t1 = tmp.tile([32, 512], dt.float32, tag="t1", bufs=2)
                    t2 = tmp.tile([32, 512], dt.float32, tag="t2", bufs=2)
                    o2 = tmp.tile([32, 512], dt.float8e4, tag="o2", bufs=2)
                    nc.vector.tensor_tensor(t1[:], pes[:32, :], cosb[:, tsl], MUL)
                    nc.vector.tensor_tensor(t2[:], x2h[:], sinb[:, tsl], MUL)
                    nc.vector.tensor_tensor(kpe[:32, tsl], t1[:], t2[:], SUB)
                    nc.vector.tensor_tensor(t1[:], x2h[:], cosb[:, tsl], MUL)
                    nc.vector.tensor_tensor(t2[:], pes[:32, :], sinb[:, tsl], MUL)
                    nc.vector.tensor_tensor(o2[:], t1[:], t2[:], ADD)
                    nc.sync.dma_start(kpe[32:, tsl], o2[:])
                    # kv_a rmsnorm rescale -> bf16 latents
                    rkv = rsqrt_tmp(kvacc, KV_LORA, "s1rc")
                    for nf in range(KI_KVL):
                        nc.vector.tensor_tensor(ckv[:, nf, tsl], ckvt[:, nf, :], rkv[:], MUL)

            # ==== attention k/v weight pool created early so hg=0 prefetches ====
            p3 = mid.enter_context(tc.tile_pool(name="p3", bufs=1))
            psC = mid.enter_context(tc.tile_pool(name="psC", bufs=2, space="PSUM"))
            ksb0 = p3.tile([P, 2, S], dt.float8e4, tag="ksb0", bufs=1)
            ksb1 = p3.tile([P, 2, S], dt.float8e4, tag="ksb1", bufs=1)
            ksbs = [ksb0, ksb1]
            for i in range(2):
                nc.vector.memset(ksbs[i][64:, 1, :], 0.0)
            kv_cache = {}

            def kvw_tile(tag, src, hg):
                key = (tag, hg)
                if key not in kv_cache:
                    tt = p3.tile([P, KI_KVL, 512], dt.bfloat16, tag=tag, bufs=2)
                    nc.sync.dma_start(tt[:], src[hg])
                    kv_cache[key] = tt
                return kv_cache.pop(key)

            kv_cache[("wkh", 0)] = p3.tile([P, KI_KVL, 512], dt.bfloat16, tag="wkh", bufs=2, name="wkh_p")
            nc.sync.dma_start(kv_cache[("wkh", 0)][:], w_kv_k[0])
            kv_cache[("wvh", 0)] = p3.tile([P, KI_KVL, 512], dt.bfloat16, tag="wvh", bufs=2, name="wvh_p")
            nc.sync.dma_start(kv_cache[("wvh", 0)][:], w_kv_v[0])

            # ==== phase B: q path ====
            with tc.tile_pool(name="p2", bufs=1) as p2:
                qlat = p2.tile([P, KI_QL, QR], dt.bfloat16)
                qacc = psC.tile([P, QR], dt.float32, tag="acc", bufs=1)
                sqp = None
                for nf in range(KI_QL):
                    wt = wqa_tile(nf)
                    if nf + 1 < KI_QL:
                        wqa_tile_pre = wq_pool.tile([P, KI_H, P], dt.float8e4, tag="wqa", bufs=2)
                        nc.sync.dma_start(wqa_tile_pre[:], w_qa[nf + 1])
                        wq_cache[nf + 1] = wqa_tile_pre
                    pt = ps.tile([P, QR], dt.float32, tag="mm")
                    for kp_ in range(KI_H // 2):
                        dr(
                            pt[:],
                            wt[:, 2 * kp_ : 2 * kp_ + 2, :],
                            xqbf[:, 2 * kp_ : 2 * kp_ + 2, :],
                            kp_ == 0,
                            kp_ == KI_H // 2 - 1,
                        )
                    nc.vector.tensor_tensor(qlat[:, nf, :], pt[:], s1qrep64[:], MUL)
                    if nf % 2 == 0:
                        sqp = tmp.tile([P, 2, QR], dt.float8e4, tag="sqb", bufs=2)
                    nc.vector.tensor_tensor(sqp[:, nf % 2, :], qlat[:, nf, :], qlat[:, nf, :], MUL)
                    if nf % 2 == 1:
                        dr(qacc[:], ones8[:], sqp[:], nf // 2 == 0, nf // 2 == KI_QL // 2 - 1)
                sqrep = p2.tile([P, QR], dt.float32)
                rsqrt_into(qacc, Q_LORA, sqrep)

                # rope tables for q with q_a_ln scale (and 1/64) folded in
                cosq = p2.tile([32, QR], dt.float32)
                sinq = p2.tile([32, QR], dt.float32)
                nc.sync.dma_start(cosq[:], cosTq[:])
                nc.sync.dma_start(sinq[:], sinTq[:])
                nc.vector.tensor_tensor(cosq[:], cosq[:], sqrep[:32, :], MUL)
                nc.vector.tensor_tensor(sinq[:], sinq[:], sqrep[:32, :], MUL)

                for h in range(NH):
                    nc.vector.memset(qpk[64:, h, 1, :], 0.0)
                    wt = p2.tile([P, KI_QL, QHD], dt.bfloat16, tag="wqb", bufs=2)
                    nc.sync.dma_start(wt[:], w_qb[h])
                    pt = ps.tile([P, QR], dt.float32, tag="mm")
                    for ki in range(KI_QL):
                        nc.tensor.matmul(
                            pt[:], wt[:, ki, :NOPE], qlat[:, ki, :],
                            start=(ki == 0), stop=(ki == KI_QL - 1),
                        )
                    nc.vector.tensor_tensor(qpk[:, h, 0, :], pt[:], sqrep[:], MUL)
                    pt2 = ps.tile([ROPE, QR], dt.float32, tag="mm")
                    for ki in range(KI_QL):
                        nc.tensor.matmul(
                            pt2[:], wt[:, ki, NOPE:QHD], qlat[:, ki, :],
                            start=(ki == 0), stop=(ki == KI_QL - 1),
                        )
                    pesq = tmp.tile([ROPE, QR], dt.float32, tag="pes", bufs=2)
                    nc.vector.tensor_copy(pesq[:], pt2[:])
                    x2q = tmp.tile([32, QR], dt.float32, tag="x2h", bufs=2)
                    nc.sync.dma_start(x2q[:], pesq[32:, :])
                    t1 = tmp.tile([32, QR], dt.float32, tag="t1", bufs=2)
                    t2 = tmp.tile([32, QR], dt.float32, tag="t2", bufs=2)
                    o2 = tmp.tile([32, QR], dt.float8e4, tag="o2", bufs=2)
                    nc.vector.tensor_tensor(t1[:], pesq[:32, :], cosq[:], MUL)
                    nc.vector.tensor_tensor(t2[:], x2q[:], sinq[:], MUL)
                    nc.vector.tensor_tensor(qpk[:32, h, 1, :], t1[:], t2[:], SUB)
                    nc.vector.tensor_tensor(t1[:], x2q[:], cosq[:], MUL)
                    nc.vector.tensor_tensor(t2[:], pesq[:32, :], sinq[:], MUL)
                    nc.vector.tensor_tensor(o2[:], t1[:], t2[:], ADD)
                    nc.sync.dma_start(qpk[32:64, h, 1, :], o2[:])

            # ==== phase C: attention ====
            wo_cache = {}

            def wo_tile(nf):
                if nf not in wo_cache:
                    tt = wo_pool.tile([P, NH, VHD], dt.bfloat16, tag="wo", bufs=2)
                    nc.sync.dma_start(tt[:], w_o[nf])
                    wo_cache[nf] = tt
                return wo_cache.pop(nf)

            def normalize(h, av_t, se_t):
                # 1/se as exp(-ln(se)) on the scalar engine; se is
                # pre-replicated [P, QR] by the all-ones se reduction.
                rc = tmp.tile([P, QR], dt.float32, tag="s1r", bufs=2)
                lnt = tmp.tile([P, QR], dt.float32, tag="h1t", bufs=2)
                nc.scalar.activation(out=lnt[:], in_=se_t[:], func=AF.Ln)
                nc.scalar.activation(out=rc[:], in_=lnt[:], func=AF.Exp, scale=-1.0)
                nc.vector.tensor_tensor(attn[:, h, :], av_t[:], rc[:], MUL)

            def k_expand(h, wkh_t):
                # expand k_nope for head h into its packed ksb tile, one head
                # ahead of the score loop so its DVE drains are off the
                # critical path.
                ksb_t = ksbs[h % 2]
                hh_ = h % 4
                for c in range(S // 512):
                    csl = slice(c * 512, (c + 1) * 512)
                    pt = ps.tile([P, 512], dt.float32, tag="mm", name="pt")
                    for l in range(KI_KVL):
                        nc.tensor.matmul(
                            pt[:],
                            wkh_t[:, l, hh_ * P : (hh_ + 1) * P],
                            ckv[:, l, csl],
                            start=(l == 0), stop=(l == KI_KVL - 1),
                        )
                    nc.vector.tensor_copy(ksb_t[:, 0, csl], pt[:])
                nc.vector.tensor_copy(ksb_t[:64, 1, :], kpe[:])

            prev = None
            wkh_cur = kvw_tile("wkh", w_kv_k, 0)
            wvh_cur = kvw_tile("wvh", w_kv_v, 0)
            wkh_nxt = wvh_nxt = None
            k_expand(0, wkh_cur)
            for hg in range(NH // 4):
                if hg + 1 < NH // 4:
                    wkh_nxt = kvw_tile("wkh", w_kv_k, hg + 1)
                    wvh_nxt = kvw_tile("wvh", w_kv_v, hg + 1)
                # v for 4 heads at once
                vsb = p3.tile([P, TK, 4 * VHD], dt.float8e4, tag="vsb", bufs=2)
                for kt in range(TK):
                    pt = ps.tile([P, 4 * VHD], dt.float32, tag="mm")
                    for l in range(KI_KVL):
                        nc.tensor.matmul(
                            pt[:],
                            ckv[:, l, kt * P : (kt + 1) * P],
                            wvh_cur[:, l, :],
                            start=(l == 0), stop=(l == KI_KVL - 1),
                        )
                    nc.vector.tensor_copy(vsb[:, kt, :], pt[:])
                for hh in range(4):
                    h = hg * 4 + hh
                    ksb = ksbs[h % 2]
                    if h + 1 < NH:
                        k_expand(h + 1, wkh_cur if hh < 3 else wkh_nxt)
                    if h == NH - 1:  # prefetch o_proj weights under last head
                        wo_cache[0] = wo_pool.tile([P, NH, VHD], dt.bfloat16, tag="wo", bufs=2, name="wo_p")
                        nc.sync.dma_start(wo_cache[0][:], w_o[0])
                        wo_cache[1] = wo_pool.tile([P, NH, VHD], dt.bfloat16, tag="wo", bufs=2, name="wo_p")
                        nc.sync.dma_start(wo_cache[1][:], w_o[1])
                    if prev is not None:
                        normalize(*prev)
                        prev = None
                    av = psC.tile([P, QR], dt.float32, tag="av")
                    se = psC.tile([P, QR], dt.float32, tag="se")

                    def _drain(p8_, prt, off):
                        dr(se[:, off:], ones8[:], prt[:, :, off:], p8_ == 0, p8_ == TK // 2 - 1)
                        dr(
                            av[:, off:],
                            vsb[:, 2 * p8_ : 2 * p8_ + 2, hh * VHD : (hh + 1) * VHD],
                            prt[:, :, off:],
                            p8_ == 0,
                            p8_ == TK // 2 - 1,
                        )

                    pend = []
                    for p8_ in range(TK // 2):
                        off = (p8_ // 2) * P
                        prt = tmp.tile([P, 2, QR], dt.float8e4, tag="pr", bufs=4)
                        for i in (0, 1):
                            kt = 2 * p8_ + i
                            sc = ps.tile([P, QR], dt.float32, tag="mm")
                            nc.tensor.matmul(
                                sc[:, off:],
                                ksb[:, :, kt * P : (kt + 1) * P],
                                qpk[:, h, :, off:],
                                start=True,
                                stop=True,
                                perf_mode=DRM,
                            )
                            nc.scalar.activation(
                                out=prt[:, i, off:], in_=sc[:, off:], func=AF.Exp,
                                scale=ATTN_SCALE,
                            )
                            nc.vector.tensor_tensor(
                                prt[:, i, off : off + P],
                                prt[:, i, off : off + P],
                                maskt[:, kt, :],
                                MUL,
                            )
                        pend.append((p8_, prt, off))
                        if len(pend) > 2:
                            _drain(*pend.pop(0))
                    while pend:
                        _drain(*pend.pop(0))
                    prev = (h, av, se)
                wkh_cur, wvh_cur = wkh_nxt, wvh_nxt
            normalize(*prev)

        # ==== phase D: o_proj + residual + ln2 (h1 resident in SBUF) ====
        with contextlib.ExitStack() as sc45:
            x2m = sc45.enter_context(tc.tile_pool(name="x2m", bufs=1))
            x2 = x2m.tile([P, KI_H, QR], dt.bfloat16)
            h1sb = x2m.tile([P, KI_H, QR], dt.bfloat16)
            msb = x2m.tile([P, NF_FF, QR], dt.bfloat16)
            p5 = sc45.enter_context(tc.tile_pool(name="p5", bufs=1))
            ffn_cache = {}

            def ffn_tile(tag, src, nf):
                key = (tag, nf)
                if key not in ffn_cache:
                    tt = p5.tile([P, KI_H, P], dt.bfloat16, tag=tag, bufs=2)
                    nc.sync.dma_start(tt[:], src[nf])
                    ffn_cache[key] = tt
                return ffn_cache.pop(key)

            ffn_cache[("wg", 0)] = p5.tile([P, KI_H, P], dt.bfloat16, tag="wg", bufs=2, name="wg_p")
            nc.sync.dma_start(ffn_cache[("wg", 0)][:], w_g[0])
            ffn_cache[("wu", 0)] = p5.tile([P, KI_H, P], dt.bfloat16, tag="wu", bufs=2, name="wu_p")
            nc.sync.dma_start(ffn_cache[("wu", 0)][:], w_u[0])


            with tc.tile_pool(name="psD", bufs=1, space="PSUM") as psD:
                oacc = psD.tile([P, QR], dt.float32, tag="acc")
                for nf in range(KI_H):
                    wt = wo_tile(nf)
                    if nf + 1 < KI_H and nf + 1 not in wo_cache:
                        wo_cache[nf + 1] = wo_pool.tile([P, NH, VHD], dt.bfloat16, tag="wo", bufs=2, name="wo_p")
                        nc.sync.dma_start(wo_cache[nf + 1][:], w_o[nf + 1])
                    pt = ps.tile([P, QR], dt.float32, tag="mm")
                    for kh in range(NH):
                        nc.tensor.matmul(
                            pt[:],
                            wt[:, kh, :],
                            attn[:, kh, :],
                            start=(kh == 0),
                            stop=(kh == NH - 1),
                        )
                    ht = ld.tile([P, QR], dt.float32, tag="hload")
                    nc.sync.dma_start(ht[:], hTq[nf * P : (nf + 1) * P, :])
                    nc.vector.tensor_tensor(h1sb[:, nf, :], pt[:], ht[:], ADD)
                    sq = tmp.tile([P, QR], dt.bfloat16, tag="sq")
                    nc.vector.tensor_tensor(sq[:], h1sb[:, nf, :], h1sb[:, nf, :], MUL)
                    nc.tensor.matmul(
                        oacc[:], ones_bfP[:], sq[:], start=(nf == 0), stop=(nf == KI_H - 1)
                    )
                s2rep = tmp.tile([P, QR], dt.float32, tag="s1r", bufs=2)
                rsqrt_into(oacc, H, s2rep)
                for nf in range(KI_H):
                    nc.vector.tensor_tensor(x2[:, nf, :], h1sb[:, nf, :], s2rep[:], MUL)

            # ==== phase E: FFN (SwiGLU), bf16 ====
            for nf in range(NF_FF):
                wtg = ffn_tile("wg", w_g, nf)
                if nf + 1 < NF_FF:
                    ffn_cache[("wg", nf + 1)] = p5.tile([P, KI_H, P], dt.bfloat16, tag="wg", bufs=2, name="wg_p")
                    nc.sync.dma_start(ffn_cache[("wg", nf + 1)][:], w_g[nf + 1])
                pg = ps.tile([P, QR], dt.float32, tag="mm")
                for ki in range(KI_H):
                    nc.tensor.matmul(
                        pg[:], wtg[:, ki, :], x2[:, ki, :],
                        start=(ki == 0), stop=(ki == KI_H - 1),
                    )
                gs = tmp.tile([P, QR], dt.bfloat16, tag="sq")
                nc.scalar.activation(out=gs[:], in_=pg[:], func=AF.Silu)
                wtu = ffn_tile("wu", w_u, nf)
                if nf + 1 < NF_FF:
                    ffn_cache[("wu", nf + 1)] = p5.tile([P, KI_H, P], dt.bfloat16, tag="wu", bufs=2, name="wu_p")
                    nc.sync.dma_start(ffn_cache[("wu", nf + 1)][:], w_u[nf + 1])
                pu = ps.tile([P, QR], dt.float32, tag="mm")
                for ki in range(KI_H):
                    nc.tensor.matmul(
                        pu[:], wtu[:, ki, :], x2[:, ki, :],
                        start=(ki == 0), stop=(ki == KI_H - 1),
                    )
                nc.vector.tensor_tensor(msb[:, nf, :], pu[:], gs[:], MUL)

            for nf in range(KI_H):
                pt = ps.tile([P, QR], dt.float32, tag="mm")
                for qtr in range(4):
                    wt = p5.tile([P, NF_FF // 4, P], dt.bfloat16, tag="wd", bufs=2)
                    nc.sync.dma_start(wt[:], w_d[nf, :, qtr * 16 : (qtr + 1) * 16, :])
                    for ki in range(NF_FF // 4):
                        kk = qtr * 16 + ki
                        nc.tensor.matmul(
                            pt[:], wt[:, ki, :], msb[:, kk, :],
                            start=(kk == 0), stop=(kk == NF_FF - 1),
                        )
                ot = tmp.tile([P, QR], dt.float32, tag="h1t", bufs=2)
                nc.vector.tensor_tensor(ot[:], pt[:], h1sb[:, nf, :], ADD)
                nc.sync.dma_start(out[nf * P : (nf + 1) * P, :], ot[:])

    return nc


# ---------------------------------------------------------------------------
# host-side packing
# ---------------------------------------------------------------------------
def _deint_perm():
    # deinterleave: out[i] = in[2i] (i<32), in[2(i-32)+1] (i>=32)
    return np.concatenate([np.arange(0, ROPE, 2), np.arange(1, ROPE, 2)])


def _pack_lhst(w, nki, nnf, nfree=P, dtype=BF16, scale=1.0):
    # w [nki*P, nnf*nfree] -> [nnf, P, nki, nfree]
    return np.ascontiguousarray(
        (w.reshape(nki, P, nnf, nfree) * scale).transpose(2, 1, 0, 3).astype(dtype)
    )


def _prep_shared(inputs):
    perm = _deint_perm()
    ln1 = inputs["ln1_w"].astype(np.float32)
    qaln = inputs["q_a_ln_w"].astype(np.float32)
    kvln = inputs["kv_a_ln_w"].astype(np.float32)
    ln2 = inputs["ln2_w"].astype(np.float32)

    w_qa = inputs["q_a_kernel"].astype(np.float32) * ln1[:, None]
    w_kva = inputs["kv_a_kernel"].astype(np.float32) * ln1[:, None]
    w_kva = w_kva.copy()
    w_kva[:, KV_LORA:] = w_kva[:, KV_LORA:][:, perm]
    w_qb = inputs["q_b_kernel"].astype(np.float32) * qaln[:, None]
    w_qb = w_qb.copy()
    for h in range(NH):
        blk = slice(h * QHD + NOPE, (h + 1) * QHD)
        w_qb[:, blk] = w_qb[:, blk][:, perm]
    w_kvb = inputs["kv_b_kernel"].astype(np.float32) * kvln[:, None]
    w_o = inputs["o_kernel"].astype(np.float32)
    w_g = inputs["gate_kernel"].astype(np.float32) * ln2[:, None]
    w_u = inputs["up_kernel"].astype(np.float32) * ln2[:, None]
    w_d = inputs["down_kernel"].astype(np.float32)

    shared = {
        "w_qa": _pack_lhst(w_qa, KI_H, KI_QL, P, F8, WS),
        # w_qb: [NH, P, KI_QL, QHD]
        "w_qb": np.ascontiguousarray(
            w_qb.reshape(KI_QL, P, NH, QHD).transpose(2, 1, 0, 3).astype(BF16)
        ),
        # w_kva resident: [P, KI_H, 576]
        "w_kva": np.ascontiguousarray(
            (w_kva.reshape(KI_H, P, KV_LORA + ROPE) * WS).transpose(1, 0, 2).astype(F8)
        ),
        # w_kvb split into k/v halves, packed per head-group of 4:
        "w_kv_k": np.ascontiguousarray(
            w_kvb.reshape(KI_KVL, P, NH // 4, 4, 2, 128)[:, :, :, :, 0, :]
            .transpose(2, 1, 0, 3, 4)
            .reshape(NH // 4, P, KI_KVL, 512)
            .astype(BF16)
        ),
        "w_kv_v": np.ascontiguousarray(
            w_kvb.reshape(KI_KVL, P, NH // 4, 4, 2, 128)[:, :, :, :, 1, :]
            .transpose(2, 1, 0, 3, 4)
            .reshape(NH // 4, P, KI_KVL, 512)
            .astype(BF16)
        ),
        # w_o: [KI_H(nf), P, NH, VHD]
        "w_o": np.ascontiguousarray(
            w_o.reshape(NH, VHD, KI_H, P).transpose(2, 1, 0, 3).astype(BF16)
        ),
        "w_g": _pack_lhst(w_g, KI_H, NF_FF),
        "w_u": _pack_lhst(w_u, KI_H, NF_FF),
        "w_d": _pack_lhst(w_d, NF_FF, KI_H),
    }
    return shared


def _prep_batch(inputs, b):
    hid = np.asarray(inputs["hidden_states"][b], dtype=np.float32)  # [S, H]
    hT = np.ascontiguousarray(hid.T)  # [H, S] f32
    hT8 = hT.astype(F8)
    pos = np.asarray(inputs["position_ids"][b]).astype(np.int64)
    cos_g = np.asarray(inputs["cos"], dtype=np.float32)[pos][:, :32]  # [S, 32]
    sin_g = np.asarray(inputs["sin"], dtype=np.float32)[pos][:, :32]
    return hT, hT8, np.ascontiguousarray(cos_g.T), np.ascontiguousarray(sin_g.T)


def _core_rows(j):
    return np.concatenate(
        [np.arange((j + 4 * i) * P, (j + 4 * i + 1) * P) for i in range(4)]
    )


def _core_masks(j):
    # one 128-col mask block per key tile, applied at query slot kt//4
    kp = np.arange(P)[:, None]
    qf = np.arange(P)[None, :]
    m = np.zeros((P, TK, P), dtype=F8)
    for kt in range(TK):
        r = kt % 4
        if j > r:
            m[:, kt, :] = 1.0
        elif j == r:
            m[:, kt, :] = (kp <= qf).astype(F8)
        # j < r: stays zero
    return m


def kernel(**inputs) -> np.ndarray:
    import concourse.bass as bass  # noqa: F401  (env check)
    from concourse.bass_utils import run_bass_kernel_spmd

    if "nc" not in _COMPILED:
        _COMPILED["nc"] = _build_nc()
    nc = _COMPILED["nc"]

    shared = _prep_shared(inputs)
    in_maps = []
    per_batch = [_prep_batch(inputs, b) for b in range(B)]
    for c in range(8):
        b, j = c // 4, c % 4
        hT, hT8, cosTb, sinTb = per_batch[b]
        rows = _core_rows(j)
        in_map = dict(shared)
        in_map["hTb"] = hT8
        in_map["hTqb"] = np.ascontiguousarray(hT8[:, rows])
        in_map["hTq"] = np.ascontiguousarray(hT[:, rows])
        in_map["cosT"] = cosTb
        in_map["sinT"] = sinTb
        in_map["cosTq"] = np.ascontiguousarray(cosTb[:, rows])
        in_map["sinTq"] = np.ascontiguousarray(sinTb[:, rows])
        in_map["masks"] = _core_masks(j)
        in_maps.append(in_map)

    res = run_bass_kernel_spmd(nc, in_maps, core_ids=list(range(8)))
    globals()["LAST_RESULT"] = res

    out = np.empty((B, S, H), dtype=np.float32)
    for c in range(8):
        b, j = c // 4, c % 4
        out[b, _core_rows(j), :] = res.results[c]["out"].T
    return out


# revision 25
# speedup vs baseline: 1.0574x; 1.0530x over previous
"""DeepseekV2 decoder layer (MLA attention + SwiGLU MLP) on 8 TRN2 NeuronCores.

Sharding: core c -> batch b = c//4, query tiles {j, j+4, j+8, j+12} (j = c%4,
128 rows each, causally interleaved). The interleave makes the causal
structure identical on every core: key tile kt is only needed by query slots
s >= kt//4, so the SPMD program skips 37.5% of score/AV work uniformly.

fp8 (e4m3, DoubleRow 2x matmul) is used for the hidden-state input, the
q_a/kv_a projections, the packed q/k score tiles, v, and the attention
probabilities; q_b/kv_b, o_proj and the FFN stay bf16 (fp8 there blows the
2e-2 error budget). fp8 weights are scaled x16 on the host to escape
denormals; the 1/16 compensation is folded into the rsqrt stat scales
(ws64 variants), so no extra ops are spent.

Per-head packed layout for scores: k/q tiles [128, 2, n] where pair slot 0 =
nope (128 feats), slot 1 = rope (64 feats) + 64 zero rows, so one DoubleRow
matmul computes a whole 192-dim score tile.
"""

import json

import numpy as np
import ml_dtypes

B, S, H = 2, 2048, 2048
NH = 16
Q_LORA = 1536
KV_LORA = 512
NOPE = 128
ROPE = 64
QHD = NOPE + ROPE  # 192
VHD = 128
FF = 8192
EPS = 1e-6
P = 128
QR = 512  # query rows per core
TK = S // P  # 16 key tiles
KI_H = H // P  # 16
KI_QL = Q_LORA // P  # 12
KI_KVL = KV_LORA // P  # 4
NF_FF = FF // P  # 64
ATTN_SCALE = QHD ** -0.5
WS = 16.0  # fp8 weight scale (16x: escapes denormals; keeps 16x-scaled k_pe/v tails far below fp8's 240 max)

BF16 = ml_dtypes.bfloat16
F8 = ml_dtypes.float8_e4m3

_COMPILED = {}


# ---------------------------------------------------------------------------
# compiler workaround: this container's walrus rejects >1 sem wait per
# instruction; split extra waits onto single-wait NoOps.
# ---------------------------------------------------------------------------
def _install_multiwait_fix(bass):
    if getattr(bass.Bass, "_multiwait_fix_installed", False):
        return
    orig = bass.Bass.to_json_bytes

    def _split(m):
        for f in m.get("functions", []):
            for b in f.get("blocks", []):
                out = []
                for inst in b.get("instructions", []):
                    si = inst.get("sync_info") or {}
                    waits = si.get("on_wait") or []
                    if len(waits) > 1:
                        for k, w in enumerate(waits[:-1]):
                            out.append(
                                {
                                    "debug": inst.get("debug", 0),
                                    "engine": inst["engine"],
                                    "ins": [],
                                    "name": f"{inst['name']}_w{k}",
                                    "opcode": "NoOp",
                                    "outs": [],
                                    "sync_info": {"on_update": [], "on_wait": [w]},
                                }
                            )
                        si["on_wait"] = [waits[-1]]
                    out.append(inst)
                b["instructions"] = out
        return m

    def patched(self):
        raw = orig(self)
        try:
            return json.dumps(_split(json.loads(raw))).encode()
        except Exception:
            return raw

    bass.Bass.to_json_bytes = patched
    bass.Bass._multiwait_fix_installed = True


def _install_drain_fix(tile, ScopedClock, VectorClock):
    if getattr(tile.TileContext, "_drain_fix_installed", False):
        return

    def _drain_and_barrier(self, tick_clock, wait_clock):
        gc = tick_clock.global_clock
        n = len(gc)
        for p in range(n):
            t = gc[p]
            if t > 0:
                vc = VectorClock([0] * n)
                vc.require_at_least(p, t)
                d = self.nc.sync.drain()
                wait_clock.add_sem_waits(d.ins, ScopedClock({None: vc}))
        self.nc.all_engine_barrier()
        popped = self.nc._tile_sem_poison_stack.pop()
        assert popped is self._sem_poison
        self.nc.clear_and_free_semaphores(list(self.sems.allocated().values()))
        self.nc.all_engine_barrier()

    tile.TileContext._drain_and_barrier = _drain_and_barrier
    tile.TileContext._drain_fix_installed = True


# ---------------------------------------------------------------------------
# device program
# ---------------------------------------------------------------------------
def _build_nc():
    import concourse.bass as bass
    import concourse.mybir as mybir
    import concourse.tile as tile
    from concourse.vector_clock import ScopedClock, VectorClock

    _install_multiwait_fix(bass)
    _install_drain_fix(tile, ScopedClock, VectorClock)

    dt = mybir.dt
    AF = mybir.ActivationFunctionType
    MUL = mybir.AluOpType.mult
    ADD = mybir.AluOpType.add
    SUB = mybir.AluOpType.subtract
    DRM = mybir.MatmulPerfMode.DoubleRow

    nc = bass.Bass()

    # register EPS so float bias=EPS works on the scalar engine
    _eps_t = nc.alloc_sbuf_tensor(f"const-float32-{EPS}", [128, 1], dt.float32)
    nc.gpsimd.memset(_eps_t.ap(), EPS)
    nc.const_aps.aps[(dt.float32, EPS)] = _eps_t.ap()
    EPS64 = EPS * WS * WS
    _eps64_t = nc.alloc_sbuf_tensor(f"const-float32-{EPS64}", [128, 1], dt.float32)
    nc.gpsimd.memset(_eps64_t.ap(), EPS64)
    nc.const_aps.aps[(dt.float32, EPS64)] = _eps64_t.ap()
    import math
    NLNWS = -math.log(WS)
    _nlnws_t = nc.alloc_sbuf_tensor(f"const-float32-{NLNWS}", [128, 1], dt.float32)
    nc.gpsimd.memset(_nlnws_t.ap(), NLNWS)
    nc.const_aps.aps[(dt.float32, NLNWS)] = _nlnws_t.ap()
    nc.all_engine_barrier()

    # ---- inputs ----
    hTb = nc.dram_tensor("hTb", [H, S], dt.float8e4, kind="ExternalInput")
    hTqb = nc.dram_tensor("hTqb", [H, QR], dt.float8e4, kind="ExternalInput")
    hTq = nc.dram_tensor("hTq", [H, QR], dt.float32, kind="ExternalInput")
    cosT = nc.dram_tensor("cosT", [32, S], dt.float32, kind="ExternalInput")
    sinT = nc.dram_tensor("sinT", [32, S], dt.float32, kind="ExternalInput")
    cosTq = nc.dram_tensor("cosTq", [32, QR], dt.float32, kind="ExternalInput")
    sinTq = nc.dram_tensor("sinTq", [32, QR], dt.float32, kind="ExternalInput")
    masks = nc.dram_tensor("masks", [P, TK, P], dt.float8e4, kind="ExternalInput")
    w_qa = nc.dram_tensor("w_qa", [KI_QL, P, KI_H, P], dt.float8e4, kind="ExternalInput")
    w_qb = nc.dram_tensor("w_qb", [NH, P, KI_QL, QHD], dt.bfloat16, kind="ExternalInput")
    w_kva = nc.dram_tensor("w_kva", [P, KI_H, KV_LORA + ROPE], dt.float8e4, kind="ExternalInput")
    w_kv_k = nc.dram_tensor("w_kv_k", [NH // 4, P, KI_KVL, 512], dt.bfloat16, kind="ExternalInput")
    w_kv_v = nc.dram_tensor("w_kv_v", [NH // 4, P, KI_KVL, 512], dt.bfloat16, kind="ExternalInput")
    w_o = nc.dram_tensor("w_o", [KI_H, P, NH, VHD], dt.bfloat16, kind="ExternalInput")
    w_g = nc.dram_tensor("w_g", [NF_FF, P, KI_H, P], dt.bfloat16, kind="ExternalInput")
    w_u = nc.dram_tensor("w_u", [NF_FF, P, KI_H, P], dt.bfloat16, kind="ExternalInput")
    w_d = nc.dram_tensor("w_d", [KI_H, P, NF_FF, P], dt.bfloat16, kind="ExternalInput")
    out = nc.dram_tensor("out", [H, QR], dt.float32, kind="ExternalOutput")

    import contextlib

    with tile.TileContext(nc) as tc, contextlib.ExitStack() as top:
        tp = lambda **kw: top.enter_context(tc.tile_pool(**kw))
        ones = tp(name="ones", bufs=1)
        tmp = tp(name="tmp", bufs=3)
        ld = tp(name="ld", bufs=3)
        ps = tp(name="ps", bufs=3, space="PSUM")
        attn_pool = tp(name="attn_pool", bufs=1)
        attn = attn_pool.tile([P, NH, QR], dt.bfloat16)
        maskp = tp(name="maskp", bufs=1)
        maskt = maskp.tile([P, TK, P], dt.float8e4)
        wo_pool = tp(name="wo_pool", bufs=2)

        ones_bfP = ones.tile([P, P], dt.bfloat16)
        nc.vector.memset(ones_bfP[:], 1.0)
        ones8 = ones.tile([P, 2, P], dt.float8e4)
        nc.vector.memset(ones8[:], 1.0)

        def dr(o, l, r, start, stop):
            nc.tensor.matmul(o, l, r, start=start, stop=stop, perf_mode=DRM)

        def rsqrt_into(acc, denom, dst, ws64=False):
            # dst = 1/sqrt(mean + eps) (optionally a further 1/WS) computed as
            # exp(-0.5*ln(.)) -- two scalar-engine table ops, avoiding the
            # 3.3us DVE reciprocal that would serialize the vector queue.
            lnt = tmp.tile([P, acc.shape[-1]], dt.float32, tag="h1t", bufs=2)
            nc.scalar.activation(
                out=lnt[:], in_=acc[:], func=AF.Ln, bias=EPS, scale=1.0 / denom
            )
            if ws64:
                nc.scalar.activation(out=dst[:], in_=lnt[:], func=AF.Exp, scale=-0.5, bias=NLNWS)
            else:
                nc.scalar.activation(out=dst[:], in_=lnt[:], func=AF.Exp, scale=-0.5)

        def rsqrt_tmp(acc, denom, tag, ws64=False):
            d = tmp.tile([P, acc.shape[-1]], dt.float32, tag=tag, bufs=2)
            rsqrt_into(acc, denom, d, ws64)
            return d

        with contextlib.ExitStack() as mid:
            lat = mid.enter_context(tc.tile_pool(name="lat", bufs=1))
            ckv = lat.tile([P, KI_KVL, S], dt.bfloat16)  # normalized kv latents (1x, bf16)
            kpe = lat.tile([ROPE, S], dt.float8e4)  # roped shared key-pe (1x)
            pA = mid.enter_context(tc.tile_pool(name="pA", bufs=1))
            xqbf = pA.tile([P, KI_H, QR], dt.float8e4)
            s1qrep64 = pA.tile([P, QR], dt.float32)
            qpk_pool = mid.enter_context(tc.tile_pool(name="qpk", bufs=1))
            qpk = qpk_pool.tile([P, NH, 2, QR], dt.float8e4)
            wq_pool = mid.enter_context(tc.tile_pool(name="wq", bufs=1))

            wq_cache = {}

            def wqa_tile(nf):
                if nf not in wq_cache:
                    t = wq_pool.tile([P, KI_H, P], dt.float8e4, tag="wqa", bufs=2)
                    nc.sync.dma_start(t[:], w_qa[nf])
                    wq_cache[nf] = t
                return wq_cache.pop(nf)

            # ==== phase A: ln1 stats + kv latents + shared roped k_pe ====
            with tc.tile_pool(name="pB", bufs=1) as pB, tc.tile_pool(
                name="psA", bufs=2, space="PSUM"
            ) as psA:
                nc.sync.dma_start(maskt[:], masks[:])
                for ki in range(KI_H):
                    nc.sync.dma_start(xqbf[:, ki, :], hTqb[ki * P : (ki + 1) * P, :])
                wkva = pB.tile([P, KI_H, KV_LORA + ROPE], dt.float8e4)
                nc.sync.dma_start(wkva[:], w_kva[:])
                cosb = pB.tile([32, S], dt.float32)
                sinb = pB.tile([32, S], dt.float32)
                nc.sync.dma_start(cosb[:], cosT[:])
                nc.sync.dma_start(sinb[:], sinT[:])
                # prefetch first q_a weights for phase B
                wqa_pre = wq_pool.tile([P, KI_H, P], dt.float8e4, tag="wqa", bufs=2)
                nc.sync.dma_start(wqa_pre[:], w_qa[0])
                wq_cache[0] = wqa_pre

                # chunk pipeline: squares for chunk t+1 run on the SCALAR
                # engine while the PE does chunk t's kv matmuls and the DVE
                # does chunk t's PSUM drains; acc DRs for t+1 are emitted after
                # kv work so the PE never waits on the square/stat chain.
                xcs = {}
                s1s = {}
                sq_tiles = {}

                def xc_dma(t):
                    tsl = slice(t * 512, (t + 1) * 512)
                    xc = pB.tile([P, KI_H, 512], dt.float8e4, tag="xc", bufs=3, name="xc")
                    for ki in range(KI_H):
                        nc.sync.dma_start(xc[:, ki, :], hTb[ki * P : (ki + 1) * P, tsl])
                    xcs[t] = xc

                def sq_emit(t):
                    xc = xcs[t]
                    tiles = []
                    for kp_ in range(KI_H // 2):
                        sqp = pB.tile([P, 2, 512], dt.float8e4, tag="sqp", bufs=8, name="sqp")
                        nc.scalar.activation(
                            out=sqp[:], in_=xc[:, 2 * kp_ : 2 * kp_ + 2, :], func=AF.Square
                        )
                        tiles.append(sqp)
                    sq_tiles[t] = tiles

                def acc_emit(t):
                    tiles = sq_tiles.pop(t)
                    acc = psA.tile([P, 512], dt.float32, tag="acc", name="acc")
                    for kp_, sqp in enumerate(tiles):
                        dr(acc[:], ones8[:], sqp[:], kp_ == 0, kp_ == KI_H // 2 - 1)
                    s1s[t] = rsqrt_tmp(acc, H, "s1rb", ws64=True)

                xc_dma(0)
                xc_dma(1)
                sq_emit(0)

                # q-slice ln1 stats on the DVE, overlapping the scalar squares
                accq = psA.tile([P, QR], dt.float32, tag="acc")
                for kp_ in range(KI_H // 2):
                    sqq = pB.tile([P, 2, QR], dt.float8e4, tag="sqq", bufs=2)
                    for i in (0, 1):
                        ki = 2 * kp_ + i
                        nc.vector.tensor_tensor(sqq[:, i, :], xqbf[:, ki, :], xqbf[:, ki, :], MUL)
                    dr(accq[:], ones8[:], sqq[:], kp_ == 0, kp_ == KI_H // 2 - 1)
                rsqrt_into(accq, H, s1qrep64, ws64=True)
                acc_emit(0)

                for t in range(S // 512):
                    tsl = slice(t * 512, (t + 1) * 512)
                    if t + 2 < S // 512:
                        xc_dma(t + 2)
                    if t + 1 < S // 512:
                        sq_emit(t + 1)
                    xc = xcs.pop(t)
                    s1rep64 = s1s.pop(t)
                    ckvt = pB.tile([P, KI_KVL, 512], dt.bfloat16, tag="ckvt", bufs=2)
                    kvacc = psA.tile([P, 512], dt.float32, tag="acc2")
                    sqp2s = []
                    sqp2 = None
                    for nf in range(KI_KVL):
                        pt = ps.tile([P, 512], dt.float32, tag="mm")
                        for kp_ in range(KI_H // 2):
                            dr(
                                pt[:],
                                wkva[:, 2 * kp_ : 2 * kp_ + 2, nf * P : (nf + 1) * P],
                                xc[:, 2 * kp_ : 2 * kp_ + 2, :],
                                kp_ == 0,
                                kp_ == KI_H // 2 - 1,
                            )
                        nc.vector.tensor_tensor(ckvt[:, nf, :], pt[:], s1rep64[:], MUL)
                        if nf % 2 == 0:
                            sqp2 = pB.tile([P, 2, 512], dt.float8e4, tag="sqp2", bufs=2)
                            sqp2s.append(sqp2)
                        nc.vector.tensor_tensor(
                            sqp2[:, nf % 2, :], ckvt[:, nf, :], ckvt[:, nf, :], MUL
                        )
                    # k_pe: last 64 cols of w_kva (1x true scale via s1rep64)
                    pt = ps.tile([ROPE, 512], dt.float32, tag="mm")
                    for kp_ in range(KI_H // 2):
                        dr(
                            pt[:],
                            wkva[:, 2 * kp_ : 2 * kp_ + 2, KV_LORA : KV_LORA + ROPE],
                            xc[:, 2 * kp_ : 2 * kp_ + 2, :],
                            kp_ == 0,
                            kp_ == KI_H // 2 - 1,
                        )
                    for q2, sqp2 in enumerate(sqp2s):
                        dr(kvacc[:], ones8[:], sqp2[:], q2 == 0, q2 == KI_KVL // 2 - 1)
                    if t + 1 < S // 512:
                        acc_emit(t + 1)
                    pes = tmp.tile([ROPE, 512], dt.float32, tag="pes", bufs=2)
                    nc.vector.tensor_tensor(pes[:], pt[:], s1rep64[:ROPE, :], MUL)
                    x2h = tmp.tile([32, 512], dt.float32, tag="x2h", bufs=2)
                    nc.sync.dma_start(x2h[:], pes[32:, :])
                    t1 = tmp.tile([32, 512], dt.float32, tag="t1", bufs=2)
                    t2 = tmp.tile([32, 512], dt.float32, tag="t2", bufs=2)
                    o2 = tmp.tile([32, 512], dt.float8e4, tag="o2", bufs=2)
                    nc.vector.tensor_tensor(t1[:], pes[:32, :], cosb[:, tsl], MUL)
                    nc.vector.tensor_tensor(t2[:], x2h[:], sinb[:, tsl], MUL)
                    nc.vector.tensor_tensor(kpe[:32, tsl], t1[:], t2[:], SUB)
                    nc.vector.tensor_tensor(t1[:], x2h[:], cosb[:, tsl], MUL)
                    nc.vector.tensor_tensor(t2[:], pes[:32, :], sinb[:, tsl], MUL)
                    nc.vector.tensor_tensor(o2[:], t1[:], t2[:], ADD)
                    nc.sync.dma_start(kpe[32:, tsl], o2[:])
                    # kv_a rmsnorm rescale -> bf16 latents
                    rkv = rsqrt_tmp(kvacc, KV_LORA, "s1rc")
                    for nf in range(KI_KVL):
                        nc.vector.tensor_tensor(ckv[:, nf, tsl], ckvt[:, nf, :], rkv[:], MUL)

            # ==== attention k/v weight pool created early so hg=0 prefetches ====
            p3 = mid.enter_context(tc.tile_pool(name="p3", bufs=1))
            psC = mid.enter_context(tc.tile_pool(name="psC", bufs=2, space="PSUM"))
            ksb0 = p3.tile([P, 2, S], dt.float8e4, tag="ksb0", bufs=1)
            ksb1 = p3.tile([P, 2, S], dt.float8e4, tag="ksb1", bufs=1)
            ksbs = [ksb0, ksb1]
            for i in range(2):
                nc.vector.memset(ksbs[i][64:, 1, :], 0.0)
            kv_cache = {}

            def kvw_tile(tag, src, hg):
                key = (tag, hg)
                if key not in kv_cache:
                    tt = p3.tile([P, KI_KVL, 512], dt.bfloat16, tag=tag, bufs=2)
                    nc.sync.dma_start(tt[:], src[hg])
                    kv_cache[key] = tt
                return kv_cache.pop(key)

            kv_cache[("wkh", 0)] = p3.tile([P, KI_KVL, 512], dt.bfloat16, tag="wkh", bufs=2, name="wkh_p")
            nc.sync.dma_start(kv_cache[("wkh", 0)][:], w_kv_k[0])
            kv_cache[("wvh", 0)] = p3.tile([P, KI_KVL, 512], dt.bfloat16, tag="wvh", bufs=2, name="wvh_p")
            nc.sync.dma_start(kv_cache[("wvh", 0)][:], w_kv_v[0])

            # ==== phase B: q path ====
            with tc.tile_pool(name="p2", bufs=1) as p2:
                qlat = p2.tile([P, KI_QL, QR], dt.bfloat16)
                qacc = psC.tile([P, QR], dt.float32, tag="acc", bufs=1)
                sqp = None
                for nf in range(KI_QL):
                    wt = wqa_tile(nf)
                    if nf + 1 < KI_QL:
                        wqa_tile_pre = wq_pool.tile([P, KI_H, P], dt.float8e4, tag="wqa", bufs=2)
                        nc.sync.dma_start(wqa_tile_pre[:], w_qa[nf + 1])
                        wq_cache[nf + 1] = wqa_tile_pre
                    pt = ps.tile([P, QR], dt.float32, tag="mm")
                    for kp_ in range(KI_H // 2):
                        dr(
                            pt[:],
                            wt[:, 2 * kp_ : 2 * kp_ + 2, :],
                            xqbf[:, 2 * kp_ : 2 * kp_ + 2, :],
                            kp_ == 0,
                            kp_ == KI_H // 2 - 1,
                        )
                    nc.vector.tensor_tensor(qlat[:, nf, :], pt[:], s1qrep64[:], MUL)
                    if nf % 2 == 0:
                        sqp = tmp.tile([P, 2, QR], dt.float8e4, tag="sqb", bufs=2)
                    nc.vector.tensor_tensor(sqp[:, nf % 2, :], qlat[:, nf, :], qlat[:, nf, :], MUL)
                    if nf % 2 == 1:
                        dr(qacc[:], ones8[:], sqp[:], nf // 2 == 0, nf // 2 == KI_QL // 2 - 1)
                sqrep = p2.tile([P, QR], dt.float32)
                rsqrt_into(qacc, Q_LORA, sqrep)

                # rope tables for q with q_a_ln scale (and 1/64) folded in
                cosq = p2.tile([32, QR], dt.float32)
                sinq = p2.tile([32, QR], dt.float32)
                nc.sync.dma_start(cosq[:], cosTq[:])
                nc.sync.dma_start(sinq[:], sinTq[:])
                nc.vector.tensor_tensor(cosq[:], cosq[:], sqrep[:32, :], MUL)
                nc.vector.tensor_tensor(sinq[:], sinq[:], sqrep[:32, :], MUL)

                for h in range(NH):
                    nc.vector.memset(qpk[64:, h, 1, :], 0.0)
                    wt = p2.tile([P, KI_QL, QHD], dt.bfloat16, tag="wqb", bufs=2)
                    nc.sync.dma_start(wt[:], w_qb[h])
                    pt = ps.tile([P, QR], dt.float32, tag="mm")
                    for ki in range(KI_QL):
                        nc.tensor.matmul(
                            pt[:], wt[:, ki, :NOPE], qlat[:, ki, :],
                            start=(ki == 0), stop=(ki == KI_QL - 1),
                        )
                    nc.vector.tensor_tensor(qpk[:, h, 0, :], pt[:], sqrep[:], MUL)
                    pt2 = ps.tile([ROPE, QR], dt.float32, tag="mm")
                    for ki in range(KI_QL):
                        nc.tensor.matmul(
                            pt2[:], wt[:, ki, NOPE:QHD], qlat[:, ki, :],
                            start=(ki == 0), stop=(ki == KI_QL - 1),
                        )
                    pesq = tmp.tile([ROPE, QR], dt.float32, tag="pes", bufs=2)
                    nc.vector.tensor_copy(pesq[:], pt2[:])
                    x2q = tmp.tile([32, QR], dt.float32, tag="x2h", bufs=2)
                    nc.sync.dma_start(x2q[:], pesq[32:, :])
                    t1 = tmp.tile([32, QR], dt.float32, tag="t1", bufs=2)
                    t2 = tmp.tile([32, QR], dt.float32, tag="t2", bufs=2)
                    o2 = tmp.tile([32, QR], dt.float8e4, tag="o2", bufs=2)
                    nc.vector.tensor_tensor(t1[:], pesq[:32, :], cosq[:], MUL)
                    nc.vector.tensor_tensor(t2[:], x2q[:], sinq[:], MUL)
                    nc.vector.tensor_tensor(qpk[:32, h, 1, :], t1[:], t2[:], SUB)
                    nc.vector.tensor_tensor(t1[:], x2q[:], cosq[:], MUL)
                    nc.vector.tensor_tensor(t2[:], pesq[:32, :], sinq[:], MUL)
                    nc.vector.tensor_tensor(o2[:], t1[:], t2[:], ADD)
                    nc.sync.dma_start(qpk[32:64, h, 1, :], o2[:])

            # ==== phase C: attention ====
            wo_cache = {}

            def wo_tile(nf):
                if nf not in wo_cache:
                    tt = wo_pool.tile([P, NH, VHD], dt.bfloat16, tag="wo", bufs=2)
                    nc.sync.dma_start(tt[:], w_o[nf])
                    wo_cache[nf] = tt
                return wo_cache.pop(nf)

            def normalize(h, av_t, se_t):
                # 1/se as exp(-ln(se)) on the scalar engine; se is
                # pre-replicated [P, QR] by the all-ones se reduction.
                rc = tmp.tile([P, QR], dt.float32, tag="s1r", bufs=2)
                lnt = tmp.tile([P, QR], dt.float32, tag="h1t", bufs=2)
                nc.scalar.activation(out=lnt[:], in_=se_t[:], func=AF.Ln)
                nc.scalar.activation(out=rc[:], in_=lnt[:], func=AF.Exp, scale=-1.0)
                nc.vector.tensor_tensor(attn[:, h, :], av_t[:], rc[:], MUL)

            def k_expand(h, wkh_t):
                # expand k_nope for head h into its packed ksb tile, one head
                # ahead of the score loop so its DVE drains are off the
                # critical path.
                ksb_t = ksbs[h % 2]
                hh_ = h % 4
                for c in range(S // 512):
                    csl = slice(c * 512, (c + 1) * 512)
                    pt = ps.tile([P, 512], dt.float32, tag="mm", name="pt")
                    for l in range(KI_KVL):
                        nc.tensor.matmul(
                            pt[:],
                            wkh_t[:, l, hh_ * P : (hh_ + 1) * P],
                            ckv[:, l, csl],
                            start=(l == 0), stop=(l == KI_KVL - 1),
                        )
                    nc.vector.tensor_copy(ksb_t[:, 0, csl], pt[:])
                nc.vector.tensor_copy(ksb_t[:64, 1, :], kpe[:])

            prev = None
            wkh_cur = kvw_tile("wkh", w_kv_k, 0)
            wvh_cur = kvw_tile("wvh", w_kv_v, 0)
            wkh_nxt = wvh_nxt = None
            k_expand(0, wkh_cur)
            for hg in range(NH // 4):
                if hg + 1 < NH // 4:
                    wkh_nxt = kvw_tile("wkh", w_kv_k, hg + 1)
                    wvh_nxt = kvw_tile("wvh", w_kv_v, hg + 1)
                # v for 4 heads at once
                vsb = p3.tile([P, TK, 4 * VHD], dt.float8e4, tag="vsb", bufs=2)
                for kt in range(TK):
                    pt = ps.tile([P, 4 * VHD], dt.float32, tag="mm")
                    for l in range(KI_KVL):
                        nc.tensor.matmul(
                            pt[:],
                            ckv[:, l, kt * P : (kt + 1) * P],
                            wvh_cur[:, l, :],
                            start=(l == 0), stop=(l == KI_KVL - 1),
                        )
                    nc.vector.tensor_copy(vsb[:, kt, :], pt[:])
                for hh in range(4):
                    h = hg * 4 + hh
                    ksb = ksbs[h % 2]
                    if h + 1 < NH:
                        k_expand(h + 1, wkh_cur if hh < 3 else wkh_nxt)
                    if h == NH - 1:  # prefetch o_proj weights under last head
                        wo_cache[0] = wo_pool.tile([P, NH, VHD], dt.bfloat16, tag="wo", bufs=2, name="wo_p")
                        nc.sync.dma_start(wo_cache[0][:], w_o[0])
                        wo_cache[1] = wo_pool.tile([P, NH, VHD], dt.bfloat16, tag="wo", bufs=2, name="wo_p")
                        nc.sync.dma_start(wo_cache[1][:], w_o[1])
                    if prev is not None:
                        normalize(*prev)
                        prev = None
                    av = psC.tile([P, QR], dt.float32, tag="av")
                    se = psC.tile([P, QR], dt.float32, tag="se")

                    def _drain(p8_, prt, off):
                        dr(se[:, off:], ones8[:], prt[:, :, off:], p8_ == 0, p8_ == TK // 2 - 1)
                        dr(
                            av[:, off:],
                            vsb[:, 2 * p8_ : 2 * p8_ + 2, hh * VHD : (hh + 1) * VHD],
                            prt[:, :, off:],
                            p8_ == 0,
                            p8_ == TK // 2 - 1,
                        )

                    pend = []
                    for p8_ in range(TK // 2):
                        off = (p8_ // 2) * P
                        prt = tmp.tile([P, 2, QR], dt.float8e4, tag="pr", bufs=4)
                        for i in (0, 1):
                            kt = 2 * p8_ + i
                            sc = ps.tile([P, QR], dt.float32, tag="mm")
                            nc.tensor.matmul(
                                sc[:, off:],
                                ksb[:, :, kt * P : (kt + 1) * P],
                                qpk[:, h, :, off:],
                                start=True,
                                stop=True,
                                perf_mode=DRM,
                            )
                            nc.scalar.activation(
                                out=prt[:, i, off:], in_=sc[:, off:], func=AF.Exp,
                                scale=ATTN_SCALE,
                            )
                            nc.vector.tensor_tensor(
                                prt[:, i, off : off + P],
                                prt[:, i, off : off + P],
                                maskt[:, kt, :],
                                MUL,
                            )
                        pend.append((p8_, prt, off))
                        if len(pend) > 2:
                            _drain(*pend.pop(0))
                    while pend:
                        _drain(*pend.pop(0))
                    prev = (h, av, se)
                wkh_cur, wvh_cur = wkh_nxt, wvh_nxt
            normalize(*prev)

        # ==== phase D: o_proj + residual + ln2 (h1 resident in SBUF) ====
        with contextlib.ExitStack() as sc45:
            x2m = sc45.enter_context(tc.tile_pool(name="x2m", bufs=1))
            x2 = x2m.tile([P, KI_H, QR], dt.bfloat16)
            h1sb = x2m.tile([P, KI_H, QR], dt.bfloat16)
            msb = x2m.tile([P, NF_FF, QR], dt.bfloat16)
            p5 = sc45.enter_context(tc.tile_pool(name="p5", bufs=1))
            ffn_cache = {}

            def ffn_tile(tag, src, nf):
                key = (tag, nf)
                if key not in ffn_cache:
                    tt = p5.tile([P, KI_H, P], dt.bfloat16, tag=tag, bufs=3)
                    nc.sync.dma_start(tt[:], src[nf])
                    ffn_cache[key] = tt
                return ffn_cache.pop(key)

            ffn_cache[("wg", 0)] = p5.tile([P, KI_H, P], dt.bfloat16, tag="wg", bufs=3, name="wg_p")
            nc.sync.dma_start(ffn_cache[("wg", 0)][:], w_g[0])
            ffn_cache[("wu", 0)] = p5.tile([P, KI_H, P], dt.bfloat16, tag="wu", bufs=3, name="wu_p")
            nc.sync.dma_start(ffn_cache[("wu", 0)][:], w_u[0])


            with tc.tile_pool(name="psD", bufs=1, space="PSUM") as psD:
                oacc = psD.tile([P, QR], dt.float32, tag="acc")
                for nf in range(KI_H):
                    wt = wo_tile(nf)
                    if nf + 1 < KI_H and nf + 1 not in wo_cache:
                        wo_cache[nf + 1] = wo_pool.tile([P, NH, VHD], dt.bfloat16, tag="wo", bufs=2, name="wo_p")
                        nc.sync.dma_start(wo_cache[nf + 1][:], w_o[nf + 1])
                    pt = ps.tile([P, QR], dt.float32, tag="mm")
                    for kh in range(NH):
                        nc.tensor.matmul(
                            pt[:],
                            wt[:, kh, :],
                            attn[:, kh, :],
                            start=(kh == 0),
                            stop=(kh == NH - 1),
                        )
                    ht = ld.tile([P, QR], dt.float32, tag="hload")
                    nc.sync.dma_start(ht[:], hTq[nf * P : (nf + 1) * P, :])
                    nc.vector.tensor_tensor(h1sb[:, nf, :], pt[:], ht[:], ADD)
                    sq = tmp.tile([P, QR], dt.bfloat16, tag="sq")
                    nc.vector.tensor_tensor(sq[:], h1sb[:, nf, :], h1sb[:, nf, :], MUL)
                    nc.tensor.matmul(
                        oacc[:], ones_bfP[:], sq[:], start=(nf == 0), stop=(nf == KI_H - 1)
                    )
                s2rep = tmp.tile([P, QR], dt.float32, tag="s1r", bufs=2)
                rsqrt_into(oacc, H, s2rep)
                for nf in range(KI_H):
                    nc.vector.tensor_tensor(x2[:, nf, :], h1sb[:, nf, :], s2rep[:], MUL)

            # ==== phase E: FFN (SwiGLU), bf16 ====
            for nf in range(NF_FF):
                wtg = ffn_tile("wg", w_g, nf)
                if nf + 1 < NF_FF:
                    ffn_cache[("wg", nf + 1)] = p5.tile([P, KI_H, P], dt.bfloat16, tag="wg", bufs=3, name="wg_p")
                    nc.sync.dma_start(ffn_cache[("wg", nf + 1)][:], w_g[nf + 1])
                pg = ps.tile([P, QR], dt.float32, tag="mm")
                for ki in range(KI_H):
                    nc.tensor.matmul(
                        pg[:], wtg[:, ki, :], x2[:, ki, :],
                        start=(ki == 0), stop=(ki == KI_H - 1),
                    )
                gs = tmp.tile([P, QR], dt.bfloat16, tag="sq")
                nc.scalar.activation(out=gs[:], in_=pg[:], func=AF.Silu)
                wtu = ffn_tile("wu", w_u, nf)
                if nf + 1 < NF_FF:
                    ffn_cache[("wu", nf + 1)] = p5.tile([P, KI_H, P], dt.bfloat16, tag="wu", bufs=3, name="wu_p")
                    nc.sync.dma_start(ffn_cache[("wu", nf + 1)][:], w_u[nf + 1])
                pu = ps.tile([P, QR], dt.float32, tag="mm")
                for ki in range(KI_H):
                    nc.tensor.matmul(
                        pu[:], wtu[:, ki, :], x2[:, ki, :],
                        start=(ki == 0), stop=(ki == KI_H - 1),
                    )
                nc.vector.tensor_tensor(msb[:, nf, :], pu[:], gs[:], MUL)

            for nf in range(KI_H):
                pt = ps.tile([P, QR], dt.float32, tag="mm")
                for qtr in range(4):
                    wt = p5.tile([P, NF_FF // 4, P], dt.bfloat16, tag="wd", bufs=3)
                    nc.sync.dma_start(wt[:], w_d[nf, :, qtr * 16 : (qtr + 1) * 16, :])
                    for ki in range(NF_FF // 4):
                        kk = qtr * 16 + ki
                        nc.tensor.matmul(
                            pt[:], wt[:, ki, :], msb[:, kk, :],
                            start=(kk == 0), stop=(kk == NF_FF - 1),
                        )
                ot = tmp.tile([P, QR], dt.float32, tag="h1t", bufs=2)
                nc.vector.tensor_tensor(ot[:], pt[:], h1sb[:, nf, :], ADD)
                nc.sync.dma_start(out[nf * P : (nf + 1) * P, :], ot[:])

    return nc


# ---------------------------------------------------------------------------
# host-side packing
# ---------------------------------------------------------------------------
def _deint_perm():
    # deinterleave: out[i] = in[2i] (i<32), in[2(i-32)+1] (i>=32)
    return np.concatenate([np.arange(0, ROPE, 2), np.arange(1, ROPE, 2)])


def _pack_lhst(w, nki, nnf, nfree=P, dtype=BF16, scale=1.0):
    # w [nki*P, nnf*nfree] -> [nnf, P, nki, nfree]
    return np.ascontiguousarray(
        (w.reshape(nki, P, nnf, nfree) * scale).transpose(2, 1, 0, 3).astype(dtype)
    )


def _prep_shared(inputs):
    perm = _deint_perm()
    ln1 = inputs["ln1_w"].astype(np.float32)
    qaln = inputs["q_a_ln_w"].astype(np.float32)
    kvln = inputs["kv_a_ln_w"].astype(np.float32)
    ln2 = inputs["ln2_w"].astype(np.float32)

    w_qa = inputs["q_a_kernel"].astype(np.float32) * ln1[:, None]
    w_kva = inputs["kv_a_kernel"].astype(np.float32) * ln1[:, None]
    w_kva = w_kva.copy()
    w_kva[:, KV_LORA:] = w_kva[:, KV_LORA:][:, perm]
    w_qb = inputs["q_b_kernel"].astype(np.float32) * qaln[:, None]
    w_qb = w_qb.copy()
    for h in range(NH):
        blk = slice(h * QHD + NOPE, (h + 1) * QHD)
        w_qb[:, blk] = w_qb[:, blk][:, perm]
    w_kvb = inputs["kv_b_kernel"].astype(np.float32) * kvln[:, None]
    w_o = inputs["o_kernel"].astype(np.float32)
    w_g = inputs["gate_kernel"].astype(np.float32) * ln2[:, None]
    w_u = inputs["up_kernel"].astype(np.float32) * ln2[:, None]
    w_d = inputs["down_kernel"].astype(np.float32)

    shared = {
        "w_qa": _pack_lhst(w_qa, KI_H, KI_QL, P, F8, WS),
        # w_qb: [NH, P, KI_QL, QHD]
        "w_qb": np.ascontiguousarray(
            w_qb.reshape(KI_QL, P, NH, QHD).transpose(2, 1, 0, 3).astype(BF16)
        ),
        # w_kva resident: [P, KI_H, 576]
        "w_kva": np.ascontiguousarray(
            (w_kva.reshape(KI_H, P, KV_LORA + ROPE) * WS).transpose(1, 0, 2).astype(F8)
        ),
        # w_kvb split into k/v halves, packed per head-group of 4:
        "w_kv_k": np.ascontiguousarray(
            w_kvb.reshape(KI_KVL, P, NH // 4, 4, 2, 128)[:, :, :, :, 0, :]
            .transpose(2, 1, 0, 3, 4)
            .reshape(NH // 4, P, KI_KVL, 512)
            .astype(BF16)
        ),
        "w_kv_v": np.ascontiguousarray(
            w_kvb.reshape(KI_KVL, P, NH // 4, 4, 2, 128)[:, :, :, :, 1, :]
            .transpose(2, 1, 0, 3, 4)
            .reshape(NH // 4, P, KI_KVL, 512)
            .astype(BF16)
        ),
        # w_o: [KI_H(nf), P, NH, VHD]
        "w_o": np.ascontiguousarray(
            w_o.reshape(NH, VHD, KI_H, P).transpose(2, 1, 0, 3).astype(BF16)
        ),
        "w_g": _pack_lhst(w_g, KI_H, NF_FF),
        "w_u": _pack_lhst(w_u, KI_H, NF_FF),
        "w_d": _pack_lhst(w_d, NF_FF, KI_H),
    }
    return shared


def _prep_batch(inputs, b):
    hid = np.asarray(inputs["hidden_states"][b], dtype=np.float32)  # [S, H]
    hT = np.ascontiguousarray(hid.T)  # [H, S] f32
    hT8 = hT.astype(F8)
    pos = np.asarray(inputs["position_ids"][b]).astype(np.int64)
    cos_g = np.asarray(inputs["cos"], dtype=np.float32)[pos][:, :32]  # [S, 32]
    sin_g = np.asarray(inputs["sin"], dtype=np.float32)[pos][:, :32]
    return hT, hT8, np.ascontiguousarray(cos_g.T), np.ascontiguousarray(sin_g.T)


def _core_rows(j):
    return np.concatenate(
        [np.arange((j + 4 * i) * P, (j + 4 * i + 1) * P) for i in range(4)]
    )


def _core_masks(j):
    # one 128-col mask block per key tile, applied at query slot kt//4
    kp = np.arange(P)[:, None]
    qf = np.arange(P)[None, :]
    m = np.zeros((P, TK, P), dtype=F8)
    for kt in range(TK):
        r = kt % 4
        if j > r:
            m[:, kt, :] = 1.0
        elif j == r:
            m[:, kt, :] = (kp <= qf).astype(F8)
        # j < r: stays zero
    return m


def kernel(**inputs) -> np.ndarray:
    import concourse.bass as bass  # noqa: F401  (env check)
    from concourse.bass_utils import run_bass_kernel_spmd

    if "nc" not in _COMPILED:
        _COMPILED["nc"] = _build_nc()
    nc = _COMPILED["nc"]

    shared = _prep_shared(inputs)
    in_maps = []
    per_batch = [_prep_batch(inputs, b) for b in range(B)]
    for c in range(8):
        b, j = c // 4, c % 4
        hT, hT8, cosTb, sinTb = per_batch[b]
        rows = _core_rows(j)
        in_map = dict(shared)
        in_map["hTb"] = hT8
        in_map["hTqb"] = np.ascontiguousarray(hT8[:, rows])
        in_map["hTq"] = np.ascontiguousarray(hT[:, rows])
        in_map["cosT"] = cosTb
        in_map["sinT"] = sinTb
        in_map["cosTq"] = np.ascontiguousarray(cosTb[:, rows])
        in_map["sinTq"] = np.ascontiguousarray(sinTb[:, rows])
        in_map["masks"] = _core_masks(j)
        in_maps.append(in_map)

    res = run_bass_kernel_spmd(nc, in_maps, core_ids=list(range(8)))
    globals()["LAST_RESULT"] = res

    out = np.empty((B, S, H), dtype=np.float32)
    for c in range(8):
        b, j = c // 4, c % 4
        out[b, _core_rows(j), :] = res.results[c]["out"].T
    return out


# revision 26
# speedup vs baseline: 1.0680x; 1.0100x over previous
"""DeepseekV2 decoder layer (MLA attention + SwiGLU MLP) on 8 TRN2 NeuronCores.

Sharding: core c -> batch b = c//4, query tiles {j, j+4, j+8, j+12} (j = c%4,
128 rows each, causally interleaved). The interleave makes the causal
structure identical on every core: key tile kt is only needed by query slots
s >= kt//4, so the SPMD program skips 37.5% of score/AV work uniformly.

fp8 (e4m3, DoubleRow 2x matmul) is used for the hidden-state input, the
q_a/kv_a projections, the packed q/k score tiles, v, and the attention
probabilities; q_b/kv_b, o_proj and the FFN stay bf16 (fp8 there blows the
2e-2 error budget). fp8 weights are scaled x16 on the host to escape
denormals; the 1/16 compensation is folded into the rsqrt stat scales
(ws64 variants), so no extra ops are spent.

Per-head packed layout for scores: k/q tiles [128, 2, n] where pair slot 0 =
nope (128 feats), slot 1 = rope (64 feats) + 64 zero rows, so one DoubleRow
matmul computes a whole 192-dim score tile.
"""

import json

import numpy as np
import ml_dtypes

B, S, H = 2, 2048, 2048
NH = 16
Q_LORA = 1536
KV_LORA = 512
NOPE = 128
ROPE = 64
QHD = NOPE + ROPE  # 192
VHD = 128
FF = 8192
EPS = 1e-6
P = 128
QR = 512  # query rows per core
TK = S // P  # 16 key tiles
KI_H = H // P  # 16
KI_QL = Q_LORA // P  # 12
KI_KVL = KV_LORA // P  # 4
NF_FF = FF // P  # 64
ATTN_SCALE = QHD ** -0.5
WS = 16.0  # fp8 weight scale (16x: escapes denormals; keeps 16x-scaled k_pe/v tails far below fp8's 240 max)

BF16 = ml_dtypes.bfloat16
F8 = ml_dtypes.float8_e4m3

_COMPILED = {}


# ---------------------------------------------------------------------------
# compiler workaround: this container's walrus rejects >1 sem wait per
# instruction; split extra waits onto single-wait NoOps.
# ---------------------------------------------------------------------------
def _install_multiwait_fix(bass):
    if getattr(bass.Bass, "_multiwait_fix_installed", False):
        return
    orig = bass.Bass.to_json_bytes

    def _split(m):
        for f in m.get("functions", []):
            for b in f.get("blocks", []):
                out = []
                for inst in b.get("instructions", []):
                    si = inst.get("sync_info") or {}
                    waits = si.get("on_wait") or []
                    if len(waits) > 1:
                        for k, w in enumerate(waits[:-1]):
                            out.append(
                                {
                                    "debug": inst.get("debug", 0),
                                    "engine": inst["engine"],
                                    "ins": [],
                                    "name": f"{inst['name']}_w{k}",
                                    "opcode": "NoOp",
                                    "outs": [],
                                    "sync_info": {"on_update": [], "on_wait": [w]},
                                }
                            )
                        si["on_wait"] = [waits[-1]]
                    out.append(inst)
                b["instructions"] = out
        return m

    def patched(self):
        raw = orig(self)
        try:
            return json.dumps(_split(json.loads(raw))).encode()
        except Exception:
            return raw

    bass.Bass.to_json_bytes = patched
    bass.Bass._multiwait_fix_installed = True


def _install_drain_fix(tile, ScopedClock, VectorClock):
    if getattr(tile.TileContext, "_drain_fix_installed", False):
        return

    def _drain_and_barrier(self, tick_clock, wait_clock):
        gc = tick_clock.global_clock
        n = len(gc)
        for p in range(n):
            t = gc[p]
            if t > 0:
                vc = VectorClock([0] * n)
                vc.require_at_least(p, t)
                d = self.nc.sync.drain()
                wait_clock.add_sem_waits(d.ins, ScopedClock({None: vc}))
        self.nc.all_engine_barrier()
        popped = self.nc._tile_sem_poison_stack.pop()
        assert popped is self._sem_poison
        self.nc.clear_and_free_semaphores(list(self.sems.allocated().values()))
        self.nc.all_engine_barrier()

    tile.TileContext._drain_and_barrier = _drain_and_barrier
    tile.TileContext._drain_fix_installed = True


# ---------------------------------------------------------------------------
# device program
# ---------------------------------------------------------------------------
def _build_nc():
    import concourse.bass as bass
    import concourse.mybir as mybir
    import concourse.tile as tile
    from concourse.vector_clock import ScopedClock, VectorClock

    _install_multiwait_fix(bass)
    _install_drain_fix(tile, ScopedClock, VectorClock)

    dt = mybir.dt
    AF = mybir.ActivationFunctionType
    MUL = mybir.AluOpType.mult
    ADD = mybir.AluOpType.add
    SUB = mybir.AluOpType.subtract
    DRM = mybir.MatmulPerfMode.DoubleRow

    nc = bass.Bass()

    # register EPS so float bias=EPS works on the scalar engine
    _eps_t = nc.alloc_sbuf_tensor(f"const-float32-{EPS}", [128, 1], dt.float32)
    nc.gpsimd.memset(_eps_t.ap(), EPS)
    nc.const_aps.aps[(dt.float32, EPS)] = _eps_t.ap()
    EPS64 = EPS * WS * WS
    _eps64_t = nc.alloc_sbuf_tensor(f"const-float32-{EPS64}", [128, 1], dt.float32)
    nc.gpsimd.memset(_eps64_t.ap(), EPS64)
    nc.const_aps.aps[(dt.float32, EPS64)] = _eps64_t.ap()
    import math
    NLNWS = -math.log(WS)
    _nlnws_t = nc.alloc_sbuf_tensor(f"const-float32-{NLNWS}", [128, 1], dt.float32)
    nc.gpsimd.memset(_nlnws_t.ap(), NLNWS)
    nc.const_aps.aps[(dt.float32, NLNWS)] = _nlnws_t.ap()
    nc.all_engine_barrier()

    # ---- inputs ----
    hTb = nc.dram_tensor("hTb", [H, S], dt.float8e4, kind="ExternalInput")
    hTqb = nc.dram_tensor("hTqb", [H, QR], dt.float8e4, kind="ExternalInput")
    hTq = nc.dram_tensor("hTq", [H, QR], dt.float32, kind="ExternalInput")
    cosT = nc.dram_tensor("cosT", [32, S], dt.float32, kind="ExternalInput")
    sinT = nc.dram_tensor("sinT", [32, S], dt.float32, kind="ExternalInput")
    cosTq = nc.dram_tensor("cosTq", [32, QR], dt.float32, kind="ExternalInput")
    sinTq = nc.dram_tensor("sinTq", [32, QR], dt.float32, kind="ExternalInput")
    masks = nc.dram_tensor("masks", [P, TK, P], dt.float8e4, kind="ExternalInput")
    w_qa = nc.dram_tensor("w_qa", [KI_QL, P, KI_H, P], dt.float8e4, kind="ExternalInput")
    w_qb = nc.dram_tensor("w_qb", [NH, P, KI_QL, QHD], dt.float8e4, kind="ExternalInput")
    w_kva = nc.dram_tensor("w_kva", [P, KI_H, KV_LORA + ROPE], dt.float8e4, kind="ExternalInput")
    w_kv_k = nc.dram_tensor("w_kv_k", [NH // 4, P, KI_KVL, 512], dt.bfloat16, kind="ExternalInput")
    w_kv_v = nc.dram_tensor("w_kv_v", [NH // 4, P, KI_KVL, 512], dt.bfloat16, kind="ExternalInput")
    w_o = nc.dram_tensor("w_o", [KI_H, P, NH, VHD], dt.bfloat16, kind="ExternalInput")
    w_g = nc.dram_tensor("w_g", [NF_FF, P, KI_H, P], dt.bfloat16, kind="ExternalInput")
    w_u = nc.dram_tensor("w_u", [NF_FF, P, KI_H, P], dt.bfloat16, kind="ExternalInput")
    w_d = nc.dram_tensor("w_d", [KI_H, P, NF_FF, P], dt.bfloat16, kind="ExternalInput")
    out = nc.dram_tensor("out", [H, QR], dt.float32, kind="ExternalOutput")

    import contextlib

    with tile.TileContext(nc) as tc, contextlib.ExitStack() as top:
        tp = lambda **kw: top.enter_context(tc.tile_pool(**kw))
        ones = tp(name="ones", bufs=1)
        tmp = tp(name="tmp", bufs=3)
        ld = tp(name="ld", bufs=3)
        ps = tp(name="ps", bufs=3, space="PSUM")
        attn_pool = tp(name="attn_pool", bufs=1)
        attn = attn_pool.tile([P, NH, QR], dt.bfloat16)
        maskp = tp(name="maskp", bufs=1)
        maskt = maskp.tile([P, TK, P], dt.float8e4)
        wo_pool = tp(name="wo_pool", bufs=2)

        ones_bfP = ones.tile([P, P], dt.bfloat16)
        nc.vector.memset(ones_bfP[:], 1.0)
        ones8 = ones.tile([P, 2, P], dt.float8e4)
        nc.vector.memset(ones8[:], 1.0)

        def dr(o, l, r, start, stop):
            nc.tensor.matmul(o, l, r, start=start, stop=stop, perf_mode=DRM)

        def rsqrt_into(acc, denom, dst, ws64=False):
            # dst = 1/sqrt(mean + eps) (optionally a further 1/WS) computed as
            # exp(-0.5*ln(.)) -- two scalar-engine table ops, avoiding the
            # 3.3us DVE reciprocal that would serialize the vector queue.
            lnt = tmp.tile([P, acc.shape[-1]], dt.float32, tag="h1t", bufs=2)
            nc.scalar.activation(
                out=lnt[:], in_=acc[:], func=AF.Ln, bias=EPS, scale=1.0 / denom
            )
            if ws64:
                nc.scalar.activation(out=dst[:], in_=lnt[:], func=AF.Exp, scale=-0.5, bias=NLNWS)
            else:
                nc.scalar.activation(out=dst[:], in_=lnt[:], func=AF.Exp, scale=-0.5)

        def rsqrt_tmp(acc, denom, tag, ws64=False):
            d = tmp.tile([P, acc.shape[-1]], dt.float32, tag=tag, bufs=2)
            rsqrt_into(acc, denom, d, ws64)
            return d

        with contextlib.ExitStack() as mid:
            lat = mid.enter_context(tc.tile_pool(name="lat", bufs=1))
            ckv = lat.tile([P, KI_KVL, S], dt.bfloat16)  # normalized kv latents (1x, bf16)
            kpe = lat.tile([ROPE, S], dt.float8e4)  # roped shared key-pe (1x)
            pA = mid.enter_context(tc.tile_pool(name="pA", bufs=1))
            xqbf = pA.tile([P, KI_H, QR], dt.float8e4)
            s1qrep64 = pA.tile([P, QR], dt.float32)
            qpk_pool = mid.enter_context(tc.tile_pool(name="qpk", bufs=1))
            qpk = qpk_pool.tile([P, NH, 2, QR], dt.float8e4)
            wq_pool = mid.enter_context(tc.tile_pool(name="wq", bufs=1))

            wq_cache = {}

            def wqa_tile(nf):
                if nf not in wq_cache:
                    t = wq_pool.tile([P, KI_H, P], dt.float8e4, tag="wqa", bufs=2)
                    nc.sync.dma_start(t[:], w_qa[nf])
                    wq_cache[nf] = t
                return wq_cache.pop(nf)

            # ==== phase A: ln1 stats + kv latents + shared roped k_pe ====
            with tc.tile_pool(name="pB", bufs=1) as pB, tc.tile_pool(
                name="psA", bufs=2, space="PSUM"
            ) as psA:
                nc.sync.dma_start(maskt[:], masks[:])
                for ki in range(KI_H):
                    nc.sync.dma_start(xqbf[:, ki, :], hTqb[ki * P : (ki + 1) * P, :])
                wkva = pB.tile([P, KI_H, KV_LORA + ROPE], dt.float8e4)
                nc.sync.dma_start(wkva[:], w_kva[:])
                cosb = pB.tile([32, S], dt.float32)
                sinb = pB.tile([32, S], dt.float32)
                nc.sync.dma_start(cosb[:], cosT[:])
                nc.sync.dma_start(sinb[:], sinT[:])
                # prefetch first q_a weights for phase B
                wqa_pre = wq_pool.tile([P, KI_H, P], dt.float8e4, tag="wqa", bufs=2)
                nc.sync.dma_start(wqa_pre[:], w_qa[0])
                wq_cache[0] = wqa_pre

                # chunk pipeline: squares for chunk t+1 run on the SCALAR
                # engine while the PE does chunk t's kv matmuls and the DVE
                # does chunk t's PSUM drains; acc DRs for t+1 are emitted after
                # kv work so the PE never waits on the square/stat chain.
                xcs = {}
                s1s = {}
                sq_tiles = {}

                def xc_dma(t):
                    tsl = slice(t * 512, (t + 1) * 512)
                    xc = pB.tile([P, KI_H, 512], dt.float8e4, tag="xc", bufs=3, name="xc")
                    for ki in range(KI_H):
                        nc.sync.dma_start(xc[:, ki, :], hTb[ki * P : (ki + 1) * P, tsl])
                    xcs[t] = xc

                def sq_emit(t):
                    xc = xcs[t]
                    tiles = []
                    for kp_ in range(KI_H // 2):
                        sqp = pB.tile([P, 2, 512], dt.float8e4, tag="sqp", bufs=8, name="sqp")
                        nc.scalar.activation(
                            out=sqp[:], in_=xc[:, 2 * kp_ : 2 * kp_ + 2, :], func=AF.Square
                        )
                        tiles.append(sqp)
                    sq_tiles[t] = tiles

                def acc_emit(t):
                    tiles = sq_tiles.pop(t)
                    acc = psA.tile([P, 512], dt.float32, tag="acc", name="acc")
                    for kp_, sqp in enumerate(tiles):
                        dr(acc[:], ones8[:], sqp[:], kp_ == 0, kp_ == KI_H // 2 - 1)
                    s1s[t] = rsqrt_tmp(acc, H, "s1rb", ws64=True)

                xc_dma(0)
                xc_dma(1)
                sq_emit(0)

                # q-slice ln1 stats on the DVE, overlapping the scalar squares
                accq = psA.tile([P, QR], dt.float32, tag="acc")
                for kp_ in range(KI_H // 2):
                    sqq = pB.tile([P, 2, QR], dt.float8e4, tag="sqq", bufs=2)
                    for i in (0, 1):
                        ki = 2 * kp_ + i
                        nc.vector.tensor_tensor(sqq[:, i, :], xqbf[:, ki, :], xqbf[:, ki, :], MUL)
                    dr(accq[:], ones8[:], sqq[:], kp_ == 0, kp_ == KI_H // 2 - 1)
                rsqrt_into(accq, H, s1qrep64, ws64=True)
                acc_emit(0)

                for t in range(S // 512):
                    tsl = slice(t * 512, (t + 1) * 512)
                    if t + 2 < S // 512:
                        xc_dma(t + 2)
                    if t + 1 < S // 512:
                        sq_emit(t + 1)
                    xc = xcs.pop(t)
                    s1rep64 = s1s.pop(t)
                    ckvt = pB.tile([P, KI_KVL, 512], dt.bfloat16, tag="ckvt", bufs=2)
                    kvacc = psA.tile([P, 512], dt.float32, tag="acc2")
                    sqp2s = []
                    sqp2 = None
                    for nf in range(KI_KVL):
                        pt = ps.tile([P, 512], dt.float32, tag="mm")
                        for kp_ in range(KI_H // 2):
                            dr(
                                pt[:],
                                wkva[:, 2 * kp_ : 2 * kp_ + 2, nf * P : (nf + 1) * P],
                                xc[:, 2 * kp_ : 2 * kp_ + 2, :],
                                kp_ == 0,
                                kp_ == KI_H // 2 - 1,
                            )
                        nc.vector.tensor_tensor(ckvt[:, nf, :], pt[:], s1rep64[:], MUL)
                        if nf % 2 == 0:
                            sqp2 = pB.tile([P, 2, 512], dt.float8e4, tag="sqp2", bufs=2)
                            sqp2s.append(sqp2)
                        nc.vector.tensor_tensor(
                            sqp2[:, nf % 2, :], ckvt[:, nf, :], ckvt[:, nf, :], MUL
                        )
                    # k_pe: last 64 cols of w_kva (1x true scale via s1rep64)
                    pt = ps.tile([ROPE, 512], dt.float32, tag="mm")
                    for kp_ in range(KI_H // 2):
                        dr(
                            pt[:],
                            wkva[:, 2 * kp_ : 2 * kp_ + 2, KV_LORA : KV_LORA + ROPE],
                            xc[:, 2 * kp_ : 2 * kp_ + 2, :],
                            kp_ == 0,
                            kp_ == KI_H // 2 - 1,
                        )
                    for q2, sqp2 in enumerate(sqp2s):
                        dr(kvacc[:], ones8[:], sqp2[:], q2 == 0, q2 == KI_KVL // 2 - 1)
                    if t + 1 < S // 512:
                        acc_emit(t + 1)
                    pes = tmp.tile([ROPE, 512], dt.float32, tag="pes", bufs=2)
                    nc.vector.tensor_tensor(pes[:], pt[:], s1rep64[:ROPE, :], MUL)
                    x2h = tmp.tile([32, 512], dt.float32, tag="x2h", bufs=2)
                    nc.sync.dma_start(x2h[:], pes[32:, :])
                    t1 = tmp.tile([32, 512], dt.float32, tag="t1", bufs=2)
                    t2 = tmp.tile([32, 512], dt.float32, tag="t2", bufs=2)
                    o2 = tmp.tile([32, 512], dt.float8e4, tag="o2", bufs=2)
                    nc.vector.tensor_tensor(t1[:], pes[:32, :], cosb[:, tsl], MUL)
                    nc.vector.tensor_tensor(t2[:], x2h[:], sinb[:, tsl], MUL)
                    nc.vector.tensor_tensor(kpe[:32, tsl], t1[:], t2[:], SUB)
                    nc.vector.tensor_tensor(t1[:], x2h[:], cosb[:, tsl], MUL)
                    nc.vector.tensor_tensor(t2[:], pes[:32, :], sinb[:, tsl], MUL)
                    nc.vector.tensor_tensor(o2[:], t1[:], t2[:], ADD)
                    nc.sync.dma_start(kpe[32:, tsl], o2[:])
                    # kv_a rmsnorm rescale -> bf16 latents
                    rkv = rsqrt_tmp(kvacc, KV_LORA, "s1rc")
                    for nf in range(KI_KVL):
                        nc.vector.tensor_tensor(ckv[:, nf, tsl], ckvt[:, nf, :], rkv[:], MUL)

            # ==== attention k/v weight pool created early so hg=0 prefetches ====
            p3 = mid.enter_context(tc.tile_pool(name="p3", bufs=1))
            psC = mid.enter_context(tc.tile_pool(name="psC", bufs=2, space="PSUM"))
            ksb0 = p3.tile([P, 2, S], dt.float8e4, tag="ksb0", bufs=1)
            ksb1 = p3.tile([P, 2, S], dt.float8e4, tag="ksb1", bufs=1)
            ksbs = [ksb0, ksb1]
            for i in range(2):
                nc.vector.memset(ksbs[i][64:, 1, :], 0.0)
            kv_cache = {}

            def kvw_tile(tag, src, hg):
                key = (tag, hg)
                if key not in kv_cache:
                    tt = p3.tile([P, KI_KVL, 512], dt.bfloat16, tag=tag, bufs=2)
                    nc.sync.dma_start(tt[:], src[hg])
                    kv_cache[key] = tt
                return kv_cache.pop(key)

            kv_cache[("wkh", 0)] = p3.tile([P, KI_KVL, 512], dt.bfloat16, tag="wkh", bufs=2, name="wkh_p")
            nc.sync.dma_start(kv_cache[("wkh", 0)][:], w_kv_k[0])
            kv_cache[("wvh", 0)] = p3.tile([P, KI_KVL, 512], dt.bfloat16, tag="wvh", bufs=2, name="wvh_p")
            nc.sync.dma_start(kv_cache[("wvh", 0)][:], w_kv_v[0])

            # ==== phase B: q path ====
            with tc.tile_pool(name="p2", bufs=1) as p2:
                qlat = p2.tile([P, KI_QL, QR], dt.float8e4)
                qacc = psC.tile([P, QR], dt.float32, tag="acc", bufs=1)
                sqp = None
                for nf in range(KI_QL):
                    wt = wqa_tile(nf)
                    if nf + 1 < KI_QL:
                        wqa_tile_pre = wq_pool.tile([P, KI_H, P], dt.float8e4, tag="wqa", bufs=2)
                        nc.sync.dma_start(wqa_tile_pre[:], w_qa[nf + 1])
                        wq_cache[nf + 1] = wqa_tile_pre
                    pt = ps.tile([P, QR], dt.float32, tag="mm")
                    for kp_ in range(KI_H // 2):
                        dr(
                            pt[:],
                            wt[:, 2 * kp_ : 2 * kp_ + 2, :],
                            xqbf[:, 2 * kp_ : 2 * kp_ + 2, :],
                            kp_ == 0,
                            kp_ == KI_H // 2 - 1,
                        )
                    nc.vector.tensor_tensor(qlat[:, nf, :], pt[:], s1qrep64[:], MUL)
                    if nf % 2 == 0:
                        sqp = tmp.tile([P, 2, QR], dt.float8e4, tag="sqb", bufs=2)
                    nc.vector.tensor_tensor(sqp[:, nf % 2, :], qlat[:, nf, :], qlat[:, nf, :], MUL)
                    if nf % 2 == 1:
                        dr(qacc[:], ones8[:], sqp[:], nf // 2 == 0, nf // 2 == KI_QL // 2 - 1)
                sqrep64 = p2.tile([P, QR], dt.float32)
                rsqrt_into(qacc, Q_LORA, sqrep64, ws64=True)

                # rope tables for q with q_a_ln scale (and 1/64) folded in
                cosq = p2.tile([32, QR], dt.float32)
                sinq = p2.tile([32, QR], dt.float32)
                nc.sync.dma_start(cosq[:], cosTq[:])
                nc.sync.dma_start(sinq[:], sinTq[:])
                nc.vector.tensor_tensor(cosq[:], cosq[:], sqrep64[:32, :], MUL)
                nc.vector.tensor_tensor(sinq[:], sinq[:], sqrep64[:32, :], MUL)

                for h in range(NH):
                    nc.vector.memset(qpk[64:, h, 1, :], 0.0)
                    wt = p2.tile([P, KI_QL, QHD], dt.float8e4, tag="wqb", bufs=2)
                    nc.sync.dma_start(wt[:], w_qb[h])
                    pt = ps.tile([P, QR], dt.float32, tag="mm")
                    for kp_ in range(KI_QL // 2):
                        dr(
                            pt[:],
                            wt[:, 2 * kp_ : 2 * kp_ + 2, :NOPE],
                            qlat[:, 2 * kp_ : 2 * kp_ + 2, :],
                            kp_ == 0,
                            kp_ == KI_QL // 2 - 1,
                        )
                    nc.vector.tensor_tensor(qpk[:, h, 0, :], pt[:], sqrep64[:], MUL)
                    pt2 = ps.tile([ROPE, QR], dt.float32, tag="mm")
                    for kp_ in range(KI_QL // 2):
                        dr(
                            pt2[:],
                            wt[:, 2 * kp_ : 2 * kp_ + 2, NOPE:QHD],
                            qlat[:, 2 * kp_ : 2 * kp_ + 2, :],
                            kp_ == 0,
                            kp_ == KI_QL // 2 - 1,
                        )
                    pesq = tmp.tile([ROPE, QR], dt.float32, tag="pes", bufs=2)
                    nc.vector.tensor_copy(pesq[:], pt2[:])
                    x2q = tmp.tile([32, QR], dt.float32, tag="x2h", bufs=2)
                    nc.sync.dma_start(x2q[:], pesq[32:, :])
                    t1 = tmp.tile([32, QR], dt.float32, tag="t1", bufs=2)
                    t2 = tmp.tile([32, QR], dt.float32, tag="t2", bufs=2)
                    o2 = tmp.tile([32, QR], dt.float8e4, tag="o2", bufs=2)
                    nc.vector.tensor_tensor(t1[:], pesq[:32, :], cosq[:], MUL)
                    nc.vector.tensor_tensor(t2[:], x2q[:], sinq[:], MUL)
                    nc.vector.tensor_tensor(qpk[:32, h, 1, :], t1[:], t2[:], SUB)
                    nc.vector.tensor_tensor(t1[:], x2q[:], cosq[:], MUL)
                    nc.vector.tensor_tensor(t2[:], pesq[:32, :], sinq[:], MUL)
                    nc.vector.tensor_tensor(o2[:], t1[:], t2[:], ADD)
                    nc.sync.dma_start(qpk[32:64, h, 1, :], o2[:])

            # ==== phase C: attention ====
            wo_cache = {}

            def wo_tile(nf):
                if nf not in wo_cache:
                    tt = wo_pool.tile([P, NH, VHD], dt.bfloat16, tag="wo", bufs=2)
                    nc.sync.dma_start(tt[:], w_o[nf])
                    wo_cache[nf] = tt
                return wo_cache.pop(nf)

            def normalize(h, av_t, se_t):
                # 1/se as exp(-ln(se)) on the scalar engine; se is
                # pre-replicated [P, QR] by the all-ones se reduction.
                rc = tmp.tile([P, QR], dt.float32, tag="s1r", bufs=2)
                lnt = tmp.tile([P, QR], dt.float32, tag="h1t", bufs=2)
                nc.scalar.activation(out=lnt[:], in_=se_t[:], func=AF.Ln)
                nc.scalar.activation(out=rc[:], in_=lnt[:], func=AF.Exp, scale=-1.0)
                nc.vector.tensor_tensor(attn[:, h, :], av_t[:], rc[:], MUL)

            def k_expand(h, wkh_t):
                # expand k_nope for head h into its packed ksb tile, one head
                # ahead of the score loop so its DVE drains are off the
                # critical path.
                ksb_t = ksbs[h % 2]
                hh_ = h % 4
                for c in range(S // 512):
                    csl = slice(c * 512, (c + 1) * 512)
                    pt = ps.tile([P, 512], dt.float32, tag="mm", name="pt")
                    for l in range(KI_KVL):
                        nc.tensor.matmul(
                            pt[:],
                            wkh_t[:, l, hh_ * P : (hh_ + 1) * P],
                            ckv[:, l, csl],
                            start=(l == 0), stop=(l == KI_KVL - 1),
                        )
                    nc.vector.tensor_copy(ksb_t[:, 0, csl], pt[:])
                nc.vector.tensor_copy(ksb_t[:64, 1, :], kpe[:])

            prev = None
            wkh_cur = kvw_tile("wkh", w_kv_k, 0)
            wvh_cur = kvw_tile("wvh", w_kv_v, 0)
            wkh_nxt = wvh_nxt = None
            k_expand(0, wkh_cur)
            for hg in range(NH // 4):
                if hg + 1 < NH // 4:
                    wkh_nxt = kvw_tile("wkh", w_kv_k, hg + 1)
                    wvh_nxt = kvw_tile("wvh", w_kv_v, hg + 1)
                # v for 4 heads at once
                vsb = p3.tile([P, TK, 4 * VHD], dt.float8e4, tag="vsb", bufs=2)
                for kt in range(TK):
                    pt = ps.tile([P, 4 * VHD], dt.float32, tag="mm")
                    for l in range(KI_KVL):
                        nc.tensor.matmul(
                            pt[:],
                            ckv[:, l, kt * P : (kt + 1) * P],
                            wvh_cur[:, l, :],
                            start=(l == 0), stop=(l == KI_KVL - 1),
                        )
                    nc.vector.tensor_copy(vsb[:, kt, :], pt[:])
                for hh in range(4):
                    h = hg * 4 + hh
                    ksb = ksbs[h % 2]
                    if h + 1 < NH:
                        k_expand(h + 1, wkh_cur if hh < 3 else wkh_nxt)
                    if h == NH - 1:  # prefetch o_proj weights under last head
                        wo_cache[0] = wo_pool.tile([P, NH, VHD], dt.bfloat16, tag="wo", bufs=2, name="wo_p")
                        nc.sync.dma_start(wo_cache[0][:], w_o[0])
                        wo_cache[1] = wo_pool.tile([P, NH, VHD], dt.bfloat16, tag="wo", bufs=2, name="wo_p")
                        nc.sync.dma_start(wo_cache[1][:], w_o[1])
                    if prev is not None:
                        normalize(*prev)
                        prev = None
                    av = psC.tile([P, QR], dt.float32, tag="av")
                    se = psC.tile([P, QR], dt.float32, tag="se")

                    def _drain(p8_, prt, off):
                        dr(se[:, off:], ones8[:], prt[:, :, off:], p8_ == 0, p8_ == TK // 2 - 1)
                        dr(
                            av[:, off:],
                            vsb[:, 2 * p8_ : 2 * p8_ + 2, hh * VHD : (hh + 1) * VHD],
                            prt[:, :, off:],
                            p8_ == 0,
                            p8_ == TK // 2 - 1,
                        )

                    pend = []
                    for p8_ in range(TK // 2):
                        off = (p8_ // 2) * P
                        prt = tmp.tile([P, 2, QR], dt.float8e4, tag="pr", bufs=4)
                        for i in (0, 1):
                            kt = 2 * p8_ + i
                            sc = ps.tile([P, QR], dt.float32, tag="mm")
                            nc.tensor.matmul(
                                sc[:, off:],
                                ksb[:, :, kt * P : (kt + 1) * P],
                                qpk[:, h, :, off:],
                                start=True,
                                stop=True,
                                perf_mode=DRM,
                            )
                            nc.scalar.activation(
                                out=prt[:, i, off:], in_=sc[:, off:], func=AF.Exp,
                                scale=ATTN_SCALE,
                            )
                            nc.vector.tensor_tensor(
                                prt[:, i, off : off + P],
                                prt[:, i, off : off + P],
                                maskt[:, kt, :],
                                MUL,
                            )
                        pend.append((p8_, prt, off))
                        if len(pend) > 2:
                            _drain(*pend.pop(0))
                    while pend:
                        _drain(*pend.pop(0))
                    prev = (h, av, se)
                wkh_cur, wvh_cur = wkh_nxt, wvh_nxt
            normalize(*prev)

        # ==== phase D: o_proj + residual + ln2 (h1 resident in SBUF) ====
        with contextlib.ExitStack() as sc45:
            x2m = sc45.enter_context(tc.tile_pool(name="x2m", bufs=1))
            x2 = x2m.tile([P, KI_H, QR], dt.bfloat16)
            h1sb = x2m.tile([P, KI_H, QR], dt.bfloat16)
            msb = x2m.tile([P, NF_FF, QR], dt.bfloat16)
            p5 = sc45.enter_context(tc.tile_pool(name="p5", bufs=1))
            ffn_cache = {}

            def ffn_tile(tag, src, nf):
                key = (tag, nf)
                if key not in ffn_cache:
                    tt = p5.tile([P, KI_H, P], dt.bfloat16, tag=tag, bufs=3)
                    nc.sync.dma_start(tt[:], src[nf])
                    ffn_cache[key] = tt
                return ffn_cache.pop(key)

            ffn_cache[("wg", 0)] = p5.tile([P, KI_H, P], dt.bfloat16, tag="wg", bufs=3, name="wg_p")
            nc.sync.dma_start(ffn_cache[("wg", 0)][:], w_g[0])
            ffn_cache[("wu", 0)] = p5.tile([P, KI_H, P], dt.bfloat16, tag="wu", bufs=3, name="wu_p")
            nc.sync.dma_start(ffn_cache[("wu", 0)][:], w_u[0])


            with tc.tile_pool(name="psD", bufs=1, space="PSUM") as psD:
                oacc = psD.tile([P, QR], dt.float32, tag="acc")
                for nf in range(KI_H):
                    wt = wo_tile(nf)
                    if nf + 1 < KI_H and nf + 1 not in wo_cache:
                        wo_cache[nf + 1] = wo_pool.tile([P, NH, VHD], dt.bfloat16, tag="wo", bufs=2, name="wo_p")
                        nc.sync.dma_start(wo_cache[nf + 1][:], w_o[nf + 1])
                    pt = ps.tile([P, QR], dt.float32, tag="mm")
                    for kh in range(NH):
                        nc.tensor.matmul(
                            pt[:],
                            wt[:, kh, :],
                            attn[:, kh, :],
                            start=(kh == 0),
                            stop=(kh == NH - 1),
                        )
                    ht = ld.tile([P, QR], dt.float32, tag="hload")
                    nc.sync.dma_start(ht[:], hTq[nf * P : (nf + 1) * P, :])
                    nc.vector.tensor_tensor(h1sb[:, nf, :], pt[:], ht[:], ADD)
                    sq = tmp.tile([P, QR], dt.bfloat16, tag="sq")
                    nc.vector.tensor_tensor(sq[:], h1sb[:, nf, :], h1sb[:, nf, :], MUL)
                    nc.tensor.matmul(
                        oacc[:], ones_bfP[:], sq[:], start=(nf == 0), stop=(nf == KI_H - 1)
                    )
                s2rep = tmp.tile([P, QR], dt.float32, tag="s1r", bufs=2)
                rsqrt_into(oacc, H, s2rep)
                for nf in range(KI_H):
                    nc.vector.tensor_tensor(x2[:, nf, :], h1sb[:, nf, :], s2rep[:], MUL)

            # ==== phase E: FFN (SwiGLU), bf16 ====
            for nf in range(NF_FF):
                wtg = ffn_tile("wg", w_g, nf)
                if nf + 1 < NF_FF:
                    ffn_cache[("wg", nf + 1)] = p5.tile([P, KI_H, P], dt.bfloat16, tag="wg", bufs=3, name="wg_p")
                    nc.sync.dma_start(ffn_cache[("wg", nf + 1)][:], w_g[nf + 1])
                pg = ps.tile([P, QR], dt.float32, tag="mm")
                for ki in range(KI_H):
                    nc.tensor.matmul(
                        pg[:], wtg[:, ki, :], x2[:, ki, :],
                        start=(ki == 0), stop=(ki == KI_H - 1),
                    )
                gs = tmp.tile([P, QR], dt.bfloat16, tag="sq")
                nc.scalar.activation(out=gs[:], in_=pg[:], func=AF.Silu)
                wtu = ffn_tile("wu", w_u, nf)
                if nf + 1 < NF_FF:
                    ffn_cache[("wu", nf + 1)] = p5.tile([P, KI_H, P], dt.bfloat16, tag="wu", bufs=3, name="wu_p")
                    nc.sync.dma_start(ffn_cache[("wu", nf + 1)][:], w_u[nf + 1])
                pu = ps.tile([P, QR], dt.float32, tag="mm")
                for ki in range(KI_H):
                    nc.tensor.matmul(
                        pu[:], wtu[:, ki, :], x2[:, ki, :],
                        start=(ki == 0), stop=(ki == KI_H - 1),
                    )
                nc.vector.tensor_tensor(msb[:, nf, :], pu[:], gs[:], MUL)

            for nf in range(KI_H):
                pt = ps.tile([P, QR], dt.float32, tag="mm")
                for qtr in range(4):
                    wt = p5.tile([P, NF_FF // 4, P], dt.bfloat16, tag="wd", bufs=3)
                    nc.sync.dma_start(wt[:], w_d[nf, :, qtr * 16 : (qtr + 1) * 16, :])
                    for ki in range(NF_FF // 4):
                        kk = qtr * 16 + ki
                        nc.tensor.matmul(
                            pt[:], wt[:, ki, :], msb[:, kk, :],
                            start=(kk == 0), stop=(kk == NF_FF - 1),
                        )
                ot = tmp.tile([P, QR], dt.float32, tag="h1t", bufs=2)
                nc.vector.tensor_tensor(ot[:], pt[:], h1sb[:, nf, :], ADD)
                nc.sync.dma_start(out[nf * P : (nf + 1) * P, :], ot[:])

    return nc


# ---------------------------------------------------------------------------
# host-side packing
# ---------------------------------------------------------------------------
def _deint_perm():
    # deinterleave: out[i] = in[2i] (i<32), in[2(i-32)+1] (i>=32)
    return np.concatenate([np.arange(0, ROPE, 2), np.arange(1, ROPE, 2)])


def _pack_lhst(w, nki, nnf, nfree=P, dtype=BF16, scale=1.0):
    # w [nki*P, nnf*nfree] -> [nnf, P, nki, nfree]
    return np.ascontiguousarray(
        (w.reshape(nki, P, nnf, nfree) * scale).transpose(2, 1, 0, 3).astype(dtype)
    )


def _prep_shared(inputs):
    perm = _deint_perm()
    ln1 = inputs["ln1_w"].astype(np.float32)
    qaln = inputs["q_a_ln_w"].astype(np.float32)
    kvln = inputs["kv_a_ln_w"].astype(np.float32)
    ln2 = inputs["ln2_w"].astype(np.float32)

    w_qa = inputs["q_a_kernel"].astype(np.float32) * ln1[:, None]
    w_kva = inputs["kv_a_kernel"].astype(np.float32) * ln1[:, None]
    w_kva = w_kva.copy()
    w_kva[:, KV_LORA:] = w_kva[:, KV_LORA:][:, perm]
    w_qb = inputs["q_b_kernel"].astype(np.float32) * qaln[:, None]
    w_qb = w_qb.copy()
    for h in range(NH):
        blk = slice(h * QHD + NOPE, (h + 1) * QHD)
        w_qb[:, blk] = w_qb[:, blk][:, perm]
    w_kvb = inputs["kv_b_kernel"].astype(np.float32) * kvln[:, None]
    w_o = inputs["o_kernel"].astype(np.float32)
    w_g = inputs["gate_kernel"].astype(np.float32) * ln2[:, None]
    w_u = inputs["up_kernel"].astype(np.float32) * ln2[:, None]
    w_d = inputs["down_kernel"].astype(np.float32)

    shared = {
        "w_qa": _pack_lhst(w_qa, KI_H, KI_QL, P, F8, WS),
        # w_qb: [NH, P, KI_QL, QHD] fp8 xWS
        "w_qb": np.ascontiguousarray(
            (w_qb.reshape(KI_QL, P, NH, QHD) * WS).transpose(2, 1, 0, 3).astype(F8)
        ),
        # w_kva resident: [P, KI_H, 576]
        "w_kva": np.ascontiguousarray(
            (w_kva.reshape(KI_H, P, KV_LORA + ROPE) * WS).transpose(1, 0, 2).astype(F8)
        ),
        # w_kvb split into k/v halves, packed per head-group of 4:
        "w_kv_k": np.ascontiguousarray(
            w_kvb.reshape(KI_KVL, P, NH // 4, 4, 2, 128)[:, :, :, :, 0, :]
            .transpose(2, 1, 0, 3, 4)
            .reshape(NH // 4, P, KI_KVL, 512)
            .astype(BF16)
        ),
        "w_kv_v": np.ascontiguousarray(
            w_kvb.reshape(KI_KVL, P, NH // 4, 4, 2, 128)[:, :, :, :, 1, :]
            .transpose(2, 1, 0, 3, 4)
            .reshape(NH // 4, P, KI_KVL, 512)
            .astype(BF16)
        ),
        # w_o: [KI_H(nf), P, NH, VHD]
        "w_o": np.ascontiguousarray(
            w_o.reshape(NH, VHD, KI_H, P).transpose(2, 1, 0, 3).astype(BF16)
        ),
        "w_g": _pack_lhst(w_g, KI_H, NF_FF),
        "w_u": _pack_lhst(w_u, KI_H, NF_FF),
        "w_d": _pack_lhst(w_d, NF_FF, KI_H),
    }
    return shared


def _prep_batch(inputs, b):
    hid = np.asarray(inputs["hidden_states"][b], dtype=np.float32)  # [S, H]
    hT = np.ascontiguousarray(hid.T)  # [H, S] f32
    hT8 = hT.astype(F8)
    pos = np.asarray(inputs["position_ids"][b]).astype(np.int64)
    cos_g = np.asarray(inputs["cos"], dtype=np.float32)[pos][:, :32]  # [S, 32]
    sin_g = np.asarray(inputs["sin"], dtype=np.float32)[pos][:, :32]
    return hT, hT8, np.ascontiguousarray(cos_g.T), np.ascontiguousarray(sin_g.T)


def _core_rows(j):
    return np.concatenate(
        [np.arange((j + 4 * i) * P, (j + 4 * i + 1) * P) for i in range(4)]
    )


def _core_masks(j):
    # one 128-col mask block per key tile, applied at query slot kt//4
    kp = np.arange(P)[:, None]
    qf = np.arange(P)[None, :]
    m = np.zeros((P, TK, P), dtype=F8)
    for kt in range(TK):
        r = kt % 4
        if j > r:
            m[:, kt, :] = 1.0
        elif j == r:
            m[:, kt, :] = (kp <= qf).astype(F8)
        # j < r: stays zero
    return m


def kernel(**inputs) -> np.ndarray:
    import concourse.bass as bass  # noqa: F401  (env check)
    from concourse.bass_utils import run_bass_kernel_spmd

    if "nc" not in _COMPILED:
        _COMPILED["nc"] = _build_nc()
    nc = _COMPILED["nc"]

    shared = _prep_shared(inputs)
    in_maps = []
    per_batch = [_prep_batch(inputs, b) for b in range(B)]
    for c in range(8):
        b, j = c // 4, c % 4
        hT, hT8, cosTb, sinTb = per_batch[b]
        rows = _core_rows(j)
        in_map = dict(shared)
        in_map["hTb"] = hT8
        in_map["hTqb"] = np.ascontiguousarray(hT8[:, rows])
        in_map["hTq"] = np.ascontiguousarray(hT[:, rows])
        in_map["cosT"] = cosTb
        in_map["sinT"] = sinTb
        in_map["cosTq"] = np.ascontiguousarray(cosTb[:, rows])
        in_map["sinTq"] = np.ascontiguousarray(sinTb[:, rows])
        in_map["masks"] = _core_masks(j)
        in_maps.append(in_map)

    res = run_bass_kernel_spmd(nc, in_maps, core_ids=list(range(8)))
    globals()["LAST_RESULT"] = res

    out = np.empty((B, S, H), dtype=np.float32)
    for c in range(8):
        b, j = c // 4, c % 4
        out[b, _core_rows(j), :] = res.results[c]["out"].T
    return out


# revision 27
# speedup vs baseline: 1.0688x; 1.0008x over previous
"""DeepseekV2 decoder layer (MLA attention + SwiGLU MLP) on 8 TRN2 NeuronCores.

Sharding: core c -> batch b = c//4, query tiles {j, j+4, j+8, j+12} (j = c%4,
128 rows each, causally interleaved). The interleave makes the causal
structure identical on every core: key tile kt is only needed by query slots
s >= kt//4, so the SPMD program skips 37.5% of score/AV work uniformly.

fp8 (e4m3, DoubleRow 2x matmul) is used for the hidden-state input, the
q_a/kv_a projections, the packed q/k score tiles, v, and the attention
probabilities; q_b/kv_b, o_proj and the FFN stay bf16 (fp8 there blows the
2e-2 error budget). fp8 weights are scaled x16 on the host to escape
denormals; the 1/16 compensation is folded into the rsqrt stat scales
(ws64 variants), so no extra ops are spent.

Per-head packed layout for scores: k/q tiles [128, 2, n] where pair slot 0 =
nope (128 feats), slot 1 = rope (64 feats) + 64 zero rows, so one DoubleRow
matmul computes a whole 192-dim score tile.
"""

import json

import numpy as np
import ml_dtypes

B, S, H = 2, 2048, 2048
NH = 16
Q_LORA = 1536
KV_LORA = 512
NOPE = 128
ROPE = 64
QHD = NOPE + ROPE  # 192
VHD = 128
FF = 8192
EPS = 1e-6
P = 128
QR = 512  # query rows per core
TK = S // P  # 16 key tiles
KI_H = H // P  # 16
KI_QL = Q_LORA // P  # 12
KI_KVL = KV_LORA // P  # 4
NF_FF = FF // P  # 64
ATTN_SCALE = QHD ** -0.5
WS = 16.0  # fp8 weight scale (16x: escapes denormals; keeps 16x-scaled k_pe/v tails far below fp8's 240 max)

BF16 = ml_dtypes.bfloat16
F8 = ml_dtypes.float8_e4m3

_COMPILED = {}


# ---------------------------------------------------------------------------
# compiler workaround: this container's walrus rejects >1 sem wait per
# instruction; split extra waits onto single-wait NoOps.
# ---------------------------------------------------------------------------
def _install_multiwait_fix(bass):
    if getattr(bass.Bass, "_multiwait_fix_installed", False):
        return
    orig = bass.Bass.to_json_bytes

    def _split(m):
        for f in m.get("functions", []):
            for b in f.get("blocks", []):
                out = []
                for inst in b.get("instructions", []):
                    si = inst.get("sync_info") or {}
                    waits = si.get("on_wait") or []
                    if len(waits) > 1:
                        for k, w in enumerate(waits[:-1]):
                            out.append(
                                {
                                    "debug": inst.get("debug", 0),
                                    "engine": inst["engine"],
                                    "ins": [],
                                    "name": f"{inst['name']}_w{k}",
                                    "opcode": "NoOp",
                                    "outs": [],
                                    "sync_info": {"on_update": [], "on_wait": [w]},
                                }
                            )
                        si["on_wait"] = [waits[-1]]
                    out.append(inst)
                b["instructions"] = out
        return m

    def patched(self):
        raw = orig(self)
        try:
            return json.dumps(_split(json.loads(raw))).encode()
        except Exception:
            return raw

    bass.Bass.to_json_bytes = patched
    bass.Bass._multiwait_fix_installed = True


def _install_drain_fix(tile, ScopedClock, VectorClock):
    if getattr(tile.TileContext, "_drain_fix_installed", False):
        return

    def _drain_and_barrier(self, tick_clock, wait_clock):
        gc = tick_clock.global_clock
        n = len(gc)
        for p in range(n):
            t = gc[p]
            if t > 0:
                vc = VectorClock([0] * n)
                vc.require_at_least(p, t)
                d = self.nc.sync.drain()
                wait_clock.add_sem_waits(d.ins, ScopedClock({None: vc}))
        self.nc.all_engine_barrier()
        popped = self.nc._tile_sem_poison_stack.pop()
        assert popped is self._sem_poison
        self.nc.clear_and_free_semaphores(list(self.sems.allocated().values()))
        self.nc.all_engine_barrier()

    tile.TileContext._drain_and_barrier = _drain_and_barrier
    tile.TileContext._drain_fix_installed = True


# ---------------------------------------------------------------------------
# device program
# ---------------------------------------------------------------------------
def _build_nc():
    import concourse.bass as bass
    import concourse.mybir as mybir
    import concourse.tile as tile
    from concourse.vector_clock import ScopedClock, VectorClock

    _install_multiwait_fix(bass)
    _install_drain_fix(tile, ScopedClock, VectorClock)

    dt = mybir.dt
    AF = mybir.ActivationFunctionType
    MUL = mybir.AluOpType.mult
    ADD = mybir.AluOpType.add
    SUB = mybir.AluOpType.subtract
    DRM = mybir.MatmulPerfMode.DoubleRow

    nc = bass.Bass()

    # register EPS so float bias=EPS works on the scalar engine
    _eps_t = nc.alloc_sbuf_tensor(f"const-float32-{EPS}", [128, 1], dt.float32)
    nc.gpsimd.memset(_eps_t.ap(), EPS)
    nc.const_aps.aps[(dt.float32, EPS)] = _eps_t.ap()
    EPS64 = EPS * WS * WS
    _eps64_t = nc.alloc_sbuf_tensor(f"const-float32-{EPS64}", [128, 1], dt.float32)
    nc.gpsimd.memset(_eps64_t.ap(), EPS64)
    nc.const_aps.aps[(dt.float32, EPS64)] = _eps64_t.ap()
    import math
    NLNWS = -math.log(WS)
    _nlnws_t = nc.alloc_sbuf_tensor(f"const-float32-{NLNWS}", [128, 1], dt.float32)
    nc.gpsimd.memset(_nlnws_t.ap(), NLNWS)
    nc.const_aps.aps[(dt.float32, NLNWS)] = _nlnws_t.ap()
    nc.all_engine_barrier()

    # ---- inputs ----
    hTb = nc.dram_tensor("hTb", [H, S], dt.float8e4, kind="ExternalInput")
    hTqb = nc.dram_tensor("hTqb", [H, QR], dt.float8e4, kind="ExternalInput")
    hTq = nc.dram_tensor("hTq", [H, QR], dt.float32, kind="ExternalInput")
    cosT = nc.dram_tensor("cosT", [32, S], dt.float32, kind="ExternalInput")
    sinT = nc.dram_tensor("sinT", [32, S], dt.float32, kind="ExternalInput")
    cosTq = nc.dram_tensor("cosTq", [32, QR], dt.float32, kind="ExternalInput")
    sinTq = nc.dram_tensor("sinTq", [32, QR], dt.float32, kind="ExternalInput")
    masks = nc.dram_tensor("masks", [P, TK, P], dt.float8e4, kind="ExternalInput")
    w_qa = nc.dram_tensor("w_qa", [KI_QL, P, KI_H, P], dt.float8e4, kind="ExternalInput")
    w_qb = nc.dram_tensor("w_qb", [NH, P, KI_QL, QHD], dt.float8e4, kind="ExternalInput")
    w_kva = nc.dram_tensor("w_kva", [P, KI_H, KV_LORA + ROPE], dt.float8e4, kind="ExternalInput")
    w_kv_k = nc.dram_tensor("w_kv_k", [NH // 4, P, KI_KVL, 512], dt.bfloat16, kind="ExternalInput")
    w_kv_v = nc.dram_tensor("w_kv_v", [NH // 4, P, KI_KVL, 512], dt.bfloat16, kind="ExternalInput")
    w_o = nc.dram_tensor("w_o", [KI_H, P, NH, VHD], dt.bfloat16, kind="ExternalInput")
    w_g = nc.dram_tensor("w_g", [NF_FF, P, KI_H, P], dt.bfloat16, kind="ExternalInput")
    w_u = nc.dram_tensor("w_u", [NF_FF, P, KI_H, P], dt.bfloat16, kind="ExternalInput")
    w_d = nc.dram_tensor("w_d", [KI_H, P, NF_FF, P], dt.bfloat16, kind="ExternalInput")
    out = nc.dram_tensor("out", [H, QR], dt.float32, kind="ExternalOutput")

    import contextlib

    with tile.TileContext(nc) as tc, contextlib.ExitStack() as top:
        tp = lambda **kw: top.enter_context(tc.tile_pool(**kw))
        ones = tp(name="ones", bufs=1)
        tmp = tp(name="tmp", bufs=3)
        ld = tp(name="ld", bufs=3)
        ps = tp(name="ps", bufs=3, space="PSUM")
        attn_pool = tp(name="attn_pool", bufs=1)
        attn = attn_pool.tile([P, NH, QR], dt.bfloat16)
        maskp = tp(name="maskp", bufs=1)
        maskt = maskp.tile([P, TK, P], dt.float8e4)
        wo_pool = tp(name="wo_pool", bufs=2)

        ones_bfP = ones.tile([P, P], dt.bfloat16)
        nc.vector.memset(ones_bfP[:], 1.0)
        ones8 = ones.tile([P, 2, P], dt.float8e4)
        nc.vector.memset(ones8[:], 1.0)

        def dr(o, l, r, start, stop):
            nc.tensor.matmul(o, l, r, start=start, stop=stop, perf_mode=DRM)

        def rsqrt_into(acc, denom, dst, ws64=False):
            # dst = 1/sqrt(mean + eps) (optionally a further 1/WS) computed as
            # exp(-0.5*ln(.)) -- two scalar-engine table ops, avoiding the
            # 3.3us DVE reciprocal that would serialize the vector queue.
            lnt = tmp.tile([P, acc.shape[-1]], dt.float32, tag="h1t", bufs=2)
            nc.scalar.activation(
                out=lnt[:], in_=acc[:], func=AF.Ln, bias=EPS, scale=1.0 / denom
            )
            if ws64:
                nc.scalar.activation(out=dst[:], in_=lnt[:], func=AF.Exp, scale=-0.5, bias=NLNWS)
            else:
                nc.scalar.activation(out=dst[:], in_=lnt[:], func=AF.Exp, scale=-0.5)

        def rsqrt_tmp(acc, denom, tag, ws64=False):
            d = tmp.tile([P, acc.shape[-1]], dt.float32, tag=tag, bufs=2)
            rsqrt_into(acc, denom, d, ws64)
            return d

        with contextlib.ExitStack() as mid:
            lat = mid.enter_context(tc.tile_pool(name="lat", bufs=1))
            ckv = lat.tile([P, KI_KVL, S], dt.bfloat16)  # normalized kv latents (1x, bf16)
            kpe = lat.tile([ROPE, S], dt.float8e4)  # roped shared key-pe (1x)
            pA = mid.enter_context(tc.tile_pool(name="pA", bufs=1))
            xqbf = pA.tile([P, KI_H, QR], dt.float8e4)
            s1qrep64 = pA.tile([P, QR], dt.float32)
            qpk_pool = mid.enter_context(tc.tile_pool(name="qpk", bufs=1))
            qpk = qpk_pool.tile([P, NH, 2, QR], dt.float8e4)
            wq_pool = mid.enter_context(tc.tile_pool(name="wq", bufs=1))

            wq_cache = {}

            def wqa_tile(nf):
                if nf not in wq_cache:
                    t = wq_pool.tile([P, KI_H, P], dt.float8e4, tag="wqa", bufs=2)
                    nc.sync.dma_start(t[:], w_qa[nf])
                    wq_cache[nf] = t
                return wq_cache.pop(nf)

            # ==== phase A: ln1 stats + kv latents + shared roped k_pe ====
            with tc.tile_pool(name="pB", bufs=1) as pB, tc.tile_pool(
                name="psA", bufs=2, space="PSUM"
            ) as psA:
                wkva = pB.tile([P, KI_H, KV_LORA + ROPE], dt.float8e4)
                cosb = pB.tile([32, S], dt.float32)
                sinb = pB.tile([32, S], dt.float32)

                # chunk pipeline: squares for chunk t+1 run on the SCALAR
                # engine while the PE does chunk t's kv matmuls and the DVE
                # does chunk t's PSUM drains; acc DRs for t+1 are emitted after
                # kv work so the PE never waits on the square/stat chain.
                xcs = {}
                s1s = {}
                sq_tiles = {}

                def xc_dma(t):
                    tsl = slice(t * 512, (t + 1) * 512)
                    xc = pB.tile([P, KI_H, 512], dt.float8e4, tag="xc", bufs=3, name="xc")
                    for ki in range(KI_H):
                        nc.sync.dma_start(xc[:, ki, :], hTb[ki * P : (ki + 1) * P, tsl])
                    xcs[t] = xc

                def sq_emit(t):
                    xc = xcs[t]
                    tiles = []
                    for kp_ in range(KI_H // 2):
                        sqp = pB.tile([P, 2, 512], dt.float8e4, tag="sqp", bufs=8, name="sqp")
                        nc.scalar.activation(
                            out=sqp[:], in_=xc[:, 2 * kp_ : 2 * kp_ + 2, :], func=AF.Square
                        )
                        tiles.append(sqp)
                    sq_tiles[t] = tiles

                def acc_emit(t):
                    tiles = sq_tiles.pop(t)
                    acc = psA.tile([P, 512], dt.float32, tag="acc", name="acc")
                    for kp_, sqp in enumerate(tiles):
                        dr(acc[:], ones8[:], sqp[:], kp_ == 0, kp_ == KI_H // 2 - 1)
                    s1s[t] = rsqrt_tmp(acc, H, "s1rb", ws64=True)

                xc_dma(0)
                for ki in range(KI_H):
                    nc.sync.dma_start(xqbf[:, ki, :], hTqb[ki * P : (ki + 1) * P, :])
                nc.sync.dma_start(wkva[:], w_kva[:])
                sq_emit(0)
                xc_dma(1)
                nc.sync.dma_start(cosb[:], cosT[:])
                nc.sync.dma_start(sinb[:], sinT[:])
                nc.sync.dma_start(maskt[:], masks[:])
                # prefetch first q_a weights for phase B
                wqa_pre = wq_pool.tile([P, KI_H, P], dt.float8e4, tag="wqa", bufs=2)
                nc.sync.dma_start(wqa_pre[:], w_qa[0])
                wq_cache[0] = wqa_pre

                # q-slice ln1 stats on the DVE, overlapping the scalar squares
                accq = psA.tile([P, QR], dt.float32, tag="acc")
                for kp_ in range(KI_H // 2):
                    sqq = pB.tile([P, 2, QR], dt.float8e4, tag="sqq", bufs=2)
                    for i in (0, 1):
                        ki = 2 * kp_ + i
                        nc.vector.tensor_tensor(sqq[:, i, :], xqbf[:, ki, :], xqbf[:, ki, :], MUL)
                    dr(accq[:], ones8[:], sqq[:], kp_ == 0, kp_ == KI_H // 2 - 1)
                rsqrt_into(accq, H, s1qrep64, ws64=True)
                acc_emit(0)

                for t in range(S // 512):
                    tsl = slice(t * 512, (t + 1) * 512)
                    if t + 2 < S // 512:
                        xc_dma(t + 2)
                    if t + 1 < S // 512:
                        sq_emit(t + 1)
                    xc = xcs.pop(t)
                    s1rep64 = s1s.pop(t)
                    ckvt = pB.tile([P, KI_KVL, 512], dt.bfloat16, tag="ckvt", bufs=2)
                    kvacc = psA.tile([P, 512], dt.float32, tag="acc2")
                    sqp2s = []
                    sqp2 = None
                    for nf in range(KI_KVL):
                        pt = ps.tile([P, 512], dt.float32, tag="mm")
                        for kp_ in range(KI_H // 2):
                            dr(
                                pt[:],
                                wkva[:, 2 * kp_ : 2 * kp_ + 2, nf * P : (nf + 1) * P],
                                xc[:, 2 * kp_ : 2 * kp_ + 2, :],
                                kp_ == 0,
                                kp_ == KI_H // 2 - 1,
                            )
                        nc.vector.tensor_tensor(ckvt[:, nf, :], pt[:], s1rep64[:], MUL)
                        if nf % 2 == 0:
                            sqp2 = pB.tile([P, 2, 512], dt.float8e4, tag="sqp2", bufs=2)
                            sqp2s.append(sqp2)
                        nc.vector.tensor_tensor(
                            sqp2[:, nf % 2, :], ckvt[:, nf, :], ckvt[:, nf, :], MUL
                        )
                    # k_pe: last 64 cols of w_kva (1x true scale via s1rep64)
                    pt = ps.tile([ROPE, 512], dt.float32, tag="mm")
                    for kp_ in range(KI_H // 2):
                        dr(
                            pt[:],
                            wkva[:, 2 * kp_ : 2 * kp_ + 2, KV_LORA : KV_LORA + ROPE],
                            xc[:, 2 * kp_ : 2 * kp_ + 2, :],
                            kp_ == 0,
                            kp_ == KI_H // 2 - 1,
                        )
                    for q2, sqp2 in enumerate(sqp2s):
                        dr(kvacc[:], ones8[:], sqp2[:], q2 == 0, q2 == KI_KVL // 2 - 1)
                    if t + 1 < S // 512:
                        acc_emit(t + 1)
                    pes = tmp.tile([ROPE, 512], dt.float32, tag="pes", bufs=2)
                    nc.vector.tensor_tensor(pes[:], pt[:], s1rep64[:ROPE, :], MUL)
                    x2h = tmp.tile([32, 512], dt.float32, tag="x2h", bufs=2)
                    nc.sync.dma_start(x2h[:], pes[32:, :])
                    t1 = tmp.tile([32, 512], dt.float32, tag="t1", bufs=2)
                    t2 = tmp.tile([32, 512], dt.float32, tag="t2", bufs=2)
                    o2 = tmp.tile([32, 512], dt.float8e4, tag="o2", bufs=2)
                    nc.vector.tensor_tensor(t1[:], pes[:32, :], cosb[:, tsl], MUL)
                    nc.vector.tensor_tensor(t2[:], x2h[:], sinb[:, tsl], MUL)
                    nc.vector.tensor_tensor(kpe[:32, tsl], t1[:], t2[:], SUB)
                    nc.vector.tensor_tensor(t1[:], x2h[:], cosb[:, tsl], MUL)
                    nc.vector.tensor_tensor(t2[:], pes[:32, :], sinb[:, tsl], MUL)
                    nc.vector.tensor_tensor(o2[:], t1[:], t2[:], ADD)
                    nc.sync.dma_start(kpe[32:, tsl], o2[:])
                    # kv_a rmsnorm rescale -> bf16 latents
                    rkv = rsqrt_tmp(kvacc, KV_LORA, "s1rc")
                    for nf in range(KI_KVL):
                        nc.vector.tensor_tensor(ckv[:, nf, tsl], ckvt[:, nf, :], rkv[:], MUL)

            # ==== attention k/v weight pool created early so hg=0 prefetches ====
            p3 = mid.enter_context(tc.tile_pool(name="p3", bufs=1))
            psC = mid.enter_context(tc.tile_pool(name="psC", bufs=2, space="PSUM"))
            ksb0 = p3.tile([P, 2, S], dt.float8e4, tag="ksb0", bufs=1)
            ksb1 = p3.tile([P, 2, S], dt.float8e4, tag="ksb1", bufs=1)
            ksbs = [ksb0, ksb1]
            for i in range(2):
                nc.vector.memset(ksbs[i][64:, 1, :], 0.0)
            kv_cache = {}

            def kvw_tile(tag, src, hg):
                key = (tag, hg)
                if key not in kv_cache:
                    tt = p3.tile([P, KI_KVL, 512], dt.bfloat16, tag=tag, bufs=2)
                    nc.sync.dma_start(tt[:], src[hg])
                    kv_cache[key] = tt
                return kv_cache.pop(key)

            kv_cache[("wkh", 0)] = p3.tile([P, KI_KVL, 512], dt.bfloat16, tag="wkh", bufs=2, name="wkh_p")
            nc.sync.dma_start(kv_cache[("wkh", 0)][:], w_kv_k[0])
            kv_cache[("wvh", 0)] = p3.tile([P, KI_KVL, 512], dt.bfloat16, tag="wvh", bufs=2, name="wvh_p")
            nc.sync.dma_start(kv_cache[("wvh", 0)][:], w_kv_v[0])

            # ==== phase B: q path ====
            with tc.tile_pool(name="p2", bufs=1) as p2:
                qlat = p2.tile([P, KI_QL, QR], dt.float8e4)
                qacc = psC.tile([P, QR], dt.float32, tag="acc", bufs=1)
                sqp = None
                for nf in range(KI_QL):
                    wt = wqa_tile(nf)
                    if nf + 1 < KI_QL:
                        wqa_tile_pre = wq_pool.tile([P, KI_H, P], dt.float8e4, tag="wqa", bufs=2)
                        nc.sync.dma_start(wqa_tile_pre[:], w_qa[nf + 1])
                        wq_cache[nf + 1] = wqa_tile_pre
                    pt = ps.tile([P, QR], dt.float32, tag="mm")
                    for kp_ in range(KI_H // 2):
                        dr(
                            pt[:],
                            wt[:, 2 * kp_ : 2 * kp_ + 2, :],
                            xqbf[:, 2 * kp_ : 2 * kp_ + 2, :],
                            kp_ == 0,
                            kp_ == KI_H // 2 - 1,
                        )
                    nc.vector.tensor_tensor(qlat[:, nf, :], pt[:], s1qrep64[:], MUL)
                    if nf % 2 == 0:
                        sqp = tmp.tile([P, 2, QR], dt.float8e4, tag="sqb", bufs=2)
                    nc.vector.tensor_tensor(sqp[:, nf % 2, :], qlat[:, nf, :], qlat[:, nf, :], MUL)
                    if nf % 2 == 1:
                        dr(qacc[:], ones8[:], sqp[:], nf // 2 == 0, nf // 2 == KI_QL // 2 - 1)
                sqrep64 = p2.tile([P, QR], dt.float32)
                rsqrt_into(qacc, Q_LORA, sqrep64, ws64=True)

                # rope tables for q with q_a_ln scale (and 1/64) folded in
                cosq = p2.tile([32, QR], dt.float32)
                sinq = p2.tile([32, QR], dt.float32)
                nc.sync.dma_start(cosq[:], cosTq[:])
                nc.sync.dma_start(sinq[:], sinTq[:])
                nc.vector.tensor_tensor(cosq[:], cosq[:], sqrep64[:32, :], MUL)
                nc.vector.tensor_tensor(sinq[:], sinq[:], sqrep64[:32, :], MUL)

                for h in range(NH):
                    nc.vector.memset(qpk[64:, h, 1, :], 0.0)
                    wt = p2.tile([P, KI_QL, QHD], dt.float8e4, tag="wqb", bufs=2)
                    nc.sync.dma_start(wt[:], w_qb[h])
                    pt = ps.tile([P, QR], dt.float32, tag="mm")
                    for kp_ in range(KI_QL // 2):
                        dr(
                            pt[:],
                            wt[:, 2 * kp_ : 2 * kp_ + 2, :NOPE],
                            qlat[:, 2 * kp_ : 2 * kp_ + 2, :],
                            kp_ == 0,
                            kp_ == KI_QL // 2 - 1,
                        )
                    nc.vector.tensor_tensor(qpk[:, h, 0, :], pt[:], sqrep64[:], MUL)
                    pta = ps.tile([32, QR], dt.float32, tag="mm", name="pta")
                    for kp_ in range(KI_QL // 2):
                        dr(
                            pta[:],
                            wt[:, 2 * kp_ : 2 * kp_ + 2, NOPE : NOPE + 32],
                            qlat[:, 2 * kp_ : 2 * kp_ + 2, :],
                            kp_ == 0,
                            kp_ == KI_QL // 2 - 1,
                        )
                    ptb = ps.tile([32, QR], dt.float32, tag="mm", name="ptb")
                    for kp_ in range(KI_QL // 2):
                        dr(
                            ptb[:],
                            wt[:, 2 * kp_ : 2 * kp_ + 2, NOPE + 32 : QHD],
                            qlat[:, 2 * kp_ : 2 * kp_ + 2, :],
                            kp_ == 0,
                            kp_ == KI_QL // 2 - 1,
                        )
                    t1 = tmp.tile([32, QR], dt.float32, tag="t1", bufs=2)
                    t2 = tmp.tile([32, QR], dt.float32, tag="t2", bufs=2)
                    o2 = tmp.tile([32, QR], dt.float8e4, tag="o2", bufs=2)
                    nc.vector.tensor_tensor(t1[:], pta[:], cosq[:], MUL)
                    nc.vector.tensor_tensor(t2[:], ptb[:], sinq[:], MUL)
                    nc.vector.tensor_tensor(qpk[:32, h, 1, :], t1[:], t2[:], SUB)
                    nc.vector.tensor_tensor(t1[:], ptb[:], cosq[:], MUL)
                    nc.vector.tensor_tensor(t2[:], pta[:], sinq[:], MUL)
                    nc.vector.tensor_tensor(o2[:], t1[:], t2[:], ADD)
                    nc.sync.dma_start(qpk[32:64, h, 1, :], o2[:])

            # ==== phase C: attention ====
            wo_cache = {}

            def wo_tile(nf):
                if nf not in wo_cache:
                    tt = wo_pool.tile([P, NH, VHD], dt.bfloat16, tag="wo", bufs=2)
                    nc.sync.dma_start(tt[:], w_o[nf])
                    wo_cache[nf] = tt
                return wo_cache.pop(nf)

            def normalize(h, av_t, se_t):
                # 1/se as exp(-ln(se)) on the scalar engine; se is
                # pre-replicated [P, QR] by the all-ones se reduction.
                rc = tmp.tile([P, QR], dt.float32, tag="s1r", bufs=2)
                lnt = tmp.tile([P, QR], dt.float32, tag="h1t", bufs=2)
                nc.scalar.activation(out=lnt[:], in_=se_t[:], func=AF.Ln)
                nc.scalar.activation(out=rc[:], in_=lnt[:], func=AF.Exp, scale=-1.0)
                nc.vector.tensor_tensor(attn[:, h, :], av_t[:], rc[:], MUL)

            def k_expand(h, wkh_t):
                # expand k_nope for head h into its packed ksb tile, one head
                # ahead of the score loop so its DVE drains are off the
                # critical path.
                ksb_t = ksbs[h % 2]
                hh_ = h % 4
                for c in range(S // 512):
                    csl = slice(c * 512, (c + 1) * 512)
                    pt = ps.tile([P, 512], dt.float32, tag="mm", name="pt")
                    for l in range(KI_KVL):
                        nc.tensor.matmul(
                            pt[:],
                            wkh_t[:, l, hh_ * P : (hh_ + 1) * P],
                            ckv[:, l, csl],
                            start=(l == 0), stop=(l == KI_KVL - 1),
                        )
                    nc.vector.tensor_copy(ksb_t[:, 0, csl], pt[:])
                nc.vector.tensor_copy(ksb_t[:64, 1, :], kpe[:])

            prev = None
            wkh_cur = kvw_tile("wkh", w_kv_k, 0)
            wvh_cur = kvw_tile("wvh", w_kv_v, 0)
            wkh_nxt = wvh_nxt = None
            k_expand(0, wkh_cur)
            for hg in range(NH // 4):
                if hg + 1 < NH // 4:
                    wkh_nxt = kvw_tile("wkh", w_kv_k, hg + 1)
                    wvh_nxt = kvw_tile("wvh", w_kv_v, hg + 1)
                # v for 4 heads at once
                vsb = p3.tile([P, TK, 4 * VHD], dt.float8e4, tag="vsb", bufs=2)
                for kt in range(TK):
                    pt = ps.tile([P, 4 * VHD], dt.float32, tag="mm")
                    for l in range(KI_KVL):
                        nc.tensor.matmul(
                            pt[:],
                            ckv[:, l, kt * P : (kt + 1) * P],
                            wvh_cur[:, l, :],
                            start=(l == 0), stop=(l == KI_KVL - 1),
                        )
                    nc.vector.tensor_copy(vsb[:, kt, :], pt[:])
                for hh in range(4):
                    h = hg * 4 + hh
                    ksb = ksbs[h % 2]
                    if h + 1 < NH:
                        k_expand(h + 1, wkh_cur if hh < 3 else wkh_nxt)
                    if h == NH - 1:  # prefetch o_proj weights under last head
                        wo_cache[0] = wo_pool.tile([P, NH, VHD], dt.bfloat16, tag="wo", bufs=2, name="wo_p")
                        nc.sync.dma_start(wo_cache[0][:], w_o[0])
                        wo_cache[1] = wo_pool.tile([P, NH, VHD], dt.bfloat16, tag="wo", bufs=2, name="wo_p")
                        nc.sync.dma_start(wo_cache[1][:], w_o[1])
                    if prev is not None:
                        normalize(*prev)
                        prev = None
                    av = psC.tile([P, QR], dt.float32, tag="av")
                    se = psC.tile([P, QR], dt.float32, tag="se")

                    def _drain(p8_, prt, off):
                        dr(se[:, off:], ones8[:], prt[:, :, off:], p8_ == 0, p8_ == TK // 2 - 1)
                        dr(
                            av[:, off:],
                            vsb[:, 2 * p8_ : 2 * p8_ + 2, hh * VHD : (hh + 1) * VHD],
                            prt[:, :, off:],
                            p8_ == 0,
                            p8_ == TK // 2 - 1,
                        )

                    pend = []
                    for p8_ in range(TK // 2):
                        off = (p8_ // 2) * P
                        prt = tmp.tile([P, 2, QR], dt.float8e4, tag="pr", bufs=4)
                        for i in (0, 1):
                            kt = 2 * p8_ + i
                            sc = ps.tile([P, QR], dt.float32, tag="mm")
                            nc.tensor.matmul(
                                sc[:, off:],
                                ksb[:, :, kt * P : (kt + 1) * P],
                                qpk[:, h, :, off:],
                                start=True,
                                stop=True,
                                perf_mode=DRM,
                            )
                            nc.scalar.activation(
                                out=prt[:, i, off:], in_=sc[:, off:], func=AF.Exp,
                                scale=ATTN_SCALE,
                            )
                            nc.vector.tensor_tensor(
                                prt[:, i, off : off + P],
                                prt[:, i, off : off + P],
                                maskt[:, kt, :],
                                MUL,
                            )
                        pend.append((p8_, prt, off))
                        if len(pend) > 2:
                            _drain(*pend.pop(0))
                    while pend:
                        _drain(*pend.pop(0))
                    prev = (h, av, se)
                wkh_cur, wvh_cur = wkh_nxt, wvh_nxt
            normalize(*prev)

        # ==== phase D: o_proj + residual + ln2 (h1 resident in SBUF) ====
        with contextlib.ExitStack() as sc45:
            x2m = sc45.enter_context(tc.tile_pool(name="x2m", bufs=1))
            x2 = x2m.tile([P, KI_H, QR], dt.bfloat16)
            h1sb = x2m.tile([P, KI_H, QR], dt.bfloat16)
            msb = x2m.tile([P, NF_FF, QR], dt.bfloat16)
            p5 = sc45.enter_context(tc.tile_pool(name="p5", bufs=1))
            ffn_cache = {}

            def ffn_tile(tag, src, nf):
                key = (tag, nf)
                if key not in ffn_cache:
                    tt = p5.tile([P, KI_H, P], dt.bfloat16, tag=tag, bufs=3)
                    nc.sync.dma_start(tt[:], src[nf])
                    ffn_cache[key] = tt
                return ffn_cache.pop(key)

            ffn_cache[("wg", 0)] = p5.tile([P, KI_H, P], dt.bfloat16, tag="wg", bufs=3, name="wg_p")
            nc.sync.dma_start(ffn_cache[("wg", 0)][:], w_g[0])
            ffn_cache[("wu", 0)] = p5.tile([P, KI_H, P], dt.bfloat16, tag="wu", bufs=3, name="wu_p")
            nc.sync.dma_start(ffn_cache[("wu", 0)][:], w_u[0])


            with tc.tile_pool(name="psD", bufs=1, space="PSUM") as psD:
                oacc = psD.tile([P, QR], dt.float32, tag="acc")
                for nf in range(KI_H):
                    wt = wo_tile(nf)
                    if nf + 1 < KI_H and nf + 1 not in wo_cache:
                        wo_cache[nf + 1] = wo_pool.tile([P, NH, VHD], dt.bfloat16, tag="wo", bufs=2, name="wo_p")
                        nc.sync.dma_start(wo_cache[nf + 1][:], w_o[nf + 1])
                    pt = ps.tile([P, QR], dt.float32, tag="mm")
                    for kh in range(NH):
                        nc.tensor.matmul(
                            pt[:],
                            wt[:, kh, :],
                            attn[:, kh, :],
                            start=(kh == 0),
                            stop=(kh == NH - 1),
                        )
                    ht = ld.tile([P, QR], dt.float32, tag="hload")
                    nc.sync.dma_start(ht[:], hTq[nf * P : (nf + 1) * P, :])
                    nc.vector.tensor_tensor(h1sb[:, nf, :], pt[:], ht[:], ADD)
                    sq = tmp.tile([P, QR], dt.bfloat16, tag="sq")
                    nc.vector.tensor_tensor(sq[:], h1sb[:, nf, :], h1sb[:, nf, :], MUL)
                    nc.tensor.matmul(
                        oacc[:], ones_bfP[:], sq[:], start=(nf == 0), stop=(nf == KI_H - 1)
                    )
                s2rep = tmp.tile([P, QR], dt.float32, tag="s1r", bufs=2)
                rsqrt_into(oacc, H, s2rep)
                for nf in range(KI_H):
                    nc.vector.tensor_tensor(x2[:, nf, :], h1sb[:, nf, :], s2rep[:], MUL)

            # ==== phase E: FFN (SwiGLU), bf16 ====
            for nf in range(NF_FF):
                wtg = ffn_tile("wg", w_g, nf)
                if nf + 1 < NF_FF:
                    ffn_cache[("wg", nf + 1)] = p5.tile([P, KI_H, P], dt.bfloat16, tag="wg", bufs=3, name="wg_p")
                    nc.sync.dma_start(ffn_cache[("wg", nf + 1)][:], w_g[nf + 1])
                pg = ps.tile([P, QR], dt.float32, tag="mm")
                for ki in range(KI_H):
                    nc.tensor.matmul(
                        pg[:], wtg[:, ki, :], x2[:, ki, :],
                        start=(ki == 0), stop=(ki == KI_H - 1),
                    )
                gs = tmp.tile([P, QR], dt.bfloat16, tag="sq")
                nc.scalar.activation(out=gs[:], in_=pg[:], func=AF.Silu)
                wtu = ffn_tile("wu", w_u, nf)
                if nf + 1 < NF_FF:
                    ffn_cache[("wu", nf + 1)] = p5.tile([P, KI_H, P], dt.bfloat16, tag="wu", bufs=3, name="wu_p")
                    nc.sync.dma_start(ffn_cache[("wu", nf + 1)][:], w_u[nf + 1])
                pu = ps.tile([P, QR], dt.float32, tag="mm")
                for ki in range(KI_H):
                    nc.tensor.matmul(
                        pu[:], wtu[:, ki, :], x2[:, ki, :],
                        start=(ki == 0), stop=(ki == KI_H - 1),
                    )
                nc.vector.tensor_tensor(msb[:, nf, :], pu[:], gs[:], MUL)

            for nf in range(KI_H):
                pt = ps.tile([P, QR], dt.float32, tag="mm")
                for qtr in range(4):
                    wt = p5.tile([P, NF_FF // 4, P], dt.bfloat16, tag="wd", bufs=3)
                    nc.sync.dma_start(wt[:], w_d[nf, :, qtr * 16 : (qtr + 1) * 16, :])
                    for ki in range(NF_FF // 4):
                        kk = qtr * 16 + ki
                        nc.tensor.matmul(
                            pt[:], wt[:, ki, :], msb[:, kk, :],
                            start=(kk == 0), stop=(kk == NF_FF - 1),
                        )
                ot = tmp.tile([P, QR], dt.float32, tag="h1t", bufs=2)
                nc.vector.tensor_tensor(ot[:], pt[:], h1sb[:, nf, :], ADD)
                nc.sync.dma_start(out[nf * P : (nf + 1) * P, :], ot[:])

    return nc


# ---------------------------------------------------------------------------
# host-side packing
# ---------------------------------------------------------------------------
def _deint_perm():
    # deinterleave: out[i] = in[2i] (i<32), in[2(i-32)+1] (i>=32)
    return np.concatenate([np.arange(0, ROPE, 2), np.arange(1, ROPE, 2)])


def _pack_lhst(w, nki, nnf, nfree=P, dtype=BF16, scale=1.0):
    # w [nki*P, nnf*nfree] -> [nnf, P, nki, nfree]
    return np.ascontiguousarray(
        (w.reshape(nki, P, nnf, nfree) * scale).transpose(2, 1, 0, 3).astype(dtype)
    )


def _prep_shared(inputs):
    perm = _deint_perm()
    ln1 = inputs["ln1_w"].astype(np.float32)
    qaln = inputs["q_a_ln_w"].astype(np.float32)
    kvln = inputs["kv_a_ln_w"].astype(np.float32)
    ln2 = inputs["ln2_w"].astype(np.float32)

    w_qa = inputs["q_a_kernel"].astype(np.float32) * ln1[:, None]
    w_kva = inputs["kv_a_kernel"].astype(np.float32) * ln1[:, None]
    w_kva = w_kva.copy()
    w_kva[:, KV_LORA:] = w_kva[:, KV_LORA:][:, perm]
    w_qb = inputs["q_b_kernel"].astype(np.float32) * qaln[:, None]
    w_qb = w_qb.copy()
    for h in range(NH):
        blk = slice(h * QHD + NOPE, (h + 1) * QHD)
        w_qb[:, blk] = w_qb[:, blk][:, perm]
    w_kvb = inputs["kv_b_kernel"].astype(np.float32) * kvln[:, None]
    w_o = inputs["o_kernel"].astype(np.float32)
    w_g = inputs["gate_kernel"].astype(np.float32) * ln2[:, None]
    w_u = inputs["up_kernel"].astype(np.float32) * ln2[:, None]
    w_d = inputs["down_kernel"].astype(np.float32)

    shared = {
        "w_qa": _pack_lhst(w_qa, KI_H, KI_QL, P, F8, WS),
        # w_qb: [NH, P, KI_QL, QHD] fp8 xWS
        "w_qb": np.ascontiguousarray(
            (w_qb.reshape(KI_QL, P, NH, QHD) * WS).transpose(2, 1, 0, 3).astype(F8)
        ),
        # w_kva resident: [P, KI_H, 576]
        "w_kva": np.ascontiguousarray(
            (w_kva.reshape(KI_H, P, KV_LORA + ROPE) * WS).transpose(1, 0, 2).astype(F8)
        ),
        # w_kvb split into k/v halves, packed per head-group of 4:
        "w_kv_k": np.ascontiguousarray(
            w_kvb.reshape(KI_KVL, P, NH // 4, 4, 2, 128)[:, :, :, :, 0, :]
            .transpose(2, 1, 0, 3, 4)
            .reshape(NH // 4, P, KI_KVL, 512)
            .astype(BF16)
        ),
        "w_kv_v": np.ascontiguousarray(
            w_kvb.reshape(KI_KVL, P, NH // 4, 4, 2, 128)[:, :, :, :, 1, :]
            .transpose(2, 1, 0, 3, 4)
            .reshape(NH // 4, P, KI_KVL, 512)
            .astype(BF16)
        ),
        # w_o: [KI_H(nf), P, NH, VHD]
        "w_o": np.ascontiguousarray(
            w_o.reshape(NH, VHD, KI_H, P).transpose(2, 1, 0, 3).astype(BF16)
        ),
        "w_g": _pack_lhst(w_g, KI_H, NF_FF),
        "w_u": _pack_lhst(w_u, KI_H, NF_FF),
        "w_d": _pack_lhst(w_d, NF_FF, KI_H),
    }
    return shared


def _prep_batch(inputs, b):
    hid = np.asarray(inputs["hidden_states"][b], dtype=np.float32)  # [S, H]
    hT = np.ascontiguousarray(hid.T)  # [H, S] f32
    hT8 = hT.astype(F8)
    pos = np.asarray(inputs["position_ids"][b]).astype(np.int64)
    cos_g = np.asarray(inputs["cos"], dtype=np.float32)[pos][:, :32]  # [S, 32]
    sin_g = np.asarray(inputs["sin"], dtype=np.float32)[pos][:, :32]
    return hT, hT8, np.ascontiguousarray(cos_g.T), np.ascontiguousarray(sin_g.T)


def _core_rows(j):
    return np.concatenate(
        [np.arange((j + 4 * i) * P, (j + 4 * i + 1) * P) for i in range(4)]
    )


def _core_masks(j):
    # one 128-col mask block per key tile, applied at query slot kt//4
    kp = np.arange(P)[:, None]
    qf = np.arange(P)[None, :]
    m = np.zeros((P, TK, P), dtype=F8)
    for kt in range(TK):
        r = kt % 4
        if j > r:
            m[:, kt, :] = 1.0
        elif j == r:
            m[:, kt, :] = (kp <= qf).astype(F8)
        # j < r: stays zero
    return m


def kernel(**inputs) -> np.ndarray:
    import concourse.bass as bass  # noqa: F401  (env check)
    from concourse.bass_utils import run_bass_kernel_spmd

    if "nc" not in _COMPILED:
        _COMPILED["nc"] = _build_nc()
    nc = _COMPILED["nc"]

    shared = _prep_shared(inputs)
    in_maps = []
    per_batch = [_prep_batch(inputs, b) for b in range(B)]
    for c in range(8):
        b, j = c // 4, c % 4
        hT, hT8, cosTb, sinTb = per_batch[b]
        rows = _core_rows(j)
        in_map = dict(shared)
        in_map["hTb"] = hT8
        in_map["hTqb"] = np.ascontiguousarray(hT8[:, rows])
        in_map["hTq"] = np.ascontiguousarray(hT[:, rows])
        in_map["cosT"] = cosTb
        in_map["sinT"] = sinTb
        in_map["cosTq"] = np.ascontiguousarray(cosTb[:, rows])
        in_map["sinTq"] = np.ascontiguousarray(sinTb[:, rows])
        in_map["masks"] = _core_masks(j)
        in_maps.append(in_map)

    res = run_bass_kernel_spmd(nc, in_maps, core_ids=list(range(8)))
    globals()["LAST_RESULT"] = res

    out = np.empty((B, S, H), dtype=np.float32)
    for c in range(8):
        b, j = c // 4, c % 4
        out[b, _core_rows(j), :] = res.results[c]["out"].T
    return out
